# revision 12
# baseline (speedup 1.0000x reference)
"""Trainium2 Bass kernel for nn_EnergyAE (B=64, D=12288, N=32, H=2048) on 8 cores.

v2 restructure (vs v1's contraction-sharded C + 16.8MB AllReduce):
  - C block-row sharding: core k computes C_k = C[hs_k, :] (256 x 2048) locally
    by streaming full W2^T (bf16, 12.6MB); C_k stays in SBUF. No C collective,
    no C DRAM round trip.
  - Per-core H-permutation (host-side) puts each core's block at H-cols 0:256,
    so the SPMD program is core-index-free.
  - G[b] computed as partial sums over ALL 64 samples using C_k; AllReduce of
    packed G (256KB). Every core then factors all 64 (same vector cost) ->
    no z_s AllGather, no sel8 gathers.
  - x_star never materialized: W2 delta = W2(x-b2) - C h2, so
    u = xmb @ W2s^T (partial), v = h2[:, 0:256] @ C_k (partial),
    wd_k = u_k - v_k, t_k = A2 wd_k (pre-AllReduce!),
    d_sq = |xmb|^2 - 2 u.h2 + v.h2 (scalar partials).
  - Final AllReduce packs [t | svec | G2] = 271KB. All cores compute all 64
    outputs; host takes core 0's.

Identities (validated numerically, rel err ~1.7e-3 with bf16 C/A/u):
  Prec = Lt D Lt^T (unit-lower LDLT)
  sum(log eig)/2 = 0.5*sum(log D);  sum(1/eig) = ||D^-1/2 Lt^-1||_F^2
  U^-1 eps = Lt^-T (eps/sqrt(D));   t^T G2^-1 t = ||D2^-1/2 Lt2^-1 t||^2
  sig_term = (n w0 w0^T + (D-n) w1 w1^T)/2   (constant across batch)
"""
import sys

for _p in ("/opt/trn_rl_repo", "/root/.axon_site/_ro/trn_rl_repo"):
    if _p not in sys.path:
        sys.path.append(_p)

import numpy as np
import ml_dtypes
from contextlib import ExitStack

import concourse.bass as bass
import concourse.mybir as mybir
import concourse.tile as tile
from concourse.masks import make_identity

B, D, N, H = 64, 12288, 32, 2048
NCORES = 8
BL = B // NCORES          # 8 local samples (only used for host sharding)
HS = H // NCORES          # 256: C block rows per core
DS = D // NCORES          # 1536
KT_D = D // 128           # 96
KT_DS = DS // 128         # 12
KT_H = H // 128           # 16
P = 128

F32 = mybir.dt.float32
F32R = mybir.dt.float32r
BF16 = mybir.dt.bfloat16
FP8 = mybir.dt.float8e4
SC8 = 256.0
Alu = mybir.AluOpType
Act = mybir.ActivationFunctionType
RG = [list(range(NCORES))]


def sub_ap(t, extra_off, dims):
    """Custom free-dim AP on a [P, F] tile; dims = [[step,count],...] in elems."""
    base = t[:, 0:1]
    return bass.AP(base.tensor, base.offset + extra_off, [base.ap[0]] + dims)


def dram_ap(t, off, dims):
    """Custom AP into a DRAM tensor; dims = [[step,count],...] in elems."""
    base = t[:]
    return bass.AP(base.tensor, off, dims)


def pe_T(nc, out_ps, in_ap, ident):
    """PE transpose: out_ps [f, p] = in_ap [p, f].T"""
    kp = in_ap.shape[0]
    nc.tensor.transpose(out_ps, in_ap, ident[0:kp, 0:kp])


def emit_ldlt(nc, T, OUT, invD, rows, n=32):
    """In-place unit-lower LDLT of T [rows, n*n] (row-major per sample).
    After: strict lower of T holds unscaled columns u; diag holds D; invD=1/D."""
    for j in range(n):
        nc.vector.reciprocal(invD[:, j:j + 1], T[:, (n + 1) * j:(n + 1) * j + 1])
        m = n - 1 - j
        if m == 0:
            break
        base = (j + 1) * n + j
        u_i = sub_ap(T, base, [[n, m], [0, m]])
        u_k = sub_ap(T, base, [[0, m], [n, m]])
        outer = sub_ap(OUT, 0, [[m, m], [1, m]])
        nc.vector.scalar_tensor_tensor(
            outer, u_i, invD[:, j:j + 1], u_k, Alu.mult, Alu.mult)
        trail = sub_ap(T, (j + 1) * (n + 1), [[n, m], [1, m]])
        nc.vector.tensor_tensor(trail, trail, outer, Alu.subtract)


def emit_ltinv(nc, LT, X, OUT, rows, n=32):
    """X = LT^{-1} for unit-lower LT [rows, n*n]; X preset to I by caller."""
    for k in range(n - 1):
        rr = n - 1 - k
        cols = k + 1
        lcol = sub_ap(LT, (k + 1) * n + k, [[n, rr], [0, cols]])
        xrow = sub_ap(X, k * n, [[0, rr], [1, cols]])
        prod = sub_ap(OUT, 0, [[cols, rr], [1, cols]])
        nc.vector.scalar_tensor_tensor(prod, lcol, -1.0, xrow, Alu.mult, Alu.mult)
        xblk = sub_ap(X, (k + 1) * n, [[n, rr], [1, cols]])
        nc.vector.tensor_tensor(xblk, xblk, prod, Alu.add)


def emit_fwd_solve_alt(nc, LT, w, rows, n=32):
    """Forward-substitute LT y = w in place, ONE STT per column:
      stored[k+1:] <- (LT[k+1:,k] * stored[k]) - stored[k+1:]
    This leaves stored[j] = (-1)^j * y[j] (every update flips the sign of the
    remaining entries, and the scalar operand carries the matching sign), so
    it is valid whenever the caller only consumes y elementwise-squared."""
    for k in range(n - 1):
        rr = n - 1 - k
        lcol = sub_ap(LT, (k + 1) * n + k, [[n, rr]])
        nc.vector.scalar_tensor_tensor(
            w[:, k + 1:n], lcol, w[:, k:k + 1], w[:, k + 1:n],
            Alu.mult, Alu.subtract)


def emit_fwd_solve(nc, LT, y, OUT, rows, n=32):
    """y <- LT^{-1} y for unit-lower LT [rows, n*n], y [rows, n] in place."""
    for k in range(n - 1):
        rr = n - 1 - k
        lcol = sub_ap(LT, (k + 1) * n + k, [[n, rr]])
        nc.vector.scalar_tensor_tensor(
            OUT[:, 0:rr], lcol, -1.0, y[:, k:k + 1].broadcast_to([rows, rr]),
            Alu.mult, Alu.mult)
        nc.vector.tensor_tensor(y[:, k + 1:n], y[:, k + 1:n], OUT[:, 0:rr], Alu.add)


def legalize_waits(nc, maxw=1):
    """Split multi-wait sync_info into standalone EventSemaphore instructions."""
    for f in nc.m.functions:
        for bb in f.blocks:
            insts = list(bb.instructions)
            out = []
            changed = False
            for inst in insts:
                si = inst.sync_info
                if si is not None and si.on_wait and len(si.on_wait) > maxw:
                    waits = list(si.on_wait)
                    imm = [w for w in waits if w.uses_immediate]
                    reg = [w for w in waits if not w.uses_immediate]
                    keep = (reg + imm)[:maxw] if len(reg) <= maxw else reg
                    extra = [w for w in waits if w not in keep]
                    if len(keep) > maxw:
                        raise RuntimeError(f"{inst.name}: {len(keep)} register waits")
                    for w in extra:
                        ev = mybir.InstEventSemaphore(
                            name=nc.get_next_instruction_name(), ins=[], outs=[])
                        ev.engine = inst.engine
                        ev.sync_info = mybir.SyncInfo(on_wait=[w], on_update=[])
                        out.append(ev)
                    inst.sync_info = mybir.SyncInfo(
                        on_wait=keep, on_update=list(si.on_update or []))
                    changed = True
                out.append(inst)
            if changed:
                bb.instructions = out
    return nc


def build_nc():
    nc = bass.Bass()

    # ---- I/O (per-core views prepared by host; H-permuted, D-tile-reordered)
    w2t8 = nc.dram_tensor("w2t8", [D // 2, 2 * H], FP8, kind="ExternalInput")
    xt = nc.dram_tensor("xt", [D, B], BF16, kind="ExternalInput")
    w1es = nc.dram_tensor("w1es", [D, HS], BF16, kind="ExternalInput")
    b1es = nc.dram_tensor("b1es", [1, HS], F32, kind="ExternalInput")
    w2es = nc.dram_tensor("w2es", [HS, N], F32, kind="ExternalInput")
    b2e = nc.dram_tensor("b2e", [1, N], F32, kind="ExternalInput")
    w1p = nc.dram_tensor("w1p", [N, H], F32, kind="ExternalInput")
    w1tp_bf = nc.dram_tensor("w1tp_bf", [H, N], BF16, kind="ExternalInput")
    w1tn = nc.dram_tensor("w1tn", [P, N * KT_H], BF16, kind="ExternalInput")
    b1dp = nc.dram_tensor("b1dp", [1, H], F32, kind="ExternalInput")
    xmbt8 = nc.dram_tensor("xmbt8", [DS // 2, 2 * B], FP8, kind="ExternalInput")
    xmb = nc.dram_tensor("xmb", [B, DS], F32, kind="ExternalInput")
    sigw = nc.dram_tensor("sigw", [1, 130], F32, kind="ExternalInput")
    epsin = nc.dram_tensor("epsin", [B, N], F32, kind="ExternalInput")
    out = nc.dram_tensor("out", [B, 1], F32, kind="ExternalOutput")

    # ---- internal DRAM ----
    zstd = nc.dram_tensor("zstd", [B, N], F32)
    zst_sh = nc.dram_tensor("zst_sh", [B, N], F32, addr_space="Shared")
    g1d = nc.dram_tensor("g1d", [B, N * N], F32)
    g1_sh = nc.dram_tensor("g1_sh", [B, N * N], F32, addr_space="Shared")
    PKW = N + 1 + N * N   # 1057
    pkd = nc.dram_tensor("pkd", [B, PKW], F32)
    pk_sh = nc.dram_tensor("pk_sh", [B, PKW], F32, addr_space="Shared")

    with tile.TileContext(nc) as tc, ExitStack() as ctx:
        consts = ctx.enter_context(tc.tile_pool(name="consts", bufs=1))
        work = ctx.enter_context(tc.tile_pool(name="work", bufs=1))
        lin = ctx.enter_context(tc.tile_pool(name="lin", bufs=1))
        pre_cm = tc.tile_pool(name="pre_ps", bufs=2, space="PSUM")
        pre_ps = pre_cm.__enter__()

        # ---- constants ----
        identf = consts.tile([P, P], F32)
        make_identity(nc, identf)
        identb = consts.tile([P, P], BF16)
        make_identity(nc, identb)
        wu_d = nc.dram_tensor("wu_d", [1, 16], F32)
        wu_sh = nc.dram_tensor("wu_sh", [1, 16], F32, addr_space="Shared")
        nc.gpsimd.collective_compute("AllReduce", Alu.add, replica_groups=RG,
                                     ins=[wu_d[:]], outs=[wu_sh[:]])
        ones1 = consts.tile([1, B], F32)
        nc.vector.memset(ones1, 1.0)
        zeros2 = consts.tile([P, 2, B], F32)
        nc.vector.memset(zeros2, 0.0)
        sigw_sb = consts.tile([1, 130], F32)
        nc.sync.dma_start(sigw_sb, sigw[:])
        eps_sb = consts.tile([B, N], F32)
        nc.sync.dma_start(eps_sb, epsin[:])
        b1es_sb = consts.tile([1, HS], F32)
        nc.sync.dma_start(b1es_sb, b1es[:])
        b2e_sb = consts.tile([1, N], F32)
        nc.sync.dma_start(b2e_sb, b2e[:])
        b1d_sb = consts.tile([1, H], F32)
        nc.sync.dma_start(b1d_sb, b1dp[:])
        w2es_sb = consts.tile([P, 2, N], F32)
        nc.sync.dma_start(w2es_sb, w2es[:].rearrange("(k p) n -> p k n", p=P))
        w1_sb = consts.tile([N, H], F32)
        nc.sync.dma_start(w1_sb, w1p[:])
        w1Tb_sb = consts.tile([P, KT_H, N], BF16)
        nc.sync.dma_start(w1Tb_sb, w1tp_bf[:].rearrange("(k p) n -> p k n", p=P))
        w1tn_sb = consts.tile([P, N, KT_H], BF16)
        nc.sync.dma_start(w1tn_sb, w1tn[:])
        sgn_sb = consts.tile([B, N], F32)
        nc.vector.memset(sgn_sb, -1.0)
        nc.vector.memset(sub_ap(sgn_sb, 1, [[2, N // 2]]), 1.0)

        sigw_rep = consts.tile([B, 130], F32)
        sigw_ps = pre_ps.tile([B, 130], F32, tag="sp")
        nc.tensor.matmul(sigw_ps, ones1, sigw_sb, start=True, stop=True)
        nc.vector.tensor_copy(sigw_rep, sigw_ps)

        # sig_term replicated [B, N*N]
        st_ps = pre_ps.tile([N, N], F32, tag="sp")
        nc.tensor.matmul(st_ps, sigw_sb[:, 66:98], sigw_sb[:, 66:98],
                         start=True, stop=False)
        nc.tensor.matmul(st_ps, sigw_sb[:, 98:130], sigw_sb[:, 98:130],
                         start=False, stop=True)
        st_sb = work.tile([N, N], F32, tag="st_sb")
        nc.vector.tensor_copy(st_sb, st_ps)
        st_flat = work.tile([1, N * N], F32, tag="st_flat")
        nc.sync.dma_start(st_flat, st_sb)
        st_rep = consts.tile([B, N * N], F32)
        for hh in range(2):
            sps2 = pre_ps.tile([B, 512], F32, tag="sp")
            nc.tensor.matmul(sps2, ones1, st_flat[:, hh * 512:(hh + 1) * 512],
                             start=True, stop=True)
            nc.vector.tensor_copy(st_rep[:, hh * 512:(hh + 1) * 512], sps2)

        def emit_sig(z_in, name):
            lg = lin.tile([B, 2, N], F32, tag="sig_lg")
            nc.vector.tensor_tensor(
                lg, z_in.unsqueeze(1).broadcast_to([B, 2, N]),
                sigw_rep[:, 0:64].rearrange("p (c n) -> p c n", c=2), Alu.mult)
            red = lin.tile([B, 2], F32, tag=f"sig_red_{name}")
            nc.vector.tensor_reduce(red, lg, mybir.AxisListType.X, Alu.add)
            nc.vector.tensor_tensor(red, red, sigw_rep[:, 64:66], Alu.add)
            s = lin.tile([B, 2], F32, tag=f"sig_s_{name}")
            nc.scalar.activation(s, red, Act.Exp)
            return s

        # ================= encoder (model-parallel over enc-H) ==============
        with tc.tile_pool(name="p_enc_s", bufs=2) as enc_s, \
             tc.tile_pool(name="p_enc_ps", bufs=1, space="PSUM") as enc_ps:
            xt_r = xt[:].rearrange("(k p) b -> p k b", p=P)
            w1es_r = w1es[:].rearrange("(k p) h -> p k h", p=P)
            a1e_ps = enc_ps.tile([B, HS], F32, tag="a1e")
            for kb in range(12):
                xtile = enc_s.tile([P, 8, B], BF16, tag="xt_t")
                nc.scalar.dma_start(xtile, xt_r[:, kb * 8:(kb + 1) * 8, :])
                wtile = enc_s.tile([P, 8, HS], BF16, tag="w1es_t")
                nc.scalar.dma_start(wtile, w1es_r[:, kb * 8:(kb + 1) * 8, :])
                for j in range(8):
                    nc.tensor.matmul(a1e_ps, xtile[:, j, :], wtile[:, j, :],
                                     start=(kb == 0 and j == 0), stop=False)
            nc.tensor.matmul(a1e_ps, ones1[:, 0:B], b1es_sb,
                             start=False, stop=True)
            h1_sb = work.tile([B, HS], F32, tag="h1")
            nc.vector.tensor_scalar(h1_sb, a1e_ps, 0.0, None, Alu.max)
            h1T_sb = work.tile([P, 2, B], F32, tag="h1T")
            for i in range(2):
                tp = enc_ps.tile([P, B], F32, tag="tp")
                pe_T(nc, tp, h1_sb[:, i * P:(i + 1) * P], identf)
                nc.scalar.copy(h1T_sb[:, i, :], tp)
            zp_ps = enc_ps.tile([B, N], F32, tag="zp")
            for i in range(2):
                nc.tensor.matmul(zp_ps, h1T_sb[:, i, :], w2es_sb[:, i, :],
                                 start=(i == 0), stop=(i == 1))
            zp_sb = work.tile([B, N], F32, tag="zp_sb")
            nc.vector.tensor_copy(zp_sb, zp_ps)
            nc.sync.dma_start(zstd[:], zp_sb)
        nc.gpsimd.collective_compute("AllReduce", Alu.add, replica_groups=RG,
                                     ins=[zstd[:]], outs=[zst_sh[:]])

        pre_cm.__exit__(None, None, None)

        # ========= u = xmb @ W2s^T (partial, fp8 DoubleRow), |xmb|^2 =======
        KT2_D = KT_D // 2       # 48 double-row tiles
        KT2_DS = KT_DS // 2     # 6 own tiles
        cpool = ctx.enter_context(tc.tile_pool(name="cpool", bufs=1))
        c_bf = cpool.tile([P, 2, H], BF16, tag="c_bf")
        ckT_bf = cpool.tile([P, KT_H, 2 * P], BF16, tag="ckT")
        w2t_r = w2t8[:].rearrange("(k p) f -> p k f", p=P)
        w2res_cm = tc.tile_pool(name="w2res", bufs=1)
        w2res_pool = w2res_cm.__enter__()
        w2res = w2res_pool.tile([P, KT2_DS, 2, H], FP8, tag="w2res")
        for kt in range(KT2_DS):
            nc.sync.dma_start(
                w2res[:, kt, :, :],
                w2t_r[:, kt, :].rearrange("p (two h) -> p two h", two=2))
        xmbT_sb = work.tile([P, KT2_DS, 2, B], FP8, tag="xmbT")
        nc.sync.dma_start(
            xmbT_sb,
            xmbt8[:].rearrange("(k p) (two b) -> p k two b", p=P, two=2))
        xmb_sb = work.tile([B, DS], F32, tag="xmb")
        nc.sync.dma_start(xmb_sb, xmb[:])
        xmbsq = lin.tile([B, 1], F32, tag="xmbsq")
        scr2 = work.tile([B, H], BF16, tag="scr2")
        nc.scalar.activation(scr2[:, 0:DS], xmb_sb, Act.Square,
                             accum_out=xmbsq)
        u_sb = work.tile([B, H], BF16, tag="u_sb")
        DR = mybir.MatmulPerfMode.DoubleRow
        with tc.tile_pool(name="p_u", bufs=1, space="PSUM") as u_ps_pool:
            u_ps = u_ps_pool.tile([B, H], F32, tag="u_ps")
            for nb in range(4):
                for kt in range(KT2_DS):
                    nc.tensor.matmul(
                        u_ps[:, nb * 512:(nb + 1) * 512],
                        xmbT_sb[:, kt, :, :],
                        w2res[:, kt, :, nb * 512:(nb + 1) * 512],
                        start=(kt == 0), stop=(kt == KT2_DS - 1),
                        perf_mode=DR)
            nc.scalar.activation(u_sb, u_ps, Act.Copy, scale=1.0 / SC8)

        # ---- z* full (+enc b2), zT, sig1, masks m1 (before C' on PE) ----
        zf_sb = work.tile([B, N], F32, tag="zf")
        nc.sync.dma_start(zf_sb, zst_sh[:])
        zall = lin.tile([B, N], F32, tag="zall")
        zT_sb = work.tile([N, B], F32, tag="zT")
        with tc.tile_pool(name="p_z", bufs=2, space="PSUM") as pz:
            za_ps = pz.tile([B, N], F32, tag="za")
            nc.tensor.matmul(za_ps, ones1[:, 0:B], b2e_sb, start=True, stop=False)
            nc.tensor.matmul(za_ps, identf[0:B, 0:B], zf_sb, start=False,
                             stop=True)
            nc.vector.tensor_copy(zall, za_ps)
            zT_ps = pz.tile([N, B], F32, tag="za")
            pe_T(nc, zT_ps, zall, identf)
            nc.vector.tensor_copy(zT_sb, zT_ps)
        s1 = emit_sig(zall, "s1")
        invsp2 = lin.tile([B, 1], F32, tag="invsp2")
        sp2t = lin.tile([B, 1], F32, tag="sp2t")
        nc.vector.tensor_tensor(sp2t, s1[:, 0:1], s1[:, 0:1], Alu.mult)
        nc.vector.reciprocal(invsp2, sp2t)
        m1n = work.tile([P, B, KT_H], BF16, tag="m1n")
        with tc.tile_pool(name="p_a1t", bufs=1, space="PSUM") as a1t_pool:
            a1T_ps = a1t_pool.tile([P, KT_H, B], F32, tag="a1T")
            for mt in range(KT_H):
                nc.tensor.matmul(a1T_ps[:, mt, :],
                                 w1_sb[:, mt * P:(mt + 1) * P], zT_sb,
                                 start=True, stop=False)
                nc.tensor.matmul(a1T_ps[:, mt, :],
                                 b1d_sb[:, mt * P:(mt + 1) * P], ones1[:, 0:B],
                                 start=False, stop=True)
            nc.vector.tensor_scalar(
                sub_ap(m1n, 0, [[1, KT_H], [KT_H, B]]), a1T_ps,
                0.0, None, Alu.is_gt)

        # ================= C' : C_k = W2[hs0,:] @ W2^T  (stream W2^T) =======
        pC_cm = tc.tile_pool(name="pC_s", bufs=3)
        pC = pC_cm.__enter__()
        pCp_cm = tc.tile_pool(name="pC_ps", bufs=1, space="PSUM")
        pCp = pCp_cm.__enter__()
        cps = pCp.tile([P, 8, 512], F32, tag="cps")
        for kt in range(KT2_D):
            if kt < KT2_DS:
                t_in = w2res[:, kt, :, :]
            else:
                t_raw = pC.tile([P, 2, H], FP8, tag="w2_t")
                dq = nc.sync if kt % 2 == 0 else nc.scalar
                dq.dma_start(
                    t_raw,
                    w2t_r[:, kt, :].rearrange("p (two h) -> p two h", two=2))
                t_in = t_raw
            for it in range(2):
                for nb in range(4):
                    nc.tensor.matmul(
                        cps[:, it * 4 + nb, :],
                        t_in[:, :, it * P:(it + 1) * P],
                        t_in[:, :, nb * 512:(nb + 1) * 512],
                        start=(kt == 0), stop=(kt == KT2_D - 1),
                        perf_mode=DR)
        for it in range(2):
            for nb in range(4):
                nc.scalar.activation(c_bf[:, it, nb * 512:(nb + 1) * 512],
                                     cps[:, it * 4 + nb, :], Act.Copy,
                                     scale=1.0 / (SC8 * SC8))
        pCp_cm.__exit__(None, None, None)
        pC_cm.__exit__(None, None, None)
        w2res_cm.__exit__(None, None, None)
        # C_k^T via PE transposes of 128x128 blocks
        with tc.tile_pool(name="p_ct", bufs=2, space="PSUM") as ct_ps_pool:
            for it in range(2):
                for jt in range(KT_H):
                    tp = ct_ps_pool.tile([P, P], BF16, tag="ct")
                    pe_T(nc, tp, c_bf[:, it, jt * P:(jt + 1) * P], identb)
                    nc.scalar.copy(ckT_bf[:, jt, it * P:(it + 1) * P], tp)

        # ================= G partials (shared emitter) ======================
        at_pool = ctx.enter_context(tc.tile_pool(name="at_pool", bufs=2))
        g_pool = ctx.enter_context(tc.tile_pool(name="g_pool", bufs=1))

        def emit_G(mn, tag):
            g_sb = g_pool.tile([N, B * N], F32, tag=f"g_{tag}")
            with tc.tile_pool(name=f"pG{tag}", bufs=2, space="PSUM") as gps_pool:
                for cb in range(4):
                    # AT'[p, s, n, kt] with kt innermost-packed on every
                    # operand -> DVE 2x mode applies
                    AT = at_pool.tile([P, 16, N, KT_H], BF16, tag="AT")
                    nc.vector.tensor_tensor(
                        AT,
                        w1tn_sb.unsqueeze(1).broadcast_to([P, 16, N, KT_H]),
                        mn[:, cb * 16:(cb + 1) * 16, :]
                        .unsqueeze(2).broadcast_to([P, 16, N, KT_H]),
                        Alu.mult)
                    m1ps = gps_pool.tile([P, 2, 512], F32, tag="m1ps")
                    for it in range(2):
                        for jt in range(KT_H):
                            nc.tensor.matmul(
                                m1ps[:, it, :],
                                ckT_bf[:, jt, it * P:(it + 1) * P],
                                AT[:, :, :, jt],
                                start=(jt == 0), stop=(jt == KT_H - 1))
                    m1sb = at_pool.tile([P, 2, 512], BF16, tag="m1sb")
                    nc.scalar.copy(m1sb, m1ps)
                    gps = gps_pool.tile([N, 512], F32, tag="gps")
                    for s in range(16):
                        for it in range(2):
                            nc.tensor.matmul(
                                gps[:, s * N:(s + 1) * N],
                                AT[:, s, :, it],
                                m1sb[:, it, s * N:(s + 1) * N],
                                start=(it == 0), stop=(it == 1))
                    nc.vector.tensor_copy(
                        g_sb[:, cb * 512:(cb + 1) * 512], gps)
            return g_sb

        # ---- G1 -> pack -> AllReduce ----
        g1_sb = emit_G(m1n, "1")
        nc.sync.dma_start(
            dram_ap(g1d, 0, [[N, N], [N * N, B], [1, N]]), g1_sb)
        nc.gpsimd.collective_compute("AllReduce", Alu.add, replica_groups=RG,
                                     ins=[g1d[:]], outs=[g1_sh[:]])

        # ---- Prec assembly + LDLT + ltinv + dz + z_s ----
        Tm = lin.tile([B, N * N], F32, tag="Tm")
        nc.sync.dma_start(Tm, g1_sh[:])
        nc.vector.tensor_scalar(Tm, Tm, invsp2, None, Alu.mult)
        nc.vector.tensor_tensor(Tm, Tm, st_rep, Alu.add)
        diag1 = sub_ap(Tm, 0, [[N + 1, N]])
        nc.vector.tensor_scalar(diag1, diag1, 1.0, None, Alu.add)

        invD = lin.tile([B, N], F32, tag="invD")
        SCR = lin.tile([B, N * N], F32, tag="SCR")
        emit_ldlt(nc, Tm, SCR, invD, B)
        LT = lin.tile([B, N * N], F32, tag="LT")
        nc.vector.tensor_tensor(
            LT.rearrange("p (a b) -> p a b", b=N),
            Tm.rearrange("p (a b) -> p a b", b=N),
            invD.unsqueeze(1).broadcast_to([B, N, N]), Alu.mult)
        # dz: solve Lt^T dz = epss by backward substitution, one STT per
        # column: stored[0:k] <- (Ltrow_k * stored[k]) - stored[0:k], which
        # leaves stored[j] = (-1)^(N-1-j) dz[j]; fixed up with sgn_sb.
        srD = lin.tile([B, N], F32, tag="srD")
        nc.scalar.activation(srD, invD, Act.Sqrt)        # 1/sqrt(D)
        dz = lin.tile([B, N], F32, tag="dz")
        nc.vector.tensor_tensor(dz, eps_sb, srD, Alu.mult)
        for k in range(N - 1, 0, -1):
            lrow = sub_ap(LT, k * N, [[1, k]])
            nc.vector.scalar_tensor_tensor(
                dz[:, 0:k], lrow, dz[:, k:k + 1], dz[:, 0:k],
                Alu.mult, Alu.subtract)
        zs = lin.tile([B, N], F32, tag="zs")
        nc.vector.tensor_tensor(dz, dz, sgn_sb, Alu.mult)
        nc.vector.tensor_tensor(zs, zall, dz, Alu.add)
        s2 = emit_sig(zs, "s2")

        # ================= stage 2 ==========================================
        zsT_sb = work.tile([N, B], F32, tag="zsT")
        with tc.tile_pool(name="p_zst", bufs=1, space="PSUM") as zst_pool:
            zsT_ps = zst_pool.tile([N, B], F32, tag="zsT_ps")
            pe_T(nc, zsT_ps, zs, identf)
            nc.vector.tensor_copy(zsT_sb, zsT_ps)

        m2n = work.tile([P, B, KT_H], BF16, tag="m2n")
        h2neg = work.tile([P, 2, B], BF16, tag="h2neg")
        with tc.tile_pool(name="p_a2t", bufs=1, space="PSUM") as a2t_pool:
            a2T_ps = a2t_pool.tile([P, KT_H, B], F32, tag="a2T")
            for mt in range(KT_H):
                nc.tensor.matmul(a2T_ps[:, mt, :],
                                 w1_sb[:, mt * P:(mt + 1) * P], zsT_sb,
                                 start=True, stop=False)
                nc.tensor.matmul(a2T_ps[:, mt, :],
                                 b1d_sb[:, mt * P:(mt + 1) * P], ones1[:, 0:B],
                                 start=False, stop=True)
            nc.vector.tensor_scalar(
                sub_ap(m2n, 0, [[1, KT_H], [KT_H, B]]), a2T_ps,
                0.0, None, Alu.is_gt)
            # -relu(a2) for local block (first 2 kt): min(-a2, 0)
            nc.vector.scalar_tensor_tensor(
                h2neg, a2T_ps[:, 0:2, :], -1.0, zeros2, Alu.mult, Alu.min)

        h2_sb = work.tile([B, H], BF16, tag="h2")
        with tc.tile_pool(name="p_a2", bufs=1, space="PSUM") as a2_pool:
            a2_ps = a2_pool.tile([B, H], F32, tag="a2")
            for nb in range(4):
                nc.tensor.matmul(a2_ps[:, nb * 512:(nb + 1) * 512],
                                 zsT_sb, w1_sb[:, nb * 512:(nb + 1) * 512],
                                 start=True, stop=False)
                nc.tensor.matmul(a2_ps[:, nb * 512:(nb + 1) * 512],
                                 ones1[:, 0:B], b1d_sb[:, nb * 512:(nb + 1) * 512],
                                 start=False, stop=True)
            nc.vector.tensor_scalar(h2_sb, a2_ps, 0.0, None, Alu.max)

        uh2 = lin.tile([B, 1], F32, tag="uh2")
        nc.vector.tensor_tensor(scr2, u_sb, h2_sb, Alu.mult)
        nc.vector.tensor_reduce(uh2, scr2, mybir.AxisListType.X, Alu.add)

        wd_sb = work.tile([B, H], BF16, tag="wd")
        vneg_bf = work.tile([B, H], BF16, tag="vneg")
        vh2m = lin.tile([B, 1], F32, tag="vh2m")
        with tc.tile_pool(name="p_v", bufs=1, space="PSUM") as v_pool:
            v_ps = v_pool.tile([B, H], F32, tag="v_ps")   # holds -v
            for nb in range(4):
                for it in range(2):
                    nc.tensor.matmul(v_ps[:, nb * 512:(nb + 1) * 512],
                                     h2neg[:, it, :],
                                     c_bf[:, it, nb * 512:(nb + 1) * 512],
                                     start=(it == 0), stop=(it == 1))
            nc.scalar.copy(vneg_bf, v_ps)
        nc.vector.tensor_tensor(scr2, vneg_bf, h2_sb, Alu.mult)
        nc.vector.tensor_reduce(vh2m, scr2, mybir.AxisListType.X, Alu.add)
        nc.vector.tensor_tensor(wd_sb, u_sb, vneg_bf, Alu.add)
        svec = lin.tile([B, 1], F32, tag="svec")
        nc.vector.scalar_tensor_tensor(svec, uh2, -2.0, xmbsq, Alu.mult, Alu.add)
        nc.vector.tensor_tensor(svec, svec, vh2m, Alu.subtract)

        # t_k = A2 wd_k : transpose wd, mask, matmul
        wdT_sb = work.tile([P, KT_H, B], BF16, tag="wdT")
        with tc.tile_pool(name="p_wdt", bufs=2, space="PSUM") as wdt_pool:
            for jt in range(KT_H):
                tp = wdt_pool.tile([P, B], BF16, tag="wdt")
                pe_T(nc, tp, wd_sb[:, jt * P:(jt + 1) * P], identb)
                nc.scalar.copy(wdT_sb[:, jt, :], tp)
        mwdT = work.tile([P, KT_H, B], BF16, tag="mwdT")
        nc.vector.tensor_tensor(
            mwdT, wdT_sb,
            sub_ap(m2n, 0, [[1, KT_H], [KT_H, B]]), Alu.mult)
        tk_sb = work.tile([N, B], F32, tag="tk")
        with tc.tile_pool(name="p_tk", bufs=1, space="PSUM") as tk_pool:
            tk_ps = tk_pool.tile([N, B], F32, tag="tk_ps")
            for jt in range(KT_H):
                nc.tensor.matmul(tk_ps, w1Tb_sb[:, jt, :], mwdT[:, jt, :],
                                 start=(jt == 0), stop=(jt == KT_H - 1))
            nc.vector.tensor_copy(tk_sb, tk_ps)

        # ---- G2 + pack [t | svec | G2] -> AllReduce ----
        g2_sb = emit_G(m2n, "2")
        nc.sync.dma_start(dram_ap(pkd, 0, [[1, N], [PKW, B]]), tk_sb)
        nc.sync.dma_start(dram_ap(pkd, N, [[PKW, B]]), svec)
        nc.sync.dma_start(
            dram_ap(pkd, N + 1, [[N, N], [PKW, B], [1, N]]), g2_sb)
        nc.gpsimd.collective_compute("AllReduce", Alu.add, replica_groups=RG,
                                     ins=[pkd[:]], outs=[pk_sh[:]])

        # ---- background (overlaps AllReduce): ltinv, tr, logdet, latent ----
        X1 = lin.tile([B, N * N], F32, tag="X1")
        nc.vector.memset(X1, 0.0)
        nc.vector.memset(sub_ap(X1, 0, [[N + 1, N]]), 1.0)
        emit_ltinv(nc, LT, X1, SCR, B)
        scrB = lin.tile([B, N * N], F32, tag="scrB")
        nc.vector.tensor_tensor(SCR, X1, X1, Alu.mult)
        trv = lin.tile([B, 1], F32, tag="trv")
        nc.vector.tensor_tensor(
            scrB.rearrange("p (a b) -> p a b", b=N),
            SCR.rearrange("p (a b) -> p a b", b=N),
            invD.unsqueeze(2).broadcast_to([B, N, N]), Alu.mult)
        nc.vector.tensor_reduce(trv, scrB, mybir.AxisListType.X, Alu.add)
        logs = lin.tile([B, N], F32, tag="logs")
        ldv = lin.tile([B, 1], F32, tag="ldv")
        nc.scalar.activation(logs, invD, Act.Ln)
        nc.vector.tensor_reduce(ldv, logs, mybir.AxisListType.X, Alu.add)
        zsq = lin.tile([B, N], F32, tag="zsq")
        latv = lin.tile([B, 1], F32, tag="latv")
        nc.vector.tensor_tensor(zsq, zall, zall, Alu.mult)
        nc.vector.tensor_reduce(latv, zsq, mybir.AxisListType.X, Alu.add)
        nc.vector.tensor_tensor(latv, latv, trv, Alu.add)
        nc.vector.tensor_scalar(latv, latv, 0.5, None, Alu.mult)
        nc.vector.tensor_scalar(ldv, ldv, -0.5, None, Alu.mult)

        # ---- post-AllReduce: solve G2 y = t, d_proj, recon, out ----
        y = lin.tile([B, N], F32, tag="y")
        nc.sync.dma_start(y, pk_sh[:, 0:N])
        svf = lin.tile([B, 1], F32, tag="svf")
        nc.sync.dma_start(svf, pk_sh[:, N:N + 1])
        Tm2 = lin.tile([B, N * N], F32, tag="Tm2")
        nc.sync.dma_start(Tm2, pk_sh[:, N + 1:PKW])
        invD2 = lin.tile([B, N], F32, tag="invD2")
        emit_ldlt(nc, Tm2, SCR, invD2, B)
        LT2 = lin.tile([B, N * N], F32, tag="LT2")
        nc.vector.tensor_tensor(
            LT2.rearrange("p (a b) -> p a b", b=N),
            Tm2.rearrange("p (a b) -> p a b", b=N),
            invD2.unsqueeze(1).broadcast_to([B, N, N]), Alu.mult)
        emit_fwd_solve_alt(nc, LT2, y, B)
        ysq = lin.tile([B, N], F32, tag="ysq")
        yw = lin.tile([B, N], F32, tag="yw")
        dproj = lin.tile([B, 1], F32, tag="dproj")
        nc.vector.tensor_tensor(ysq, y, y, Alu.mult)
        nc.vector.tensor_tensor(yw, ysq, invD2, Alu.mult)
        nc.vector.tensor_reduce(dproj, yw, mybir.AxisListType.X, Alu.add)

        sq2 = lin.tile([B, 2], F32, tag="sq2")
        nc.vector.tensor_tensor(sq2, s2, s2, Alu.mult)
        nc.vector.tensor_scalar(sq2, sq2, 2.0, None, Alu.mult)
        inv2 = lin.tile([B, 2], F32, tag="inv2")
        nc.vector.reciprocal(inv2, sq2)     # [1/(2sp2^2), 1/(2sv2^2)]
        logs2 = lin.tile([B, 2], F32, tag="logs2")
        logw = lin.tile([B, 2], F32, tag="logw")
        nc.vector.memset(logw[:, 0:1], float(N))
        nc.vector.memset(logw[:, 1:2], float(D - N))
        nc.scalar.activation(logs2, s2, Act.Ln)
        logterm = lin.tile([B, 1], F32, tag="logterm")
        junk2 = lin.tile([B, 2], F32, tag="junk2")
        nc.vector.tensor_tensor(junk2, logs2, logw, Alu.mult)
        nc.vector.tensor_reduce(logterm, junk2, mybir.AxisListType.X, Alu.add)
        isub = lin.tile([B, 1], F32, tag="isub")
        nc.vector.tensor_tensor(isub, inv2[:, 0:1], inv2[:, 1:2], Alu.subtract)
        recon = lin.tile([B, 1], F32, tag="recon")
        nc.vector.tensor_tensor(recon, dproj, isub, Alu.mult)
        p2t = lin.tile([B, 1], F32, tag="p2t")
        nc.vector.tensor_tensor(p2t, svf, inv2[:, 1:2], Alu.mult)
        nc.vector.tensor_tensor(recon, recon, p2t, Alu.add)
        nc.vector.tensor_tensor(recon, recon, logterm, Alu.add)
        ov = lin.tile([B, 1], F32, tag="ov")
        nc.vector.tensor_tensor(ov, recon, latv, Alu.add)
        nc.vector.tensor_tensor(ov, ov, ldv, Alu.add)
        nc.vector.tensor_scalar(ov, ov, 1.0 / D, None, Alu.mult)
        nc.sync.dma_start(out[:], ov)

    legalize_waits(nc)
    return nc


def shard_inputs(inputs):
    """Host-side prep: per-core H-permutation + D-tile reordering."""
    bf = ml_dtypes.bfloat16
    x = np.ascontiguousarray(np.asarray(inputs["x"], np.float32))
    eps = np.ascontiguousarray(np.asarray(inputs["eps"], np.float32))
    eW1 = np.asarray(inputs["enc_W1"], np.float32)
    eb1 = np.asarray(inputs["enc_b1"], np.float32)
    eW2 = np.asarray(inputs["enc_W2"], np.float32)
    eb2 = np.asarray(inputs["enc_b2"], np.float32)
    dW1 = np.asarray(inputs["dec_W1"], np.float32)
    db1 = np.asarray(inputs["dec_b1"], np.float32)
    dW2 = np.asarray(inputs["dec_W2"], np.float32)
    db2 = np.asarray(inputs["dec_b2"], np.float32)
    sW = np.asarray(inputs["sig_W"], np.float32)
    sb = np.asarray(inputs["sig_b"], np.float32)

    xT_bf = np.ascontiguousarray(x.T).astype(bf)
    xmb_full = x - db2[None, :]
    W2T = np.ascontiguousarray(dW2.T)      # [D, H]
    sigv = np.zeros((1, 130), np.float32)
    sigv[0, 0:32] = sW[:, 0]
    sigv[0, 32:64] = sW[:, 1]
    sigv[0, 64:66] = sb
    sigv[0, 66:98] = sW[:, 0] * np.sqrt(N / 2.0)
    sigv[0, 98:130] = sW[:, 1] * np.sqrt((D - N) / 2.0)

    maps = []
    for k in range(NCORES):
        hperm = np.concatenate([np.arange(k * HS, (k + 1) * HS),
                                np.arange(0, k * HS),
                                np.arange((k + 1) * HS, H)])
        # D-tile order: own 12 tiles first
        own = np.arange(k * KT_DS, (k + 1) * KT_DS)
        rest = np.concatenate([np.arange(0, k * KT_DS),
                               np.arange((k + 1) * KT_DS, KT_D)])
        tord = np.concatenate([own, rest])
        w2t_p = W2T[:, hperm].reshape(KT_D, P, H)[tord].reshape(D, H)
        w2q = np.asarray(w2t_p * 256.0, np.float32).astype(ml_dtypes.float8_e4m3)
        w2q = np.ascontiguousarray(
            w2q.reshape(KT_D // 2, 2, P, H).transpose(0, 2, 1, 3)
            .reshape(D // 2, 2 * H))
        dsl = slice(k * DS, (k + 1) * DS)
        xq = np.asarray(xmb_full[:, dsl].T, np.float32).astype(
            ml_dtypes.float8_e4m3)
        xq = np.ascontiguousarray(
            xq.reshape(KT_DS // 2, 2, P, B).transpose(0, 2, 1, 3)
            .reshape(DS // 2, 2 * B))
        maps.append({
            "w2t8": w2q,
            "xt": xT_bf,
            "w1es": np.ascontiguousarray(
                eW1[:, k * HS:(k + 1) * HS]).astype(bf),
            "b1es": np.ascontiguousarray(eb1[None, k * HS:(k + 1) * HS]),
            "w2es": np.ascontiguousarray(eW2[k * HS:(k + 1) * HS, :]),
            "b2e": np.ascontiguousarray(eb2[None, :]),
            "w1p": np.ascontiguousarray(dW1[:, hperm]),
            "w1tp_bf": np.ascontiguousarray(dW1[:, hperm].T).astype(bf),
            "w1tn": np.ascontiguousarray(
                dW1[:, hperm].reshape(N, KT_H, P).transpose(2, 0, 1)
                .reshape(P, N * KT_H)).astype(bf),
            "b1dp": np.ascontiguousarray(db1[None, hperm]),
            "xmbt8": xq,
            "xmb": np.ascontiguousarray(xmb_full[:, dsl]),
            "sigw": sigv,
            "epsin": eps,
        })
    return maps


_NC_CACHE = None


def kernel(**inputs) -> np.ndarray:
    global _NC_CACHE
    from concourse.bass_utils import run_bass_kernel_spmd
    if _NC_CACHE is None:
        _NC_CACHE = build_nc()
    nc = _NC_CACHE
    maps = shard_inputs(inputs)
    res = run_bass_kernel_spmd(nc, maps, list(range(NCORES)))
    return np.asarray(res.results[0]["out"]).reshape(B).astype(np.float32)


# revision 13
# speedup vs baseline: 1.1784x; 1.1784x over previous
"""Trainium2 Bass kernel for nn_EnergyAE (B=64, D=12288, N=32, H=2048) on 8 cores.

v2 restructure (vs v1's contraction-sharded C + 16.8MB AllReduce):
  - C block-row sharding: core k computes C_k = C[hs_k, :] (256 x 2048) locally
    by streaming full W2^T (bf16, 12.6MB); C_k stays in SBUF. No C collective,
    no C DRAM round trip.
  - Per-core H-permutation (host-side) puts each core's block at H-cols 0:256,
    so the SPMD program is core-index-free.
  - G[b] computed as partial sums over ALL 64 samples using C_k; AllReduce of
    packed G (256KB). Every core then factors all 64 (same vector cost) ->
    no z_s AllGather, no sel8 gathers.
  - x_star never materialized: W2 delta = W2(x-b2) - C h2, so
    u = xmb @ W2s^T (partial), v = h2[:, 0:256] @ C_k (partial),
    wd_k = u_k - v_k, t_k = A2 wd_k (pre-AllReduce!),
    d_sq = |xmb|^2 - 2 u.h2 + v.h2 (scalar partials).
  - Final AllReduce packs [t | svec | G2] = 271KB. All cores compute all 64
    outputs; host takes core 0's.

Identities (validated numerically, rel err ~1.7e-3 with bf16 C/A/u):
  Prec = Lt D Lt^T (unit-lower LDLT)
  sum(log eig)/2 = 0.5*sum(log D);  sum(1/eig) = ||D^-1/2 Lt^-1||_F^2
  U^-1 eps = Lt^-T (eps/sqrt(D));   t^T G2^-1 t = ||D2^-1/2 Lt2^-1 t||^2
  sig_term = (n w0 w0^T + (D-n) w1 w1^T)/2   (constant across batch)
"""
import sys

for _p in ("/opt/trn_rl_repo", "/root/.axon_site/_ro/trn_rl_repo"):
    if _p not in sys.path:
        sys.path.append(_p)

import numpy as np
import ml_dtypes
from contextlib import ExitStack

import concourse.bass as bass
import concourse.mybir as mybir
import concourse.tile as tile
from concourse.masks import make_identity

B, D, N, H = 64, 12288, 32, 2048
NCORES = 8
BL = B // NCORES          # 8 local samples (only used for host sharding)
HS = H // NCORES          # 256: C block rows per core
DS = D // NCORES          # 1536
KT_D = D // 128           # 96
KT_DS = DS // 128         # 12
KT_H = H // 128           # 16
P = 128

F32 = mybir.dt.float32
F32R = mybir.dt.float32r
BF16 = mybir.dt.bfloat16
FP8 = mybir.dt.float8e4
SC8 = 256.0
Alu = mybir.AluOpType
Act = mybir.ActivationFunctionType
RG = [list(range(NCORES))]


def sub_ap(t, extra_off, dims):
    """Custom free-dim AP on a [P, F] tile; dims = [[step,count],...] in elems."""
    base = t[:, 0:1]
    return bass.AP(base.tensor, base.offset + extra_off, [base.ap[0]] + dims)


def dram_ap(t, off, dims):
    """Custom AP into a DRAM tensor; dims = [[step,count],...] in elems."""
    base = t[:]
    return bass.AP(base.tensor, off, dims)


def pe_T(nc, out_ps, in_ap, ident):
    """PE transpose: out_ps [f, p] = in_ap [p, f].T"""
    kp = in_ap.shape[0]
    nc.tensor.transpose(out_ps, in_ap, ident[0:kp, 0:kp])


def emit_ldlt(nc, T, OUT, invD, rows, n=32):
    """In-place unit-lower LDLT of T [rows, n*n] (row-major per sample).
    After: strict lower of T holds unscaled columns u; diag holds D; invD=1/D."""
    for j in range(n):
        nc.vector.reciprocal(invD[:, j:j + 1], T[:, (n + 1) * j:(n + 1) * j + 1])
        m = n - 1 - j
        if m == 0:
            break
        base = (j + 1) * n + j
        u_i = sub_ap(T, base, [[n, m], [0, m]])
        u_k = sub_ap(T, base, [[0, m], [n, m]])
        outer = sub_ap(OUT, 0, [[m, m], [1, m]])
        nc.vector.scalar_tensor_tensor(
            outer, u_i, invD[:, j:j + 1], u_k, Alu.mult, Alu.mult)
        trail = sub_ap(T, (j + 1) * (n + 1), [[n, m], [1, m]])
        nc.vector.tensor_tensor(trail, trail, outer, Alu.subtract)


def emit_ltinv(nc, LT, X, OUT, rows, n=32):
    """X = LT^{-1} for unit-lower LT [rows, n*n]; X preset to I by caller."""
    for k in range(n - 1):
        rr = n - 1 - k
        cols = k + 1
        lcol = sub_ap(LT, (k + 1) * n + k, [[n, rr], [0, cols]])
        xrow = sub_ap(X, k * n, [[0, rr], [1, cols]])
        prod = sub_ap(OUT, 0, [[cols, rr], [1, cols]])
        nc.vector.scalar_tensor_tensor(prod, lcol, -1.0, xrow, Alu.mult, Alu.mult)
        xblk = sub_ap(X, (k + 1) * n, [[n, rr], [1, cols]])
        nc.vector.tensor_tensor(xblk, xblk, prod, Alu.add)


def emit_fwd_solve_alt(nc, LT, w, rows, n=32):
    """Forward-substitute LT y = w in place, ONE STT per column:
      stored[k+1:] <- (LT[k+1:,k] * stored[k]) - stored[k+1:]
    This leaves stored[j] = (-1)^j * y[j] (every update flips the sign of the
    remaining entries, and the scalar operand carries the matching sign), so
    it is valid whenever the caller only consumes y elementwise-squared."""
    for k in range(n - 1):
        rr = n - 1 - k
        lcol = sub_ap(LT, (k + 1) * n + k, [[n, rr]])
        nc.vector.scalar_tensor_tensor(
            w[:, k + 1:n], lcol, w[:, k:k + 1], w[:, k + 1:n],
            Alu.mult, Alu.subtract)


def emit_fwd_solve(nc, LT, y, OUT, rows, n=32):
    """y <- LT^{-1} y for unit-lower LT [rows, n*n], y [rows, n] in place."""
    for k in range(n - 1):
        rr = n - 1 - k
        lcol = sub_ap(LT, (k + 1) * n + k, [[n, rr]])
        nc.vector.scalar_tensor_tensor(
            OUT[:, 0:rr], lcol, -1.0, y[:, k:k + 1].broadcast_to([rows, rr]),
            Alu.mult, Alu.mult)
        nc.vector.tensor_tensor(y[:, k + 1:n], y[:, k + 1:n], OUT[:, 0:rr], Alu.add)


def legalize_waits(nc, maxw=1):
    """Split multi-wait sync_info into standalone EventSemaphore instructions."""
    for f in nc.m.functions:
        for bb in f.blocks:
            insts = list(bb.instructions)
            out = []
            changed = False
            for inst in insts:
                si = inst.sync_info
                if si is not None and si.on_wait and len(si.on_wait) > maxw:
                    waits = list(si.on_wait)
                    imm = [w for w in waits if w.uses_immediate]
                    reg = [w for w in waits if not w.uses_immediate]
                    keep = (reg + imm)[:maxw] if len(reg) <= maxw else reg
                    extra = [w for w in waits if w not in keep]
                    if len(keep) > maxw:
                        raise RuntimeError(f"{inst.name}: {len(keep)} register waits")
                    for w in extra:
                        ev = mybir.InstEventSemaphore(
                            name=nc.get_next_instruction_name(), ins=[], outs=[])
                        ev.engine = inst.engine
                        ev.sync_info = mybir.SyncInfo(on_wait=[w], on_update=[])
                        out.append(ev)
                    inst.sync_info = mybir.SyncInfo(
                        on_wait=keep, on_update=list(si.on_update or []))
                    changed = True
                out.append(inst)
            if changed:
                bb.instructions = out
    return nc


def build_nc():
    nc = bass.Bass()

    # ---- I/O (per-core views prepared by host; H-permuted, D-tile-reordered)
    w2t8 = nc.dram_tensor("w2t8", [D // 2, 2 * H], FP8, kind="ExternalInput")
    xt = nc.dram_tensor("xt", [D, B], BF16, kind="ExternalInput")
    w1es = nc.dram_tensor("w1es", [D, HS], BF16, kind="ExternalInput")
    b1es = nc.dram_tensor("b1es", [1, HS], F32, kind="ExternalInput")
    w2es = nc.dram_tensor("w2es", [HS, N], F32, kind="ExternalInput")
    b2e = nc.dram_tensor("b2e", [1, N], F32, kind="ExternalInput")
    w1p = nc.dram_tensor("w1p", [N, H], F32, kind="ExternalInput")
    w1tp_bf = nc.dram_tensor("w1tp_bf", [H, N], BF16, kind="ExternalInput")
    w1tn = nc.dram_tensor("w1tn", [P, N * KT_H], BF16, kind="ExternalInput")
    b1dp = nc.dram_tensor("b1dp", [1, H], F32, kind="ExternalInput")
    xmbt8 = nc.dram_tensor("xmbt8", [DS // 2, 2 * B], FP8, kind="ExternalInput")
    xmb = nc.dram_tensor("xmb", [B, DS], F32, kind="ExternalInput")
    sigw = nc.dram_tensor("sigw", [1, 130], F32, kind="ExternalInput")
    epsin = nc.dram_tensor("epsin", [B, N], F32, kind="ExternalInput")
    out = nc.dram_tensor("out", [B, 1], F32, kind="ExternalOutput")

    # ---- internal DRAM ----
    zstd = nc.dram_tensor("zstd", [B, N], F32)
    zst_sh = nc.dram_tensor("zst_sh", [B, N], F32, addr_space="Shared")
    g1d = nc.dram_tensor("g1d", [B, N * N], F32)
    g1_sh = nc.dram_tensor("g1_sh", [B, N * N], F32, addr_space="Shared")
    PKW = N + 1 + N * N   # 1057
    pkd = nc.dram_tensor("pkd", [B, PKW], F32)
    pk_sh = nc.dram_tensor("pk_sh", [B, PKW], F32, addr_space="Shared")

    with tile.TileContext(nc) as tc, ExitStack() as ctx:
        consts = ctx.enter_context(tc.tile_pool(name="consts", bufs=1))
        work = ctx.enter_context(tc.tile_pool(name="work", bufs=1))
        lin = ctx.enter_context(tc.tile_pool(name="lin", bufs=1))
        pre_cm = tc.tile_pool(name="pre_ps", bufs=2, space="PSUM")
        pre_ps = pre_cm.__enter__()

        # ---- constants ----
        identf = consts.tile([P, P], F32)
        make_identity(nc, identf)
        identb = consts.tile([P, P], BF16)
        make_identity(nc, identb)
        wu_d = nc.dram_tensor("wu_d", [1, 16], F32)
        wu_sh = nc.dram_tensor("wu_sh", [1, 16], F32, addr_space="Shared")
        nc.gpsimd.collective_compute("AllReduce", Alu.add, replica_groups=RG,
                                     ins=[wu_d[:]], outs=[wu_sh[:]])
        ones1 = consts.tile([1, B], F32)
        nc.vector.memset(ones1, 1.0)
        zeros2 = consts.tile([P, 2, B], F32)
        nc.vector.memset(zeros2, 0.0)
        sigw_sb = consts.tile([1, 130], F32)
        nc.sync.dma_start(sigw_sb, sigw[:])
        eps_sb = consts.tile([B, N], F32)
        nc.sync.dma_start(eps_sb, epsin[:])
        b1es_sb = consts.tile([1, HS], F32)
        nc.sync.dma_start(b1es_sb, b1es[:])
        b2e_sb = consts.tile([1, N], F32)
        nc.sync.dma_start(b2e_sb, b2e[:])
        b1d_sb = consts.tile([1, H], F32)
        nc.sync.dma_start(b1d_sb, b1dp[:])
        w2es_sb = consts.tile([P, 2, N], F32)
        nc.sync.dma_start(w2es_sb, w2es[:].rearrange("(k p) n -> p k n", p=P))
        w1_sb = consts.tile([N, H], F32)
        nc.sync.dma_start(w1_sb, w1p[:])
        w1Tb_sb = consts.tile([P, KT_H, N], BF16)
        nc.sync.dma_start(w1Tb_sb, w1tp_bf[:].rearrange("(k p) n -> p k n", p=P))
        w1tn_sb = consts.tile([P, N, KT_H], BF16)
        nc.sync.dma_start(w1tn_sb, w1tn[:])
        sgn_sb = consts.tile([B, N], F32)
        nc.vector.memset(sgn_sb, -1.0)
        nc.vector.memset(sub_ap(sgn_sb, 1, [[2, N // 2]]), 1.0)

        sigw_rep = consts.tile([B, 130], F32)
        sigw_ps = pre_ps.tile([B, 130], F32, tag="sp")
        nc.tensor.matmul(sigw_ps, ones1, sigw_sb, start=True, stop=True)
        nc.vector.tensor_copy(sigw_rep, sigw_ps)

        # sig_term replicated [B, N*N]
        st_ps = pre_ps.tile([N, N], F32, tag="sp")
        nc.tensor.matmul(st_ps, sigw_sb[:, 66:98], sigw_sb[:, 66:98],
                         start=True, stop=False)
        nc.tensor.matmul(st_ps, sigw_sb[:, 98:130], sigw_sb[:, 98:130],
                         start=False, stop=True)
        st_sb = work.tile([N, N], F32, tag="st_sb")
        nc.vector.tensor_copy(st_sb, st_ps)
        st_flat = work.tile([1, N * N], F32, tag="st_flat")
        nc.sync.dma_start(st_flat, st_sb)
        st_rep = consts.tile([B, N * N], F32)
        for hh in range(2):
            sps2 = pre_ps.tile([B, 512], F32, tag="sp")
            nc.tensor.matmul(sps2, ones1, st_flat[:, hh * 512:(hh + 1) * 512],
                             start=True, stop=True)
            nc.vector.tensor_copy(st_rep[:, hh * 512:(hh + 1) * 512], sps2)

        def emit_sig(z_in, name):
            lg = lin.tile([B, 2, N], F32, tag="sig_lg")
            nc.vector.tensor_tensor(
                lg, z_in.unsqueeze(1).broadcast_to([B, 2, N]),
                sigw_rep[:, 0:64].rearrange("p (c n) -> p c n", c=2), Alu.mult)
            red = lin.tile([B, 2], F32, tag=f"sig_red_{name}")
            nc.vector.tensor_reduce(red, lg, mybir.AxisListType.X, Alu.add)
            nc.vector.tensor_tensor(red, red, sigw_rep[:, 64:66], Alu.add)
            s = lin.tile([B, 2], F32, tag=f"sig_s_{name}")
            nc.scalar.activation(s, red, Act.Exp)
            return s

        # ================= encoder (model-parallel over enc-H) ==============
        with tc.tile_pool(name="p_enc_s", bufs=2) as enc_s, \
             tc.tile_pool(name="p_enc_ps", bufs=1, space="PSUM") as enc_ps:
            xt_r = xt[:].rearrange("(k p) b -> p k b", p=P)
            w1es_r = w1es[:].rearrange("(k p) h -> p k h", p=P)
            a1e_ps = enc_ps.tile([B, HS], F32, tag="a1e")
            for kb in range(12):
                xtile = enc_s.tile([P, 8, B], BF16, tag="xt_t")
                nc.scalar.dma_start(xtile, xt_r[:, kb * 8:(kb + 1) * 8, :])
                wtile = enc_s.tile([P, 8, HS], BF16, tag="w1es_t")
                nc.scalar.dma_start(wtile, w1es_r[:, kb * 8:(kb + 1) * 8, :])
                for j in range(8):
                    nc.tensor.matmul(a1e_ps, xtile[:, j, :], wtile[:, j, :],
                                     start=(kb == 0 and j == 0), stop=False)
            nc.tensor.matmul(a1e_ps, ones1[:, 0:B], b1es_sb,
                             start=False, stop=True)
            h1_sb = work.tile([B, HS], F32, tag="h1")
            nc.vector.tensor_scalar(h1_sb, a1e_ps, 0.0, None, Alu.max)
            h1T_sb = work.tile([P, 2, B], F32, tag="h1T")
            for i in range(2):
                tp = enc_ps.tile([P, B], F32, tag="tp")
                pe_T(nc, tp, h1_sb[:, i * P:(i + 1) * P], identf)
                nc.scalar.copy(h1T_sb[:, i, :], tp)
            zp_ps = enc_ps.tile([B, N], F32, tag="zp")
            for i in range(2):
                nc.tensor.matmul(zp_ps, h1T_sb[:, i, :], w2es_sb[:, i, :],
                                 start=(i == 0), stop=(i == 1))
            zp_sb = work.tile([B, N], F32, tag="zp_sb")
            nc.vector.tensor_copy(zp_sb, zp_ps)
            nc.sync.dma_start(zstd[:], zp_sb)
        nc.gpsimd.collective_compute("AllReduce", Alu.add, replica_groups=RG,
                                     ins=[zstd[:]], outs=[zst_sh[:]])

        pre_cm.__exit__(None, None, None)

        # ========= u = xmb @ W2s^T (partial, fp8 DoubleRow), |xmb|^2 =======
        KT2_D = KT_D // 2       # 48 double-row tiles
        KT2_DS = KT_DS // 2     # 6 own tiles
        cpool = ctx.enter_context(tc.tile_pool(name="cpool", bufs=1))
        c_bf = cpool.tile([P, 2, H], BF16, tag="c_bf")
        ckT_bf = cpool.tile([P, KT_H, 2 * P], BF16, tag="ckT")
        w2t_r = w2t8[:].rearrange("(k p) f -> p k f", p=P)
        w2res_cm = tc.tile_pool(name="w2res", bufs=1)
        w2res_pool = w2res_cm.__enter__()
        w2res = w2res_pool.tile([P, KT2_DS, 2, H], FP8, tag="w2res")
        for kt in range(KT2_DS):
            nc.sync.dma_start(
                w2res[:, kt, :, :],
                w2t_r[:, kt, :].rearrange("p (two h) -> p two h", two=2))
        xmbT_sb = work.tile([P, KT2_DS, 2, B], FP8, tag="xmbT")
        nc.sync.dma_start(
            xmbT_sb,
            xmbt8[:].rearrange("(k p) (two b) -> p k two b", p=P, two=2))
        xmb_sb = work.tile([B, DS], F32, tag="xmb")
        nc.sync.dma_start(xmb_sb, xmb[:])
        xmbsq = lin.tile([B, 1], F32, tag="xmbsq")
        scr2 = work.tile([B, H], BF16, tag="scr2")
        nc.scalar.activation(scr2[:, 0:DS], xmb_sb, Act.Square,
                             accum_out=xmbsq)
        u_sb = work.tile([B, H], BF16, tag="u_sb")
        DR = mybir.MatmulPerfMode.DoubleRow
        with tc.tile_pool(name="p_u", bufs=1, space="PSUM") as u_ps_pool:
            u_ps = u_ps_pool.tile([B, H], F32, tag="u_ps")
            for nb in range(4):
                for kt in range(KT2_DS):
                    nc.tensor.matmul(
                        u_ps[:, nb * 512:(nb + 1) * 512],
                        xmbT_sb[:, kt, :, :],
                        w2res[:, kt, :, nb * 512:(nb + 1) * 512],
                        start=(kt == 0), stop=(kt == KT2_DS - 1),
                        perf_mode=DR)
            nc.scalar.activation(u_sb, u_ps, Act.Copy, scale=1.0 / SC8)


        # ================= C' : C_k = W2[hs0,:] @ W2^T  (stream W2^T) =======
        pC_cm = tc.tile_pool(name="pC_s", bufs=3)
        pC = pC_cm.__enter__()
        pCp_cm = tc.tile_pool(name="pC_ps", bufs=1, space="PSUM")
        pCp = pCp_cm.__enter__()
        cps = pCp.tile([P, 8, 512], F32, tag="cps")
        for kt in range(KT2_D):
            if kt < KT2_DS:
                t_in = w2res[:, kt, :, :]
            else:
                t_raw = pC.tile([P, 2, H], FP8, tag="w2_t")
                dq = nc.sync if kt % 2 == 0 else nc.scalar
                dq.dma_start(
                    t_raw,
                    w2t_r[:, kt, :].rearrange("p (two h) -> p two h", two=2))
                t_in = t_raw
            for it in range(2):
                for nb in range(4):
                    nc.tensor.matmul(
                        cps[:, it * 4 + nb, :],
                        t_in[:, :, it * P:(it + 1) * P],
                        t_in[:, :, nb * 512:(nb + 1) * 512],
                        start=(kt == 0), stop=(kt == KT2_D - 1),
                        perf_mode=DR)
        for it in range(2):
            for nb in range(4):
                nc.scalar.activation(c_bf[:, it, nb * 512:(nb + 1) * 512],
                                     cps[:, it * 4 + nb, :], Act.Copy,
                                     scale=1.0 / (SC8 * SC8))
        pCp_cm.__exit__(None, None, None)
        pC_cm.__exit__(None, None, None)
        w2res_cm.__exit__(None, None, None)
        # C_k^T via PE transposes of 128x128 blocks
        with tc.tile_pool(name="p_ct", bufs=2, space="PSUM") as ct_ps_pool:
            for it in range(2):
                for jt in range(KT_H):
                    tp = ct_ps_pool.tile([P, P], BF16, tag="ct")
                    pe_T(nc, tp, c_bf[:, it, jt * P:(jt + 1) * P], identb)
                    nc.scalar.copy(ckT_bf[:, jt, it * P:(it + 1) * P], tp)

        # ---- z* full (+enc b2), zT, sig1, masks m1 (post-C') ----
        zf_sb = work.tile([B, N], F32, tag="zf")
        nc.sync.dma_start(zf_sb, zst_sh[:])
        zall = lin.tile([B, N], F32, tag="zall")
        zT_sb = work.tile([N, B], F32, tag="zT")
        with tc.tile_pool(name="p_z", bufs=2, space="PSUM") as pz:
            za_ps = pz.tile([B, N], F32, tag="za")
            nc.tensor.matmul(za_ps, ones1[:, 0:B], b2e_sb, start=True, stop=False)
            nc.tensor.matmul(za_ps, identf[0:B, 0:B], zf_sb, start=False,
                             stop=True)
            nc.vector.tensor_copy(zall, za_ps)
            zT_ps = pz.tile([N, B], F32, tag="za")
            pe_T(nc, zT_ps, zall, identf)
            nc.vector.tensor_copy(zT_sb, zT_ps)
        s1 = emit_sig(zall, "s1")
        invsp2 = lin.tile([B, 1], F32, tag="invsp2")
        sp2t = lin.tile([B, 1], F32, tag="sp2t")
        nc.vector.tensor_tensor(sp2t, s1[:, 0:1], s1[:, 0:1], Alu.mult)
        nc.vector.reciprocal(invsp2, sp2t)
        m1T_bf = work.tile([P, KT_H, B], BF16, tag="m1T")
        with tc.tile_pool(name="p_a1t", bufs=1, space="PSUM") as a1t_pool:
            a1T_ps = a1t_pool.tile([P, KT_H, B], F32, tag="a1T")
            for mt in range(KT_H):
                nc.tensor.matmul(a1T_ps[:, mt, :],
                                 w1_sb[:, mt * P:(mt + 1) * P], zT_sb,
                                 start=True, stop=False)
                nc.tensor.matmul(a1T_ps[:, mt, :],
                                 b1d_sb[:, mt * P:(mt + 1) * P], ones1[:, 0:B],
                                 start=False, stop=True)
            nc.vector.tensor_scalar(m1T_bf, a1T_ps, 0.0, None, Alu.is_gt)

        # ================= G partials (shared emitter) ======================
        at_pool = ctx.enter_context(tc.tile_pool(name="at_pool", bufs=2))
        g_pool = ctx.enter_context(tc.tile_pool(name="g_pool", bufs=1))

        def emit_G(mT_bf, tag):
            g_sb = g_pool.tile([N, B * N], F32, tag=f"g_{tag}")
            with tc.tile_pool(name=f"pG{tag}", bufs=2, space="PSUM") as gps_pool:
                for cb in range(4):
                    AT = at_pool.tile([P, KT_H, 16, N], BF16, tag="AT")
                    nc.vector.tensor_tensor(
                        AT,
                        w1Tb_sb.unsqueeze(2).broadcast_to([P, KT_H, 16, N]),
                        mT_bf[:, :, cb * 16:(cb + 1) * 16]
                        .unsqueeze(3).broadcast_to([P, KT_H, 16, N]),
                        Alu.mult)
                    m1ps = gps_pool.tile([P, 2, 512], F32, tag="m1ps")
                    for it in range(2):
                        for jt in range(KT_H):
                            nc.tensor.matmul(
                                m1ps[:, it, :],
                                ckT_bf[:, jt, it * P:(it + 1) * P],
                                AT[:, jt, :, :],
                                start=(jt == 0), stop=(jt == KT_H - 1))
                    m1sb = at_pool.tile([P, 2, 512], BF16, tag="m1sb")
                    nc.scalar.copy(m1sb, m1ps)
                    gps = gps_pool.tile([N, 512], F32, tag="gps")
                    for s in range(16):
                        for it in range(2):
                            nc.tensor.matmul(
                                gps[:, s * N:(s + 1) * N],
                                AT[:, it, s, :],
                                m1sb[:, it, s * N:(s + 1) * N],
                                start=(it == 0), stop=(it == 1))
                    nc.vector.tensor_copy(
                        g_sb[:, cb * 512:(cb + 1) * 512], gps)
            return g_sb

        # ---- G1 -> pack -> AllReduce ----
        g1_sb = emit_G(m1T_bf, "1")
        nc.sync.dma_start(
            dram_ap(g1d, 0, [[N, N], [N * N, B], [1, N]]), g1_sb)
        nc.gpsimd.collective_compute("AllReduce", Alu.add, replica_groups=RG,
                                     ins=[g1d[:]], outs=[g1_sh[:]])

        # ---- Prec assembly + LDLT + ltinv + dz + z_s ----
        Tm = lin.tile([B, N * N], F32, tag="Tm")
        nc.sync.dma_start(Tm, g1_sh[:])
        nc.vector.tensor_scalar(Tm, Tm, invsp2, None, Alu.mult)
        nc.vector.tensor_tensor(Tm, Tm, st_rep, Alu.add)
        diag1 = sub_ap(Tm, 0, [[N + 1, N]])
        nc.vector.tensor_scalar(diag1, diag1, 1.0, None, Alu.add)

        invD = lin.tile([B, N], F32, tag="invD")
        SCR = lin.tile([B, N * N], F32, tag="SCR")
        emit_ldlt(nc, Tm, SCR, invD, B)
        LT = lin.tile([B, N * N], F32, tag="LT")
        nc.vector.tensor_tensor(
            LT.rearrange("p (a b) -> p a b", b=N),
            Tm.rearrange("p (a b) -> p a b", b=N),
            invD.unsqueeze(1).broadcast_to([B, N, N]), Alu.mult)
        # dz: solve Lt^T dz = epss by backward substitution, one STT per
        # column: stored[0:k] <- (Ltrow_k * stored[k]) - stored[0:k], which
        # leaves stored[j] = (-1)^(N-1-j) dz[j]; fixed up with sgn_sb.
        srD = lin.tile([B, N], F32, tag="srD")
        nc.scalar.activation(srD, invD, Act.Sqrt)        # 1/sqrt(D)
        dz = lin.tile([B, N], F32, tag="dz")
        nc.vector.tensor_tensor(dz, eps_sb, srD, Alu.mult)
        for k in range(N - 1, 0, -1):
            lrow = sub_ap(LT, k * N, [[1, k]])
            nc.vector.scalar_tensor_tensor(
                dz[:, 0:k], lrow, dz[:, k:k + 1], dz[:, 0:k],
                Alu.mult, Alu.subtract)
        zs = lin.tile([B, N], F32, tag="zs")
        nc.vector.tensor_tensor(dz, dz, sgn_sb, Alu.mult)
        nc.vector.tensor_tensor(zs, zall, dz, Alu.add)
        s2 = emit_sig(zs, "s2")

        # ================= stage 2 ==========================================
        zsT_sb = work.tile([N, B], F32, tag="zsT")
        with tc.tile_pool(name="p_zst", bufs=1, space="PSUM") as zst_pool:
            zsT_ps = zst_pool.tile([N, B], F32, tag="zsT_ps")
            pe_T(nc, zsT_ps, zs, identf)
            nc.vector.tensor_copy(zsT_sb, zsT_ps)

        m2T_bf = work.tile([P, KT_H, B], BF16, tag="m2T")
        h2neg = work.tile([P, 2, B], BF16, tag="h2neg")
        with tc.tile_pool(name="p_a2t", bufs=1, space="PSUM") as a2t_pool:
            a2T_ps = a2t_pool.tile([P, KT_H, B], F32, tag="a2T")
            for mt in range(KT_H):
                nc.tensor.matmul(a2T_ps[:, mt, :],
                                 w1_sb[:, mt * P:(mt + 1) * P], zsT_sb,
                                 start=True, stop=False)
                nc.tensor.matmul(a2T_ps[:, mt, :],
                                 b1d_sb[:, mt * P:(mt + 1) * P], ones1[:, 0:B],
                                 start=False, stop=True)
            nc.vector.tensor_scalar(m2T_bf, a2T_ps, 0.0, None, Alu.is_gt)
            # -relu(a2) for local block (first 2 kt): min(-a2, 0)
            nc.vector.scalar_tensor_tensor(
                h2neg, a2T_ps[:, 0:2, :], -1.0, zeros2, Alu.mult, Alu.min)

        h2_sb = work.tile([B, H], BF16, tag="h2")
        with tc.tile_pool(name="p_a2", bufs=1, space="PSUM") as a2_pool:
            a2_ps = a2_pool.tile([B, H], F32, tag="a2")
            for nb in range(4):
                nc.tensor.matmul(a2_ps[:, nb * 512:(nb + 1) * 512],
                                 zsT_sb, w1_sb[:, nb * 512:(nb + 1) * 512],
                                 start=True, stop=False)
                nc.tensor.matmul(a2_ps[:, nb * 512:(nb + 1) * 512],
                                 ones1[:, 0:B], b1d_sb[:, nb * 512:(nb + 1) * 512],
                                 start=False, stop=True)
            nc.vector.tensor_scalar(h2_sb, a2_ps, 0.0, None, Alu.max)

        uh2 = lin.tile([B, 1], F32, tag="uh2")
        nc.vector.tensor_tensor(scr2, u_sb, h2_sb, Alu.mult)
        nc.vector.tensor_reduce(uh2, scr2, mybir.AxisListType.X, Alu.add)

        wd_sb = work.tile([B, H], BF16, tag="wd")
        vneg_bf = work.tile([B, H], BF16, tag="vneg")
        vh2m = lin.tile([B, 1], F32, tag="vh2m")
        with tc.tile_pool(name="p_v", bufs=1, space="PSUM") as v_pool:
            v_ps = v_pool.tile([B, H], F32, tag="v_ps")   # holds -v
            for nb in range(4):
                for it in range(2):
                    nc.tensor.matmul(v_ps[:, nb * 512:(nb + 1) * 512],
                                     h2neg[:, it, :],
                                     c_bf[:, it, nb * 512:(nb + 1) * 512],
                                     start=(it == 0), stop=(it == 1))
            nc.scalar.copy(vneg_bf, v_ps)
        nc.vector.tensor_tensor(scr2, vneg_bf, h2_sb, Alu.mult)
        nc.vector.tensor_reduce(vh2m, scr2, mybir.AxisListType.X, Alu.add)
        nc.vector.tensor_tensor(wd_sb, u_sb, vneg_bf, Alu.add)
        svec = lin.tile([B, 1], F32, tag="svec")
        nc.vector.scalar_tensor_tensor(svec, uh2, -2.0, xmbsq, Alu.mult, Alu.add)
        nc.vector.tensor_tensor(svec, svec, vh2m, Alu.subtract)

        # t_k = A2 wd_k : transpose wd, mask, matmul
        wdT_sb = work.tile([P, KT_H, B], BF16, tag="wdT")
        with tc.tile_pool(name="p_wdt", bufs=2, space="PSUM") as wdt_pool:
            for jt in range(KT_H):
                tp = wdt_pool.tile([P, B], BF16, tag="wdt")
                pe_T(nc, tp, wd_sb[:, jt * P:(jt + 1) * P], identb)
                nc.scalar.copy(wdT_sb[:, jt, :], tp)
        mwdT = work.tile([P, KT_H, B], BF16, tag="mwdT")
        nc.vector.tensor_tensor(mwdT, wdT_sb, m2T_bf, Alu.mult)
        tk_sb = work.tile([N, B], F32, tag="tk")
        with tc.tile_pool(name="p_tk", bufs=1, space="PSUM") as tk_pool:
            tk_ps = tk_pool.tile([N, B], F32, tag="tk_ps")
            for jt in range(KT_H):
                nc.tensor.matmul(tk_ps, w1Tb_sb[:, jt, :], mwdT[:, jt, :],
                                 start=(jt == 0), stop=(jt == KT_H - 1))
            nc.vector.tensor_copy(tk_sb, tk_ps)

        # ---- G2 + pack [t | svec | G2] -> AllReduce ----
        g2_sb = emit_G(m2T_bf, "2")
        nc.sync.dma_start(dram_ap(pkd, 0, [[1, N], [PKW, B]]), tk_sb)
        nc.sync.dma_start(dram_ap(pkd, N, [[PKW, B]]), svec)
        nc.sync.dma_start(
            dram_ap(pkd, N + 1, [[N, N], [PKW, B], [1, N]]), g2_sb)
        nc.gpsimd.collective_compute("AllReduce", Alu.add, replica_groups=RG,
                                     ins=[pkd[:]], outs=[pk_sh[:]])

        # ---- background (overlaps AllReduce): ltinv, tr, logdet, latent ----
        X1 = lin.tile([B, N * N], F32, tag="X1")
        nc.vector.memset(X1, 0.0)
        nc.vector.memset(sub_ap(X1, 0, [[N + 1, N]]), 1.0)
        emit_ltinv(nc, LT, X1, SCR, B)
        scrB = lin.tile([B, N * N], F32, tag="scrB")
        nc.vector.tensor_tensor(SCR, X1, X1, Alu.mult)
        trv = lin.tile([B, 1], F32, tag="trv")
        nc.vector.tensor_tensor(
            scrB.rearrange("p (a b) -> p a b", b=N),
            SCR.rearrange("p (a b) -> p a b", b=N),
            invD.unsqueeze(2).broadcast_to([B, N, N]), Alu.mult)
        nc.vector.tensor_reduce(trv, scrB, mybir.AxisListType.X, Alu.add)
        logs = lin.tile([B, N], F32, tag="logs")
        ldv = lin.tile([B, 1], F32, tag="ldv")
        nc.scalar.activation(logs, invD, Act.Ln)
        nc.vector.tensor_reduce(ldv, logs, mybir.AxisListType.X, Alu.add)
        zsq = lin.tile([B, N], F32, tag="zsq")
        latv = lin.tile([B, 1], F32, tag="latv")
        nc.vector.tensor_tensor(zsq, zall, zall, Alu.mult)
        nc.vector.tensor_reduce(latv, zsq, mybir.AxisListType.X, Alu.add)
        nc.vector.tensor_tensor(latv, latv, trv, Alu.add)
        nc.vector.tensor_scalar(latv, latv, 0.5, None, Alu.mult)
        nc.vector.tensor_scalar(ldv, ldv, -0.5, None, Alu.mult)

        # ---- post-AllReduce: solve G2 y = t, d_proj, recon, out ----
        y = lin.tile([B, N], F32, tag="y")
        nc.sync.dma_start(y, pk_sh[:, 0:N])
        svf = lin.tile([B, 1], F32, tag="svf")
        nc.sync.dma_start(svf, pk_sh[:, N:N + 1])
        Tm2 = lin.tile([B, N * N], F32, tag="Tm2")
        nc.sync.dma_start(Tm2, pk_sh[:, N + 1:PKW])
        invD2 = lin.tile([B, N], F32, tag="invD2")
        emit_ldlt(nc, Tm2, SCR, invD2, B)
        LT2 = lin.tile([B, N * N], F32, tag="LT2")
        nc.vector.tensor_tensor(
            LT2.rearrange("p (a b) -> p a b", b=N),
            Tm2.rearrange("p (a b) -> p a b", b=N),
            invD2.unsqueeze(1).broadcast_to([B, N, N]), Alu.mult)
        emit_fwd_solve_alt(nc, LT2, y, B)
        ysq = lin.tile([B, N], F32, tag="ysq")
        yw = lin.tile([B, N], F32, tag="yw")
        dproj = lin.tile([B, 1], F32, tag="dproj")
        nc.vector.tensor_tensor(ysq, y, y, Alu.mult)
        nc.vector.tensor_tensor(yw, ysq, invD2, Alu.mult)
        nc.vector.tensor_reduce(dproj, yw, mybir.AxisListType.X, Alu.add)

        sq2 = lin.tile([B, 2], F32, tag="sq2")
        nc.vector.tensor_tensor(sq2, s2, s2, Alu.mult)
        nc.vector.tensor_scalar(sq2, sq2, 2.0, None, Alu.mult)
        inv2 = lin.tile([B, 2], F32, tag="inv2")
        nc.vector.reciprocal(inv2, sq2)     # [1/(2sp2^2), 1/(2sv2^2)]
        logs2 = lin.tile([B, 2], F32, tag="logs2")
        logw = lin.tile([B, 2], F32, tag="logw")
        nc.vector.memset(logw[:, 0:1], float(N))
        nc.vector.memset(logw[:, 1:2], float(D - N))
        nc.scalar.activation(logs2, s2, Act.Ln)
        logterm = lin.tile([B, 1], F32, tag="logterm")
        junk2 = lin.tile([B, 2], F32, tag="junk2")
        nc.vector.tensor_tensor(junk2, logs2, logw, Alu.mult)
        nc.vector.tensor_reduce(logterm, junk2, mybir.AxisListType.X, Alu.add)
        isub = lin.tile([B, 1], F32, tag="isub")
        nc.vector.tensor_tensor(isub, inv2[:, 0:1], inv2[:, 1:2], Alu.subtract)
        recon = lin.tile([B, 1], F32, tag="recon")
        nc.vector.tensor_tensor(recon, dproj, isub, Alu.mult)
        p2t = lin.tile([B, 1], F32, tag="p2t")
        nc.vector.tensor_tensor(p2t, svf, inv2[:, 1:2], Alu.mult)
        nc.vector.tensor_tensor(recon, recon, p2t, Alu.add)
        nc.vector.tensor_tensor(recon, recon, logterm, Alu.add)
        ov = lin.tile([B, 1], F32, tag="ov")
        nc.vector.tensor_tensor(ov, recon, latv, Alu.add)
        nc.vector.tensor_tensor(ov, ov, ldv, Alu.add)
        nc.vector.tensor_scalar(ov, ov, 1.0 / D, None, Alu.mult)
        nc.sync.dma_start(out[:], ov)

    legalize_waits(nc)
    return nc


def shard_inputs(inputs):
    """Host-side prep: per-core H-permutation + D-tile reordering."""
    bf = ml_dtypes.bfloat16
    x = np.ascontiguousarray(np.asarray(inputs["x"], np.float32))
    eps = np.ascontiguousarray(np.asarray(inputs["eps"], np.float32))
    eW1 = np.asarray(inputs["enc_W1"], np.float32)
    eb1 = np.asarray(inputs["enc_b1"], np.float32)
    eW2 = np.asarray(inputs["enc_W2"], np.float32)
    eb2 = np.asarray(inputs["enc_b2"], np.float32)
    dW1 = np.asarray(inputs["dec_W1"], np.float32)
    db1 = np.asarray(inputs["dec_b1"], np.float32)
    dW2 = np.asarray(inputs["dec_W2"], np.float32)
    db2 = np.asarray(inputs["dec_b2"], np.float32)
    sW = np.asarray(inputs["sig_W"], np.float32)
    sb = np.asarray(inputs["sig_b"], np.float32)

    xT_bf = np.ascontiguousarray(x.T).astype(bf)
    xmb_full = x - db2[None, :]
    W2T = np.ascontiguousarray(dW2.T)      # [D, H]
    sigv = np.zeros((1, 130), np.float32)
    sigv[0, 0:32] = sW[:, 0]
    sigv[0, 32:64] = sW[:, 1]
    sigv[0, 64:66] = sb
    sigv[0, 66:98] = sW[:, 0] * np.sqrt(N / 2.0)
    sigv[0, 98:130] = sW[:, 1] * np.sqrt((D - N) / 2.0)

    maps = []
    for k in range(NCORES):
        hperm = np.concatenate([np.arange(k * HS, (k + 1) * HS),
                                np.arange(0, k * HS),
                                np.arange((k + 1) * HS, H)])
        # D-tile order: own 12 tiles first
        own = np.arange(k * KT_DS, (k + 1) * KT_DS)
        rest = np.concatenate([np.arange(0, k * KT_DS),
                               np.arange((k + 1) * KT_DS, KT_D)])
        tord = np.concatenate([own, rest])
        w2t_p = W2T[:, hperm].reshape(KT_D, P, H)[tord].reshape(D, H)
        w2q = np.asarray(w2t_p * 256.0, np.float32).astype(ml_dtypes.float8_e4m3)
        w2q = np.ascontiguousarray(
            w2q.reshape(KT_D // 2, 2, P, H).transpose(0, 2, 1, 3)
            .reshape(D // 2, 2 * H))
        dsl = slice(k * DS, (k + 1) * DS)
        xq = np.asarray(xmb_full[:, dsl].T, np.float32).astype(
            ml_dtypes.float8_e4m3)
        xq = np.ascontiguousarray(
            xq.reshape(KT_DS // 2, 2, P, B).transpose(0, 2, 1, 3)
            .reshape(DS // 2, 2 * B))
        maps.append({
            "w2t8": w2q,
            "xt": xT_bf,
            "w1es": np.ascontiguousarray(
                eW1[:, k * HS:(k + 1) * HS]).astype(bf),
            "b1es": np.ascontiguousarray(eb1[None, k * HS:(k + 1) * HS]),
            "w2es": np.ascontiguousarray(eW2[k * HS:(k + 1) * HS, :]),
            "b2e": np.ascontiguousarray(eb2[None, :]),
            "w1p": np.ascontiguousarray(dW1[:, hperm]),
            "w1tp_bf": np.ascontiguousarray(dW1[:, hperm].T).astype(bf),
            "w1tn": np.ascontiguousarray(
                dW1[:, hperm].reshape(N, KT_H, P).transpose(2, 0, 1)
                .reshape(P, N * KT_H)).astype(bf),
            "b1dp": np.ascontiguousarray(db1[None, hperm]),
            "xmbt8": xq,
            "xmb": np.ascontiguousarray(xmb_full[:, dsl]),
            "sigw": sigv,
            "epsin": eps,
        })
    return maps


_NC_CACHE = None


def kernel(**inputs) -> np.ndarray:
    global _NC_CACHE
    from concourse.bass_utils import run_bass_kernel_spmd
    if _NC_CACHE is None:
        _NC_CACHE = build_nc()
    nc = _NC_CACHE
    maps = shard_inputs(inputs)
    res = run_bass_kernel_spmd(nc, maps, list(range(NCORES)))
    return np.asarray(res.results[0]["out"]).reshape(B).astype(np.float32)


# revision 14
# speedup vs baseline: 1.3741x; 1.1661x over previous
"""Trainium2 Bass kernel for nn_EnergyAE (B=64, D=12288, N=32, H=2048) on 8 cores.

v2 restructure (vs v1's contraction-sharded C + 16.8MB AllReduce):
  - C block-row sharding: core k computes C_k = C[hs_k, :] (256 x 2048) locally
    by streaming full W2^T (bf16, 12.6MB); C_k stays in SBUF. No C collective,
    no C DRAM round trip.
  - Per-core H-permutation (host-side) puts each core's block at H-cols 0:256,
    so the SPMD program is core-index-free.
  - G[b] computed as partial sums over ALL 64 samples using C_k; AllReduce of
    packed G (256KB). Every core then factors all 64 (same vector cost) ->
    no z_s AllGather, no sel8 gathers.
  - x_star never materialized: W2 delta = W2(x-b2) - C h2, so
    u = xmb @ W2s^T (partial), v = h2[:, 0:256] @ C_k (partial),
    wd_k = u_k - v_k, t_k = A2 wd_k (pre-AllReduce!),
    d_sq = |xmb|^2 - 2 u.h2 + v.h2 (scalar partials).
  - Final AllReduce packs [t | svec | G2] = 271KB. All cores compute all 64
    outputs; host takes core 0's.

Identities (validated numerically, rel err ~1.7e-3 with bf16 C/A/u):
  Prec = Lt D Lt^T (unit-lower LDLT)
  sum(log eig)/2 = 0.5*sum(log D);  sum(1/eig) = ||D^-1/2 Lt^-1||_F^2
  U^-1 eps = Lt^-T (eps/sqrt(D));   t^T G2^-1 t = ||D2^-1/2 Lt2^-1 t||^2
  sig_term = (n w0 w0^T + (D-n) w1 w1^T)/2   (constant across batch)
"""
import sys

for _p in ("/opt/trn_rl_repo", "/root/.axon_site/_ro/trn_rl_repo"):
    if _p not in sys.path:
        sys.path.append(_p)

import numpy as np
import ml_dtypes
from contextlib import ExitStack

import concourse.bass as bass
import concourse.mybir as mybir
import concourse.tile as tile
from concourse.masks import make_identity

B, D, N, H = 64, 12288, 32, 2048
NCORES = 8
BL = B // NCORES          # 8 local samples (only used for host sharding)
HS = H // NCORES          # 256: C block rows per core
DS = D // NCORES          # 1536
KT_D = D // 128           # 96
KT_DS = DS // 128         # 12
KT_H = H // 128           # 16
P = 128

F32 = mybir.dt.float32
F32R = mybir.dt.float32r
BF16 = mybir.dt.bfloat16
FP8 = mybir.dt.float8e4
SC8 = 256.0
Alu = mybir.AluOpType
Act = mybir.ActivationFunctionType
RG = [list(range(NCORES))]


def sub_ap(t, extra_off, dims):
    """Custom free-dim AP on a [P, F] tile; dims = [[step,count],...] in elems."""
    base = t[:, 0:1]
    return bass.AP(base.tensor, base.offset + extra_off, [base.ap[0]] + dims)


def dram_ap(t, off, dims):
    """Custom AP into a DRAM tensor; dims = [[step,count],...] in elems."""
    base = t[:]
    return bass.AP(base.tensor, off, dims)


def pe_T(nc, out_ps, in_ap, ident):
    """PE transpose: out_ps [f, p] = in_ap [p, f].T"""
    kp = in_ap.shape[0]
    nc.tensor.transpose(out_ps, in_ap, ident[0:kp, 0:kp])


def emit_ldlt(nc, T, OUT, invD, rows, n=32):
    """In-place unit-lower LDLT of T [rows, n*n] (row-major per sample).
    After: strict lower of T holds unscaled columns u; diag holds D; invD=1/D."""
    for j in range(n):
        nc.vector.reciprocal(invD[:, j:j + 1], T[:, (n + 1) * j:(n + 1) * j + 1])
        m = n - 1 - j
        if m == 0:
            break
        base = (j + 1) * n + j
        u_i = sub_ap(T, base, [[n, m], [0, m]])
        u_k = sub_ap(T, base, [[0, m], [n, m]])
        outer = sub_ap(OUT, 0, [[m, m], [1, m]])
        nc.vector.scalar_tensor_tensor(
            outer, u_i, invD[:, j:j + 1], u_k, Alu.mult, Alu.mult)
        trail = sub_ap(T, (j + 1) * (n + 1), [[n, m], [1, m]])
        nc.vector.tensor_tensor(trail, trail, outer, Alu.subtract)


def emit_ltinv(nc, LT, X, OUT, rows, n=32):
    """X = LT^{-1} for unit-lower LT [rows, n*n]; X preset to I by caller."""
    for k in range(n - 1):
        rr = n - 1 - k
        cols = k + 1
        lcol = sub_ap(LT, (k + 1) * n + k, [[n, rr], [0, cols]])
        xrow = sub_ap(X, k * n, [[0, rr], [1, cols]])
        prod = sub_ap(OUT, 0, [[cols, rr], [1, cols]])
        nc.vector.scalar_tensor_tensor(prod, lcol, -1.0, xrow, Alu.mult, Alu.mult)
        xblk = sub_ap(X, (k + 1) * n, [[n, rr], [1, cols]])
        nc.vector.tensor_tensor(xblk, xblk, prod, Alu.add)


def emit_fwd_solve_alt(nc, LT, w, rows, n=32):
    """Forward-substitute LT y = w in place, ONE STT per column:
      stored[k+1:] <- (LT[k+1:,k] * stored[k]) - stored[k+1:]
    This leaves stored[j] = (-1)^j * y[j] (every update flips the sign of the
    remaining entries, and the scalar operand carries the matching sign), so
    it is valid whenever the caller only consumes y elementwise-squared."""
    for k in range(n - 1):
        rr = n - 1 - k
        lcol = sub_ap(LT, (k + 1) * n + k, [[n, rr]])
        nc.vector.scalar_tensor_tensor(
            w[:, k + 1:n], lcol, w[:, k:k + 1], w[:, k + 1:n],
            Alu.mult, Alu.subtract)


def emit_fwd_solve(nc, LT, y, OUT, rows, n=32):
    """y <- LT^{-1} y for unit-lower LT [rows, n*n], y [rows, n] in place."""
    for k in range(n - 1):
        rr = n - 1 - k
        lcol = sub_ap(LT, (k + 1) * n + k, [[n, rr]])
        nc.vector.scalar_tensor_tensor(
            OUT[:, 0:rr], lcol, -1.0, y[:, k:k + 1].broadcast_to([rows, rr]),
            Alu.mult, Alu.mult)
        nc.vector.tensor_tensor(y[:, k + 1:n], y[:, k + 1:n], OUT[:, 0:rr], Alu.add)


def legalize_waits(nc, maxw=1):
    """Split multi-wait sync_info into standalone EventSemaphore instructions."""
    for f in nc.m.functions:
        for bb in f.blocks:
            insts = list(bb.instructions)
            out = []
            changed = False
            for inst in insts:
                si = inst.sync_info
                if si is not None and si.on_wait and len(si.on_wait) > maxw:
                    waits = list(si.on_wait)
                    imm = [w for w in waits if w.uses_immediate]
                    reg = [w for w in waits if not w.uses_immediate]
                    keep = (reg + imm)[:maxw] if len(reg) <= maxw else reg
                    extra = [w for w in waits if w not in keep]
                    if len(keep) > maxw:
                        raise RuntimeError(f"{inst.name}: {len(keep)} register waits")
                    for w in extra:
                        ev = mybir.InstEventSemaphore(
                            name=nc.get_next_instruction_name(), ins=[], outs=[])
                        ev.engine = inst.engine
                        ev.sync_info = mybir.SyncInfo(on_wait=[w], on_update=[])
                        out.append(ev)
                    inst.sync_info = mybir.SyncInfo(
                        on_wait=keep, on_update=list(si.on_update or []))
                    changed = True
                out.append(inst)
            if changed:
                bb.instructions = out
    return nc


def build_nc():
    nc = bass.Bass()

    # ---- I/O (per-core views prepared by host; H-permuted, D-tile-reordered)
    w2t8 = nc.dram_tensor("w2t8", [D // 2, 2 * H], FP8, kind="ExternalInput")
    xt = nc.dram_tensor("xt", [D, B], BF16, kind="ExternalInput")
    w1es = nc.dram_tensor("w1es", [D, HS], BF16, kind="ExternalInput")
    b1es = nc.dram_tensor("b1es", [1, HS], F32, kind="ExternalInput")
    w2es = nc.dram_tensor("w2es", [HS, N], F32, kind="ExternalInput")
    b2e = nc.dram_tensor("b2e", [1, N], F32, kind="ExternalInput")
    w1p = nc.dram_tensor("w1p", [N, H], F32, kind="ExternalInput")
    w1tp_bf = nc.dram_tensor("w1tp_bf", [H, N], BF16, kind="ExternalInput")
    w1tp8 = nc.dram_tensor("w1tp8", [H, N], FP8, kind="ExternalInput")
    b1dp = nc.dram_tensor("b1dp", [1, H], F32, kind="ExternalInput")
    xmbt8 = nc.dram_tensor("xmbt8", [DS // 2, 2 * B], FP8, kind="ExternalInput")
    xmb = nc.dram_tensor("xmb", [B, DS], F32, kind="ExternalInput")
    sigw = nc.dram_tensor("sigw", [1, 130], F32, kind="ExternalInput")
    epsin = nc.dram_tensor("epsin", [B, N], F32, kind="ExternalInput")
    out = nc.dram_tensor("out", [B, 1], F32, kind="ExternalOutput")

    # ---- internal DRAM ----
    zstd = nc.dram_tensor("zstd", [B, N], F32)
    zst_sh = nc.dram_tensor("zst_sh", [B, N], F32, addr_space="Shared")
    g1d = nc.dram_tensor("g1d", [B, N * N], F32)
    g1_sh = nc.dram_tensor("g1_sh", [B, N * N], F32, addr_space="Shared")
    PKW = N + 1 + N * N   # 1057
    pkd = nc.dram_tensor("pkd", [B, PKW], F32)
    pk_sh = nc.dram_tensor("pk_sh", [B, PKW], F32, addr_space="Shared")

    with tile.TileContext(nc) as tc, ExitStack() as ctx:
        consts = ctx.enter_context(tc.tile_pool(name="consts", bufs=1))
        work = ctx.enter_context(tc.tile_pool(name="work", bufs=1))
        lin = ctx.enter_context(tc.tile_pool(name="lin", bufs=1))
        pre_cm = tc.tile_pool(name="pre_ps", bufs=2, space="PSUM")
        pre_ps = pre_cm.__enter__()

        # ---- constants ----
        identf = consts.tile([P, P], F32)
        make_identity(nc, identf)
        identb = consts.tile([P, P], BF16)
        make_identity(nc, identb)
        wu_d = nc.dram_tensor("wu_d", [1, 16], F32)
        wu_sh = nc.dram_tensor("wu_sh", [1, 16], F32, addr_space="Shared")
        nc.gpsimd.collective_compute("AllReduce", Alu.add, replica_groups=RG,
                                     ins=[wu_d[:]], outs=[wu_sh[:]])
        ones1 = consts.tile([1, B], F32)
        nc.vector.memset(ones1, 1.0)
        zeros2 = consts.tile([P, 2, B], F32)
        nc.vector.memset(zeros2, 0.0)
        sigw_sb = consts.tile([1, 130], F32)
        nc.sync.dma_start(sigw_sb, sigw[:])
        eps_sb = consts.tile([B, N], F32)
        nc.sync.dma_start(eps_sb, epsin[:])
        b1es_sb = consts.tile([1, HS], F32)
        nc.sync.dma_start(b1es_sb, b1es[:])
        b2e_sb = consts.tile([1, N], F32)
        nc.sync.dma_start(b2e_sb, b2e[:])
        b1d_sb = consts.tile([1, H], F32)
        nc.sync.dma_start(b1d_sb, b1dp[:])
        w2es_sb = consts.tile([P, 2, N], F32)
        nc.sync.dma_start(w2es_sb, w2es[:].rearrange("(k p) n -> p k n", p=P))
        w1_sb = consts.tile([N, H], F32)
        nc.sync.dma_start(w1_sb, w1p[:])
        w1Tb_sb = consts.tile([P, KT_H, N], BF16)
        nc.sync.dma_start(w1Tb_sb, w1tp_bf[:].rearrange("(k p) n -> p k n", p=P))
        w1T8_sb = consts.tile([P, KT_H, N], FP8)
        nc.sync.dma_start(w1T8_sb, w1tp8[:].rearrange("(k p) n -> p k n", p=P))
        sgn_sb = consts.tile([B, N], F32)
        nc.vector.memset(sgn_sb, -1.0)
        nc.vector.memset(sub_ap(sgn_sb, 1, [[2, N // 2]]), 1.0)

        sigw_rep = consts.tile([B, 130], F32)
        sigw_ps = pre_ps.tile([B, 130], F32, tag="sp")
        nc.tensor.matmul(sigw_ps, ones1, sigw_sb, start=True, stop=True)
        nc.vector.tensor_copy(sigw_rep, sigw_ps)

        # sig_term replicated [B, N*N]
        st_ps = pre_ps.tile([N, N], F32, tag="sp")
        nc.tensor.matmul(st_ps, sigw_sb[:, 66:98], sigw_sb[:, 66:98],
                         start=True, stop=False)
        nc.tensor.matmul(st_ps, sigw_sb[:, 98:130], sigw_sb[:, 98:130],
                         start=False, stop=True)
        st_sb = work.tile([N, N], F32, tag="st_sb")
        nc.vector.tensor_copy(st_sb, st_ps)
        st_flat = work.tile([1, N * N], F32, tag="st_flat")
        nc.sync.dma_start(st_flat, st_sb)
        st_rep = consts.tile([B, N * N], F32)
        for hh in range(2):
            sps2 = pre_ps.tile([B, 512], F32, tag="sp")
            nc.tensor.matmul(sps2, ones1, st_flat[:, hh * 512:(hh + 1) * 512],
                             start=True, stop=True)
            nc.vector.tensor_copy(st_rep[:, hh * 512:(hh + 1) * 512], sps2)

        def emit_sig(z_in, name):
            lg = lin.tile([B, 2, N], F32, tag="sig_lg")
            nc.vector.tensor_tensor(
                lg, z_in.unsqueeze(1).broadcast_to([B, 2, N]),
                sigw_rep[:, 0:64].rearrange("p (c n) -> p c n", c=2), Alu.mult)
            red = lin.tile([B, 2], F32, tag=f"sig_red_{name}")
            nc.vector.tensor_reduce(red, lg, mybir.AxisListType.X, Alu.add)
            nc.vector.tensor_tensor(red, red, sigw_rep[:, 64:66], Alu.add)
            s = lin.tile([B, 2], F32, tag=f"sig_s_{name}")
            nc.scalar.activation(s, red, Act.Exp)
            return s

        # ================= encoder (model-parallel over enc-H) ==============
        with tc.tile_pool(name="p_enc_s", bufs=2) as enc_s, \
             tc.tile_pool(name="p_enc_ps", bufs=1, space="PSUM") as enc_ps:
            xt_r = xt[:].rearrange("(k p) b -> p k b", p=P)
            w1es_r = w1es[:].rearrange("(k p) h -> p k h", p=P)
            a1e_ps = enc_ps.tile([B, HS], F32, tag="a1e")
            for kb in range(12):
                xtile = enc_s.tile([P, 8, B], BF16, tag="xt_t")
                nc.scalar.dma_start(xtile, xt_r[:, kb * 8:(kb + 1) * 8, :])
                wtile = enc_s.tile([P, 8, HS], BF16, tag="w1es_t")
                nc.scalar.dma_start(wtile, w1es_r[:, kb * 8:(kb + 1) * 8, :])
                for j in range(8):
                    nc.tensor.matmul(a1e_ps, xtile[:, j, :], wtile[:, j, :],
                                     start=(kb == 0 and j == 0), stop=False)
            nc.tensor.matmul(a1e_ps, ones1[:, 0:B], b1es_sb,
                             start=False, stop=True)
            h1_sb = work.tile([B, HS], F32, tag="h1")
            nc.vector.tensor_scalar(h1_sb, a1e_ps, 0.0, None, Alu.max)
            h1T_sb = work.tile([P, 2, B], F32, tag="h1T")
            for i in range(2):
                tp = enc_ps.tile([P, B], F32, tag="tp")
                pe_T(nc, tp, h1_sb[:, i * P:(i + 1) * P], identf)
                nc.scalar.copy(h1T_sb[:, i, :], tp)
            zp_ps = enc_ps.tile([B, N], F32, tag="zp")
            for i in range(2):
                nc.tensor.matmul(zp_ps, h1T_sb[:, i, :], w2es_sb[:, i, :],
                                 start=(i == 0), stop=(i == 1))
            zp_sb = work.tile([B, N], F32, tag="zp_sb")
            nc.vector.tensor_copy(zp_sb, zp_ps)
            nc.sync.dma_start(zstd[:], zp_sb)
        nc.gpsimd.collective_compute("AllReduce", Alu.add, replica_groups=RG,
                                     ins=[zstd[:]], outs=[zst_sh[:]])

        pre_cm.__exit__(None, None, None)

        # ========= u = xmb @ W2s^T (partial, fp8 DoubleRow), |xmb|^2 =======
        KT2_D = KT_D // 2       # 48 double-row tiles
        KT2_DS = KT_DS // 2     # 6 own tiles
        cpool = ctx.enter_context(tc.tile_pool(name="cpool", bufs=1))
        c_bf = cpool.tile([P, 2, H], BF16, tag="c_bf")
        ckT8 = cpool.tile([P, KT_H, 2 * P], FP8, tag="ckT")
        w2t_r = w2t8[:].rearrange("(k p) f -> p k f", p=P)
        w2res_cm = tc.tile_pool(name="w2res", bufs=1)
        w2res_pool = w2res_cm.__enter__()
        w2res = w2res_pool.tile([P, KT2_DS, 2, H], FP8, tag="w2res")
        for kt in range(KT2_DS):
            nc.sync.dma_start(
                w2res[:, kt, :, :],
                w2t_r[:, kt, :].rearrange("p (two h) -> p two h", two=2))
        xmbT_sb = work.tile([P, KT2_DS, 2, B], FP8, tag="xmbT")
        nc.sync.dma_start(
            xmbT_sb,
            xmbt8[:].rearrange("(k p) (two b) -> p k two b", p=P, two=2))
        xmb_sb = work.tile([B, DS], F32, tag="xmb")
        nc.sync.dma_start(xmb_sb, xmb[:])
        xmbsq = lin.tile([B, 1], F32, tag="xmbsq")
        scr2 = work.tile([B, H], BF16, tag="scr2")
        nc.scalar.activation(scr2[:, 0:DS], xmb_sb, Act.Square,
                             accum_out=xmbsq)
        u_sb = work.tile([B, H], BF16, tag="u_sb")
        DR = mybir.MatmulPerfMode.DoubleRow
        with tc.tile_pool(name="p_u", bufs=1, space="PSUM") as u_ps_pool:
            u_ps = u_ps_pool.tile([B, H], F32, tag="u_ps")
            for nb in range(4):
                for kt in range(KT2_DS):
                    nc.tensor.matmul(
                        u_ps[:, nb * 512:(nb + 1) * 512],
                        xmbT_sb[:, kt, :, :],
                        w2res[:, kt, :, nb * 512:(nb + 1) * 512],
                        start=(kt == 0), stop=(kt == KT2_DS - 1),
                        perf_mode=DR)
            nc.scalar.activation(u_sb, u_ps, Act.Copy, scale=1.0 / SC8)


        # ================= C' : C_k = W2[hs0,:] @ W2^T  (stream W2^T) =======
        pC_cm = tc.tile_pool(name="pC_s", bufs=3)
        pC = pC_cm.__enter__()
        pCp_cm = tc.tile_pool(name="pC_ps", bufs=1, space="PSUM")
        pCp = pCp_cm.__enter__()
        cps = pCp.tile([P, 8, 512], F32, tag="cps")
        for kt in range(KT2_D):
            if kt < KT2_DS:
                t_in = w2res[:, kt, :, :]
            else:
                t_raw = pC.tile([P, 2, H], FP8, tag="w2_t")
                dq = nc.sync if kt % 2 == 0 else nc.scalar
                dq.dma_start(
                    t_raw,
                    w2t_r[:, kt, :].rearrange("p (two h) -> p two h", two=2))
                t_in = t_raw
            for it in range(2):
                for nb in range(4):
                    nc.tensor.matmul(
                        cps[:, it * 4 + nb, :],
                        t_in[:, :, it * P:(it + 1) * P],
                        t_in[:, :, nb * 512:(nb + 1) * 512],
                        start=(kt == 0), stop=(kt == KT2_D - 1),
                        perf_mode=DR)
        for it in range(2):
            for nb in range(4):
                nc.scalar.activation(c_bf[:, it, nb * 512:(nb + 1) * 512],
                                     cps[:, it * 4 + nb, :], Act.Copy,
                                     scale=1.0 / (SC8 * SC8))
        pCp_cm.__exit__(None, None, None)
        pC_cm.__exit__(None, None, None)
        w2res_cm.__exit__(None, None, None)
        # C_k^T via PE transposes of 128x128 blocks
        with tc.tile_pool(name="p_ct", bufs=2, space="PSUM") as ct_ps_pool:
            for it in range(2):
                for jt in range(KT_H):
                    tp = ct_ps_pool.tile([P, P], BF16, tag="ct")
                    pe_T(nc, tp, c_bf[:, it, jt * P:(jt + 1) * P], identb)
                    nc.scalar.activation(ckT8[:, jt, it * P:(it + 1) * P], tp,
                                         Act.Copy, scale=16.0)

        # ---- z* full (+enc b2), zT, sig1, masks m1 (post-C') ----
        zf_sb = work.tile([B, N], F32, tag="zf")
        nc.sync.dma_start(zf_sb, zst_sh[:])
        zall = lin.tile([B, N], F32, tag="zall")
        zT_sb = work.tile([N, B], F32, tag="zT")
        with tc.tile_pool(name="p_z", bufs=2, space="PSUM") as pz:
            za_ps = pz.tile([B, N], F32, tag="za")
            nc.tensor.matmul(za_ps, ones1[:, 0:B], b2e_sb, start=True, stop=False)
            nc.tensor.matmul(za_ps, identf[0:B, 0:B], zf_sb, start=False,
                             stop=True)
            nc.vector.tensor_copy(zall, za_ps)
            zT_ps = pz.tile([N, B], F32, tag="za")
            pe_T(nc, zT_ps, zall, identf)
            nc.vector.tensor_copy(zT_sb, zT_ps)
        s1 = emit_sig(zall, "s1")
        invsp2 = lin.tile([B, 1], F32, tag="invsp2")
        sp2t = lin.tile([B, 1], F32, tag="sp2t")
        nc.vector.tensor_tensor(sp2t, s1[:, 0:1], s1[:, 0:1], Alu.mult)
        nc.vector.reciprocal(invsp2, sp2t)
        m1T_bf = work.tile([P, KT_H, B], BF16, tag="m1T")
        with tc.tile_pool(name="p_a1t", bufs=1, space="PSUM") as a1t_pool:
            a1T_ps = a1t_pool.tile([P, KT_H, B], F32, tag="a1T")
            for mt in range(KT_H):
                nc.tensor.matmul(a1T_ps[:, mt, :],
                                 w1_sb[:, mt * P:(mt + 1) * P], zT_sb,
                                 start=True, stop=False)
                nc.tensor.matmul(a1T_ps[:, mt, :],
                                 b1d_sb[:, mt * P:(mt + 1) * P], ones1[:, 0:B],
                                 start=False, stop=True)
            nc.vector.tensor_scalar(m1T_bf, a1T_ps, 0.0, None, Alu.is_gt)

        # ================= G partials (shared emitter) ======================
        at_pool = ctx.enter_context(tc.tile_pool(name="at_pool", bufs=2))
        g_pool = ctx.enter_context(tc.tile_pool(name="g_pool", bufs=1))

        def emit_G(mT_bf, tag):
            # A is scaled x64 (fp8), C x16 (fp8): M1 psum = 1024*M1; m1sb8
            # stores 16*M1 (scale 16/1024); G psum = 64*16*G -> g_sb scale
            # 1/1024.
            g_sb = g_pool.tile([N, B * N], F32, tag=f"g_{tag}")
            with tc.tile_pool(name=f"pG{tag}", bufs=2, space="PSUM") as gps_pool:
                for cb in range(4):
                    AT = at_pool.tile([P, KT_H, 16, N], FP8, tag="AT")
                    nc.vector.tensor_tensor(
                        AT,
                        w1T8_sb.unsqueeze(2).broadcast_to([P, KT_H, 16, N]),
                        mT_bf[:, :, cb * 16:(cb + 1) * 16]
                        .unsqueeze(3).broadcast_to([P, KT_H, 16, N]),
                        Alu.mult)
                    m1ps = gps_pool.tile([P, 2, 512], F32, tag="m1ps")
                    for it in range(2):
                        for q in range(KT_H // 2):
                            nc.tensor.matmul(
                                m1ps[:, it, :],
                                ckT8[:, 2 * q:2 * q + 2,
                                     it * P:(it + 1) * P],
                                AT[:, 2 * q:2 * q + 2, :, :],
                                start=(q == 0), stop=(q == KT_H // 2 - 1),
                                perf_mode=DR)
                    m1sb = at_pool.tile([P, 2, 512], FP8, tag="m1sb")
                    nc.scalar.activation(m1sb, m1ps, Act.Copy,
                                         scale=16.0 / 1024.0)
                    gps = gps_pool.tile([N, 512], F32, tag="gps")
                    for s in range(16):
                        nc.tensor.matmul(
                            gps[:, s * N:(s + 1) * N],
                            AT[:, 0:2, s, :],
                            m1sb[:, :, s * N:(s + 1) * N],
                            start=True, stop=True, perf_mode=DR)
                    nc.vector.tensor_scalar(
                        g_sb[:, cb * 512:(cb + 1) * 512], gps,
                        1.0 / 1024.0, None, Alu.mult)
            return g_sb

        # ---- G1 -> pack -> AllReduce ----
        g1_sb = emit_G(m1T_bf, "1")
        nc.sync.dma_start(
            dram_ap(g1d, 0, [[N, N], [N * N, B], [1, N]]), g1_sb)
        nc.gpsimd.collective_compute("AllReduce", Alu.add, replica_groups=RG,
                                     ins=[g1d[:]], outs=[g1_sh[:]])

        # ---- Prec assembly + LDLT + ltinv + dz + z_s ----
        Tm = lin.tile([B, N * N], F32, tag="Tm")
        nc.sync.dma_start(Tm, g1_sh[:])
        nc.vector.tensor_scalar(Tm, Tm, invsp2, None, Alu.mult)
        nc.vector.tensor_tensor(Tm, Tm, st_rep, Alu.add)
        diag1 = sub_ap(Tm, 0, [[N + 1, N]])
        nc.vector.tensor_scalar(diag1, diag1, 1.0, None, Alu.add)

        invD = lin.tile([B, N], F32, tag="invD")
        SCR = lin.tile([B, N * N], F32, tag="SCR")
        emit_ldlt(nc, Tm, SCR, invD, B)
        LT = lin.tile([B, N * N], F32, tag="LT")
        nc.vector.tensor_tensor(
            LT.rearrange("p (a b) -> p a b", b=N),
            Tm.rearrange("p (a b) -> p a b", b=N),
            invD.unsqueeze(1).broadcast_to([B, N, N]), Alu.mult)
        # dz: solve Lt^T dz = epss by backward substitution, one STT per
        # column: stored[0:k] <- (Ltrow_k * stored[k]) - stored[0:k], which
        # leaves stored[j] = (-1)^(N-1-j) dz[j]; fixed up with sgn_sb.
        srD = lin.tile([B, N], F32, tag="srD")
        nc.scalar.activation(srD, invD, Act.Sqrt)        # 1/sqrt(D)
        dz = lin.tile([B, N], F32, tag="dz")
        nc.vector.tensor_tensor(dz, eps_sb, srD, Alu.mult)
        for k in range(N - 1, 0, -1):
            lrow = sub_ap(LT, k * N, [[1, k]])
            nc.vector.scalar_tensor_tensor(
                dz[:, 0:k], lrow, dz[:, k:k + 1], dz[:, 0:k],
                Alu.mult, Alu.subtract)
        zs = lin.tile([B, N], F32, tag="zs")
        nc.vector.tensor_tensor(dz, dz, sgn_sb, Alu.mult)
        nc.vector.tensor_tensor(zs, zall, dz, Alu.add)
        s2 = emit_sig(zs, "s2")

        # ================= stage 2 ==========================================
        zsT_sb = work.tile([N, B], F32, tag="zsT")
        with tc.tile_pool(name="p_zst", bufs=1, space="PSUM") as zst_pool:
            zsT_ps = zst_pool.tile([N, B], F32, tag="zsT_ps")
            pe_T(nc, zsT_ps, zs, identf)
            nc.vector.tensor_copy(zsT_sb, zsT_ps)

        m2T_bf = work.tile([P, KT_H, B], BF16, tag="m2T")
        h2neg = work.tile([P, 2, B], BF16, tag="h2neg")
        with tc.tile_pool(name="p_a2t", bufs=1, space="PSUM") as a2t_pool:
            a2T_ps = a2t_pool.tile([P, KT_H, B], F32, tag="a2T")
            for mt in range(KT_H):
                nc.tensor.matmul(a2T_ps[:, mt, :],
                                 w1_sb[:, mt * P:(mt + 1) * P], zsT_sb,
                                 start=True, stop=False)
                nc.tensor.matmul(a2T_ps[:, mt, :],
                                 b1d_sb[:, mt * P:(mt + 1) * P], ones1[:, 0:B],
                                 start=False, stop=True)
            nc.vector.tensor_scalar(m2T_bf, a2T_ps, 0.0, None, Alu.is_gt)
            # -relu(a2) for local block (first 2 kt): min(-a2, 0)
            nc.vector.scalar_tensor_tensor(
                h2neg, a2T_ps[:, 0:2, :], -1.0, zeros2, Alu.mult, Alu.min)

        h2_sb = work.tile([B, H], BF16, tag="h2")
        with tc.tile_pool(name="p_a2", bufs=1, space="PSUM") as a2_pool:
            a2_ps = a2_pool.tile([B, H], F32, tag="a2")
            for nb in range(4):
                nc.tensor.matmul(a2_ps[:, nb * 512:(nb + 1) * 512],
                                 zsT_sb, w1_sb[:, nb * 512:(nb + 1) * 512],
                                 start=True, stop=False)
                nc.tensor.matmul(a2_ps[:, nb * 512:(nb + 1) * 512],
                                 ones1[:, 0:B], b1d_sb[:, nb * 512:(nb + 1) * 512],
                                 start=False, stop=True)
            nc.vector.tensor_scalar(h2_sb, a2_ps, 0.0, None, Alu.max)

        uh2 = lin.tile([B, 1], F32, tag="uh2")
        nc.vector.tensor_tensor(scr2, u_sb, h2_sb, Alu.mult)
        nc.vector.tensor_reduce(uh2, scr2, mybir.AxisListType.X, Alu.add)

        wd_sb = work.tile([B, H], BF16, tag="wd")
        vneg_bf = work.tile([B, H], BF16, tag="vneg")
        vh2m = lin.tile([B, 1], F32, tag="vh2m")
        with tc.tile_pool(name="p_v", bufs=1, space="PSUM") as v_pool:
            v_ps = v_pool.tile([B, H], F32, tag="v_ps")   # holds -v
            for nb in range(4):
                for it in range(2):
                    nc.tensor.matmul(v_ps[:, nb * 512:(nb + 1) * 512],
                                     h2neg[:, it, :],
                                     c_bf[:, it, nb * 512:(nb + 1) * 512],
                                     start=(it == 0), stop=(it == 1))
            nc.scalar.copy(vneg_bf, v_ps)
        nc.vector.tensor_tensor(scr2, vneg_bf, h2_sb, Alu.mult)
        nc.vector.tensor_reduce(vh2m, scr2, mybir.AxisListType.X, Alu.add)
        nc.vector.tensor_tensor(wd_sb, u_sb, vneg_bf, Alu.add)
        svec = lin.tile([B, 1], F32, tag="svec")
        nc.vector.scalar_tensor_tensor(svec, uh2, -2.0, xmbsq, Alu.mult, Alu.add)
        nc.vector.tensor_tensor(svec, svec, vh2m, Alu.subtract)

        # t_k = A2 wd_k : transpose wd, mask, matmul
        wdT_sb = work.tile([P, KT_H, B], BF16, tag="wdT")
        with tc.tile_pool(name="p_wdt", bufs=2, space="PSUM") as wdt_pool:
            for jt in range(KT_H):
                tp = wdt_pool.tile([P, B], BF16, tag="wdt")
                pe_T(nc, tp, wd_sb[:, jt * P:(jt + 1) * P], identb)
                nc.scalar.copy(wdT_sb[:, jt, :], tp)
        mwdT = work.tile([P, KT_H, B], BF16, tag="mwdT")
        nc.vector.tensor_tensor(mwdT, wdT_sb, m2T_bf, Alu.mult)
        tk_sb = work.tile([N, B], F32, tag="tk")
        with tc.tile_pool(name="p_tk", bufs=1, space="PSUM") as tk_pool:
            tk_ps = tk_pool.tile([N, B], F32, tag="tk_ps")
            for jt in range(KT_H):
                nc.tensor.matmul(tk_ps, w1Tb_sb[:, jt, :], mwdT[:, jt, :],
                                 start=(jt == 0), stop=(jt == KT_H - 1))
            nc.vector.tensor_copy(tk_sb, tk_ps)

        # ---- G2 + pack [t | svec | G2] -> AllReduce ----
        g2_sb = emit_G(m2T_bf, "2")
        nc.sync.dma_start(dram_ap(pkd, 0, [[1, N], [PKW, B]]), tk_sb)
        nc.sync.dma_start(dram_ap(pkd, N, [[PKW, B]]), svec)
        nc.sync.dma_start(
            dram_ap(pkd, N + 1, [[N, N], [PKW, B], [1, N]]), g2_sb)
        nc.gpsimd.collective_compute("AllReduce", Alu.add, replica_groups=RG,
                                     ins=[pkd[:]], outs=[pk_sh[:]])

        # ---- background (overlaps AllReduce): ltinv, tr, logdet, latent ----
        X1 = lin.tile([B, N * N], F32, tag="X1")
        nc.vector.memset(X1, 0.0)
        nc.vector.memset(sub_ap(X1, 0, [[N + 1, N]]), 1.0)
        emit_ltinv(nc, LT, X1, SCR, B)
        scrB = lin.tile([B, N * N], F32, tag="scrB")
        nc.vector.tensor_tensor(SCR, X1, X1, Alu.mult)
        trv = lin.tile([B, 1], F32, tag="trv")
        nc.vector.tensor_tensor(
            scrB.rearrange("p (a b) -> p a b", b=N),
            SCR.rearrange("p (a b) -> p a b", b=N),
            invD.unsqueeze(2).broadcast_to([B, N, N]), Alu.mult)
        nc.vector.tensor_reduce(trv, scrB, mybir.AxisListType.X, Alu.add)
        logs = lin.tile([B, N], F32, tag="logs")
        ldv = lin.tile([B, 1], F32, tag="ldv")
        nc.scalar.activation(logs, invD, Act.Ln)
        nc.vector.tensor_reduce(ldv, logs, mybir.AxisListType.X, Alu.add)
        zsq = lin.tile([B, N], F32, tag="zsq")
        latv = lin.tile([B, 1], F32, tag="latv")
        nc.vector.tensor_tensor(zsq, zall, zall, Alu.mult)
        nc.vector.tensor_reduce(latv, zsq, mybir.AxisListType.X, Alu.add)
        nc.vector.tensor_tensor(latv, latv, trv, Alu.add)
        nc.vector.tensor_scalar(latv, latv, 0.5, None, Alu.mult)
        nc.vector.tensor_scalar(ldv, ldv, -0.5, None, Alu.mult)

        # ---- post-AllReduce: solve G2 y = t, d_proj, recon, out ----
        y = lin.tile([B, N], F32, tag="y")
        nc.sync.dma_start(y, pk_sh[:, 0:N])
        svf = lin.tile([B, 1], F32, tag="svf")
        nc.sync.dma_start(svf, pk_sh[:, N:N + 1])
        Tm2 = lin.tile([B, N * N], F32, tag="Tm2")
        nc.sync.dma_start(Tm2, pk_sh[:, N + 1:PKW])
        invD2 = lin.tile([B, N], F32, tag="invD2")
        emit_ldlt(nc, Tm2, SCR, invD2, B)
        LT2 = lin.tile([B, N * N], F32, tag="LT2")
        nc.vector.tensor_tensor(
            LT2.rearrange("p (a b) -> p a b", b=N),
            Tm2.rearrange("p (a b) -> p a b", b=N),
            invD2.unsqueeze(1).broadcast_to([B, N, N]), Alu.mult)
        emit_fwd_solve_alt(nc, LT2, y, B)
        ysq = lin.tile([B, N], F32, tag="ysq")
        yw = lin.tile([B, N], F32, tag="yw")
        dproj = lin.tile([B, 1], F32, tag="dproj")
        nc.vector.tensor_tensor(ysq, y, y, Alu.mult)
        nc.vector.tensor_tensor(yw, ysq, invD2, Alu.mult)
        nc.vector.tensor_reduce(dproj, yw, mybir.AxisListType.X, Alu.add)

        sq2 = lin.tile([B, 2], F32, tag="sq2")
        nc.vector.tensor_tensor(sq2, s2, s2, Alu.mult)
        nc.vector.tensor_scalar(sq2, sq2, 2.0, None, Alu.mult)
        inv2 = lin.tile([B, 2], F32, tag="inv2")
        nc.vector.reciprocal(inv2, sq2)     # [1/(2sp2^2), 1/(2sv2^2)]
        logs2 = lin.tile([B, 2], F32, tag="logs2")
        logw = lin.tile([B, 2], F32, tag="logw")
        nc.vector.memset(logw[:, 0:1], float(N))
        nc.vector.memset(logw[:, 1:2], float(D - N))
        nc.scalar.activation(logs2, s2, Act.Ln)
        logterm = lin.tile([B, 1], F32, tag="logterm")
        junk2 = lin.tile([B, 2], F32, tag="junk2")
        nc.vector.tensor_tensor(junk2, logs2, logw, Alu.mult)
        nc.vector.tensor_reduce(logterm, junk2, mybir.AxisListType.X, Alu.add)
        isub = lin.tile([B, 1], F32, tag="isub")
        nc.vector.tensor_tensor(isub, inv2[:, 0:1], inv2[:, 1:2], Alu.subtract)
        recon = lin.tile([B, 1], F32, tag="recon")
        nc.vector.tensor_tensor(recon, dproj, isub, Alu.mult)
        p2t = lin.tile([B, 1], F32, tag="p2t")
        nc.vector.tensor_tensor(p2t, svf, inv2[:, 1:2], Alu.mult)
        nc.vector.tensor_tensor(recon, recon, p2t, Alu.add)
        nc.vector.tensor_tensor(recon, recon, logterm, Alu.add)
        ov = lin.tile([B, 1], F32, tag="ov")
        nc.vector.tensor_tensor(ov, recon, latv, Alu.add)
        nc.vector.tensor_tensor(ov, ov, ldv, Alu.add)
        nc.vector.tensor_scalar(ov, ov, 1.0 / D, None, Alu.mult)
        nc.sync.dma_start(out[:], ov)

    legalize_waits(nc)
    return nc


def shard_inputs(inputs):
    """Host-side prep: per-core H-permutation + D-tile reordering."""
    bf = ml_dtypes.bfloat16
    x = np.ascontiguousarray(np.asarray(inputs["x"], np.float32))
    eps = np.ascontiguousarray(np.asarray(inputs["eps"], np.float32))
    eW1 = np.asarray(inputs["enc_W1"], np.float32)
    eb1 = np.asarray(inputs["enc_b1"], np.float32)
    eW2 = np.asarray(inputs["enc_W2"], np.float32)
    eb2 = np.asarray(inputs["enc_b2"], np.float32)
    dW1 = np.asarray(inputs["dec_W1"], np.float32)
    db1 = np.asarray(inputs["dec_b1"], np.float32)
    dW2 = np.asarray(inputs["dec_W2"], np.float32)
    db2 = np.asarray(inputs["dec_b2"], np.float32)
    sW = np.asarray(inputs["sig_W"], np.float32)
    sb = np.asarray(inputs["sig_b"], np.float32)

    xT_bf = np.ascontiguousarray(x.T).astype(bf)
    xmb_full = x - db2[None, :]
    W2T = np.ascontiguousarray(dW2.T)      # [D, H]
    sigv = np.zeros((1, 130), np.float32)
    sigv[0, 0:32] = sW[:, 0]
    sigv[0, 32:64] = sW[:, 1]
    sigv[0, 64:66] = sb
    sigv[0, 66:98] = sW[:, 0] * np.sqrt(N / 2.0)
    sigv[0, 98:130] = sW[:, 1] * np.sqrt((D - N) / 2.0)

    maps = []
    for k in range(NCORES):
        hperm = np.concatenate([np.arange(k * HS, (k + 1) * HS),
                                np.arange(0, k * HS),
                                np.arange((k + 1) * HS, H)])
        # D-tile order: own 12 tiles first
        own = np.arange(k * KT_DS, (k + 1) * KT_DS)
        rest = np.concatenate([np.arange(0, k * KT_DS),
                               np.arange((k + 1) * KT_DS, KT_D)])
        tord = np.concatenate([own, rest])
        w2t_p = W2T[:, hperm].reshape(KT_D, P, H)[tord].reshape(D, H)
        w2q = np.asarray(w2t_p * 256.0, np.float32).astype(ml_dtypes.float8_e4m3)
        w2q = np.ascontiguousarray(
            w2q.reshape(KT_D // 2, 2, P, H).transpose(0, 2, 1, 3)
            .reshape(D // 2, 2 * H))
        dsl = slice(k * DS, (k + 1) * DS)
        xq = np.asarray(xmb_full[:, dsl].T, np.float32).astype(
            ml_dtypes.float8_e4m3)
        xq = np.ascontiguousarray(
            xq.reshape(KT_DS // 2, 2, P, B).transpose(0, 2, 1, 3)
            .reshape(DS // 2, 2 * B))
        maps.append({
            "w2t8": w2q,
            "xt": xT_bf,
            "w1es": np.ascontiguousarray(
                eW1[:, k * HS:(k + 1) * HS]).astype(bf),
            "b1es": np.ascontiguousarray(eb1[None, k * HS:(k + 1) * HS]),
            "w2es": np.ascontiguousarray(eW2[k * HS:(k + 1) * HS, :]),
            "b2e": np.ascontiguousarray(eb2[None, :]),
            "w1p": np.ascontiguousarray(dW1[:, hperm]),
            "w1tp_bf": np.ascontiguousarray(dW1[:, hperm].T).astype(bf),
            "w1tp8": np.ascontiguousarray(dW1[:, hperm].T * 64.0).astype(
                ml_dtypes.float8_e4m3),
            "b1dp": np.ascontiguousarray(db1[None, hperm]),
            "xmbt8": xq,
            "xmb": np.ascontiguousarray(xmb_full[:, dsl]),
            "sigw": sigv,
            "epsin": eps,
        })
    return maps


_NC_CACHE = None


def kernel(**inputs) -> np.ndarray:
    global _NC_CACHE
    from concourse.bass_utils import run_bass_kernel_spmd
    if _NC_CACHE is None:
        _NC_CACHE = build_nc()
    nc = _NC_CACHE
    maps = shard_inputs(inputs)
    res = run_bass_kernel_spmd(nc, maps, list(range(NCORES)))
    return np.asarray(res.results[0]["out"]).reshape(B).astype(np.float32)


# revision 17
# speedup vs baseline: 1.3813x; 1.0052x over previous
"""Trainium2 Bass kernel for nn_EnergyAE (B=64, D=12288, N=32, H=2048) on 8 cores.

v2 restructure (vs v1's contraction-sharded C + 16.8MB AllReduce):
  - C block-row sharding: core k computes C_k = C[hs_k, :] (256 x 2048) locally
    by streaming full W2^T (bf16, 12.6MB); C_k stays in SBUF. No C collective,
    no C DRAM round trip.
  - Per-core H-permutation (host-side) puts each core's block at H-cols 0:256,
    so the SPMD program is core-index-free.
  - G[b] computed as partial sums over ALL 64 samples using C_k; AllReduce of
    packed G (256KB). Every core then factors all 64 (same vector cost) ->
    no z_s AllGather, no sel8 gathers.
  - x_star never materialized: W2 delta = W2(x-b2) - C h2, so
    u = xmb @ W2s^T (partial), v = h2[:, 0:256] @ C_k (partial),
    wd_k = u_k - v_k, t_k = A2 wd_k (pre-AllReduce!),
    d_sq = |xmb|^2 - 2 u.h2 + v.h2 (scalar partials).
  - Final AllReduce packs [t | svec | G2] = 271KB. All cores compute all 64
    outputs; host takes core 0's.

Identities (validated numerically, rel err ~1.7e-3 with bf16 C/A/u):
  Prec = Lt D Lt^T (unit-lower LDLT)
  sum(log eig)/2 = 0.5*sum(log D);  sum(1/eig) = ||D^-1/2 Lt^-1||_F^2
  U^-1 eps = Lt^-T (eps/sqrt(D));   t^T G2^-1 t = ||D2^-1/2 Lt2^-1 t||^2
  sig_term = (n w0 w0^T + (D-n) w1 w1^T)/2   (constant across batch)
"""
import sys

for _p in ("/opt/trn_rl_repo", "/root/.axon_site/_ro/trn_rl_repo"):
    if _p not in sys.path:
        sys.path.append(_p)

import numpy as np
import ml_dtypes
from contextlib import ExitStack

import concourse.bass as bass
import concourse.mybir as mybir
import concourse.tile as tile
from concourse.masks import make_identity

B, D, N, H = 64, 12288, 32, 2048
NCORES = 8
BL = B // NCORES          # 8 local samples (only used for host sharding)
HS = H // NCORES          # 256: C block rows per core
DS = D // NCORES          # 1536
KT_D = D // 128           # 96
KT_DS = DS // 128         # 12
KT_H = H // 128           # 16
P = 128

F32 = mybir.dt.float32
F32R = mybir.dt.float32r
BF16 = mybir.dt.bfloat16
FP8 = mybir.dt.float8e4
SC8 = 256.0
Alu = mybir.AluOpType
Act = mybir.ActivationFunctionType
RG = [list(range(NCORES))]


def sub_ap(t, extra_off, dims):
    """Custom free-dim AP on a [P, F] tile; dims = [[step,count],...] in elems."""
    base = t[:, 0:1]
    return bass.AP(base.tensor, base.offset + extra_off, [base.ap[0]] + dims)


def dram_ap(t, off, dims):
    """Custom AP into a DRAM tensor; dims = [[step,count],...] in elems."""
    base = t[:]
    return bass.AP(base.tensor, off, dims)


def pe_T(nc, out_ps, in_ap, ident):
    """PE transpose: out_ps [f, p] = in_ap [p, f].T"""
    kp = in_ap.shape[0]
    nc.tensor.transpose(out_ps, in_ap, ident[0:kp, 0:kp])


def emit_ldlt(nc, T, OUT, invD, rows, n=32):
    """In-place unit-lower LDLT of T [rows, n*n] (row-major per sample).
    After: strict lower of T holds unscaled columns u; diag holds D; invD=1/D."""
    for j in range(n):
        nc.vector.reciprocal(invD[:, j:j + 1], T[:, (n + 1) * j:(n + 1) * j + 1])
        m = n - 1 - j
        if m == 0:
            break
        base = (j + 1) * n + j
        u_i = sub_ap(T, base, [[n, m], [0, m]])
        u_k = sub_ap(T, base, [[0, m], [n, m]])
        outer = sub_ap(OUT, 0, [[m, m], [1, m]])
        nc.vector.scalar_tensor_tensor(
            outer, u_i, invD[:, j:j + 1], u_k, Alu.mult, Alu.mult)
        trail = sub_ap(T, (j + 1) * (n + 1), [[n, m], [1, m]])
        nc.vector.tensor_tensor(trail, trail, outer, Alu.subtract)


def emit_ltinv(nc, LT, X, OUT, rows, n=32):
    """X = LT^{-1} for unit-lower LT [rows, n*n]; X preset to I by caller."""
    for k in range(n - 1):
        rr = n - 1 - k
        cols = k + 1
        lcol = sub_ap(LT, (k + 1) * n + k, [[n, rr], [0, cols]])
        xrow = sub_ap(X, k * n, [[0, rr], [1, cols]])
        prod = sub_ap(OUT, 0, [[cols, rr], [1, cols]])
        nc.vector.scalar_tensor_tensor(prod, lcol, -1.0, xrow, Alu.mult, Alu.mult)
        xblk = sub_ap(X, (k + 1) * n, [[n, rr], [1, cols]])
        nc.vector.tensor_tensor(xblk, xblk, prod, Alu.add)


def emit_fwd_solve_alt(nc, LT, w, rows, n=32):
    """Forward-substitute LT y = w in place, ONE STT per column:
      stored[k+1:] <- (LT[k+1:,k] * stored[k]) - stored[k+1:]
    This leaves stored[j] = (-1)^j * y[j] (every update flips the sign of the
    remaining entries, and the scalar operand carries the matching sign), so
    it is valid whenever the caller only consumes y elementwise-squared."""
    for k in range(n - 1):
        rr = n - 1 - k
        lcol = sub_ap(LT, (k + 1) * n + k, [[n, rr]])
        nc.vector.scalar_tensor_tensor(
            w[:, k + 1:n], lcol, w[:, k:k + 1], w[:, k + 1:n],
            Alu.mult, Alu.subtract)


def emit_fwd_solve(nc, LT, y, OUT, rows, n=32):
    """y <- LT^{-1} y for unit-lower LT [rows, n*n], y [rows, n] in place."""
    for k in range(n - 1):
        rr = n - 1 - k
        lcol = sub_ap(LT, (k + 1) * n + k, [[n, rr]])
        nc.vector.scalar_tensor_tensor(
            OUT[:, 0:rr], lcol, -1.0, y[:, k:k + 1].broadcast_to([rows, rr]),
            Alu.mult, Alu.mult)
        nc.vector.tensor_tensor(y[:, k + 1:n], y[:, k + 1:n], OUT[:, 0:rr], Alu.add)


def legalize_waits(nc, maxw=1):
    """Split multi-wait sync_info into standalone EventSemaphore instructions."""
    for f in nc.m.functions:
        for bb in f.blocks:
            insts = list(bb.instructions)
            out = []
            changed = False
            for inst in insts:
                si = inst.sync_info
                if si is not None and si.on_wait and len(si.on_wait) > maxw:
                    waits = list(si.on_wait)
                    imm = [w for w in waits if w.uses_immediate]
                    reg = [w for w in waits if not w.uses_immediate]
                    keep = (reg + imm)[:maxw] if len(reg) <= maxw else reg
                    extra = [w for w in waits if w not in keep]
                    if len(keep) > maxw:
                        raise RuntimeError(f"{inst.name}: {len(keep)} register waits")
                    for w in extra:
                        ev = mybir.InstEventSemaphore(
                            name=nc.get_next_instruction_name(), ins=[], outs=[])
                        ev.engine = inst.engine
                        ev.sync_info = mybir.SyncInfo(on_wait=[w], on_update=[])
                        out.append(ev)
                    inst.sync_info = mybir.SyncInfo(
                        on_wait=keep, on_update=list(si.on_update or []))
                    changed = True
                out.append(inst)
            if changed:
                bb.instructions = out
    return nc


def build_nc():
    nc = bass.Bass()

    # ---- I/O (per-core views prepared by host; H-permuted, D-tile-reordered)
    w2t8 = nc.dram_tensor("w2t8", [D // 2, 2 * H], FP8, kind="ExternalInput")
    xt = nc.dram_tensor("xt", [D, B], BF16, kind="ExternalInput")
    w1es = nc.dram_tensor("w1es", [D, HS], BF16, kind="ExternalInput")
    b1es = nc.dram_tensor("b1es", [1, HS], F32, kind="ExternalInput")
    w2es = nc.dram_tensor("w2es", [HS, N], F32, kind="ExternalInput")
    b2e = nc.dram_tensor("b2e", [1, N], F32, kind="ExternalInput")
    w1p = nc.dram_tensor("w1p", [N, H], F32, kind="ExternalInput")
    w1tp_bf = nc.dram_tensor("w1tp_bf", [H, N], BF16, kind="ExternalInput")
    w1tp8 = nc.dram_tensor("w1tp8", [H, N], FP8, kind="ExternalInput")
    b1dp = nc.dram_tensor("b1dp", [1, H], F32, kind="ExternalInput")
    xmbt8 = nc.dram_tensor("xmbt8", [DS // 2, 2 * B], FP8, kind="ExternalInput")
    xmb = nc.dram_tensor("xmb", [B, DS], F32, kind="ExternalInput")
    sigw = nc.dram_tensor("sigw", [1, 130], F32, kind="ExternalInput")
    epsin = nc.dram_tensor("epsin", [B, N], F32, kind="ExternalInput")
    out = nc.dram_tensor("out", [B, 1], F32, kind="ExternalOutput")

    # ---- internal DRAM ----
    zstd = nc.dram_tensor("zstd", [B, N], F32)
    zst_sh = nc.dram_tensor("zst_sh", [B, N], F32, addr_space="Shared")
    g1d = nc.dram_tensor("g1d", [B, N * N], F32)
    g1_sh = nc.dram_tensor("g1_sh", [B, N * N], F32, addr_space="Shared")
    PKW = N + 1 + N * N   # 1057
    pkd = nc.dram_tensor("pkd", [B, PKW], F32)
    pk_sh = nc.dram_tensor("pk_sh", [B, PKW], F32, addr_space="Shared")

    with tile.TileContext(nc) as tc, ExitStack() as ctx:
        consts = ctx.enter_context(tc.tile_pool(name="consts", bufs=1))
        work = ctx.enter_context(tc.tile_pool(name="work", bufs=1))
        lin = ctx.enter_context(tc.tile_pool(name="lin", bufs=1))
        pre_cm = tc.tile_pool(name="pre_ps", bufs=2, space="PSUM")
        pre_ps = pre_cm.__enter__()

        # ---- constants ----
        identf = consts.tile([P, P], F32)
        make_identity(nc, identf)
        identb = consts.tile([P, P], BF16)
        make_identity(nc, identb)
        wu_d = nc.dram_tensor("wu_d", [1, 16], F32)
        wu_sh = nc.dram_tensor("wu_sh", [1, 16], F32, addr_space="Shared")
        nc.gpsimd.collective_compute("AllReduce", Alu.add, replica_groups=RG,
                                     ins=[wu_d[:]], outs=[wu_sh[:]])
        ones1 = consts.tile([1, B], F32)
        nc.vector.memset(ones1, 1.0)
        zeros2 = consts.tile([P, 2, B], F32)
        nc.vector.memset(zeros2, 0.0)
        sigw_sb = consts.tile([1, 130], F32)
        nc.sync.dma_start(sigw_sb, sigw[:])
        eps_sb = consts.tile([B, N], F32)
        nc.sync.dma_start(eps_sb, epsin[:])
        b1es_sb = consts.tile([1, HS], F32)
        nc.sync.dma_start(b1es_sb, b1es[:])
        b2e_sb = consts.tile([1, N], F32)
        nc.sync.dma_start(b2e_sb, b2e[:])
        b1d_sb = consts.tile([1, H], F32)
        nc.sync.dma_start(b1d_sb, b1dp[:])
        w2es_sb = consts.tile([P, 2, N], F32)
        nc.sync.dma_start(w2es_sb, w2es[:].rearrange("(k p) n -> p k n", p=P))
        w1_sb = consts.tile([N, H], F32)
        nc.sync.dma_start(w1_sb, w1p[:])
        w1Tb_sb = consts.tile([P, KT_H, N], BF16)
        nc.sync.dma_start(w1Tb_sb, w1tp_bf[:].rearrange("(k p) n -> p k n", p=P))
        w1T8_sb = consts.tile([P, KT_H, N], FP8)
        nc.sync.dma_start(w1T8_sb, w1tp8[:].rearrange("(k p) n -> p k n", p=P))
        sgn_sb = consts.tile([B, N], F32)
        nc.vector.memset(sgn_sb, -1.0)
        nc.vector.memset(sub_ap(sgn_sb, 1, [[2, N // 2]]), 1.0)

        sigw_rep = consts.tile([B, 130], F32)
        sigw_ps = pre_ps.tile([B, 130], F32, tag="sp")
        nc.tensor.matmul(sigw_ps, ones1, sigw_sb, start=True, stop=True)
        nc.vector.tensor_copy(sigw_rep, sigw_ps)

        # sig_term replicated [B, N*N]
        st_ps = pre_ps.tile([N, N], F32, tag="sp")
        nc.tensor.matmul(st_ps, sigw_sb[:, 66:98], sigw_sb[:, 66:98],
                         start=True, stop=False)
        nc.tensor.matmul(st_ps, sigw_sb[:, 98:130], sigw_sb[:, 98:130],
                         start=False, stop=True)
        st_sb = work.tile([N, N], F32, tag="st_sb")
        nc.vector.tensor_copy(st_sb, st_ps)
        st_flat = work.tile([1, N * N], F32, tag="st_flat")
        nc.sync.dma_start(st_flat, st_sb)
        st_rep = consts.tile([B, N * N], F32)
        for hh in range(2):
            sps2 = pre_ps.tile([B, 512], F32, tag="sp")
            nc.tensor.matmul(sps2, ones1, st_flat[:, hh * 512:(hh + 1) * 512],
                             start=True, stop=True)
            nc.vector.tensor_copy(st_rep[:, hh * 512:(hh + 1) * 512], sps2)

        def emit_sig(z_in, name):
            lg = lin.tile([B, 2, N], F32, tag="sig_lg")
            nc.vector.tensor_tensor(
                lg, z_in.unsqueeze(1).broadcast_to([B, 2, N]),
                sigw_rep[:, 0:64].rearrange("p (c n) -> p c n", c=2), Alu.mult)
            red = lin.tile([B, 2], F32, tag=f"sig_red_{name}")
            nc.vector.tensor_reduce(red, lg, mybir.AxisListType.X, Alu.add)
            nc.vector.tensor_tensor(red, red, sigw_rep[:, 64:66], Alu.add)
            s = lin.tile([B, 2], F32, tag=f"sig_s_{name}")
            nc.scalar.activation(s, red, Act.Exp)
            return s

        # ================= encoder (model-parallel over enc-H) ==============
        with tc.tile_pool(name="p_enc_s", bufs=2) as enc_s, \
             tc.tile_pool(name="p_enc_ps", bufs=1, space="PSUM") as enc_ps:
            xt_r = xt[:].rearrange("(k p) b -> p k b", p=P)
            w1es_r = w1es[:].rearrange("(k p) h -> p k h", p=P)
            a1e_ps = enc_ps.tile([B, HS], F32, tag="a1e")
            for kb in range(12):
                xtile = enc_s.tile([P, 8, B], BF16, tag="xt_t")
                nc.scalar.dma_start(xtile, xt_r[:, kb * 8:(kb + 1) * 8, :])
                wtile = enc_s.tile([P, 8, HS], BF16, tag="w1es_t")
                nc.scalar.dma_start(wtile, w1es_r[:, kb * 8:(kb + 1) * 8, :])
                for j in range(8):
                    nc.tensor.matmul(a1e_ps, xtile[:, j, :], wtile[:, j, :],
                                     start=(kb == 0 and j == 0), stop=False)
            nc.tensor.matmul(a1e_ps, ones1[:, 0:B], b1es_sb,
                             start=False, stop=True)
            h1_sb = work.tile([B, HS], F32, tag="h1")
            nc.vector.tensor_scalar(h1_sb, a1e_ps, 0.0, None, Alu.max)
            h1T_sb = work.tile([P, 2, B], F32, tag="h1T")
            for i in range(2):
                tp = enc_ps.tile([P, B], F32, tag="tp")
                pe_T(nc, tp, h1_sb[:, i * P:(i + 1) * P], identf)
                nc.scalar.copy(h1T_sb[:, i, :], tp)
            zp_ps = enc_ps.tile([B, N], F32, tag="zp")
            for i in range(2):
                nc.tensor.matmul(zp_ps, h1T_sb[:, i, :], w2es_sb[:, i, :],
                                 start=(i == 0), stop=(i == 1))
            zp_sb = work.tile([B, N], F32, tag="zp_sb")
            nc.vector.tensor_copy(zp_sb, zp_ps)
            nc.sync.dma_start(zstd[:], zp_sb)
        nc.gpsimd.collective_compute("AllReduce", Alu.add, replica_groups=RG,
                                     ins=[zstd[:]], outs=[zst_sh[:]])

        pre_cm.__exit__(None, None, None)

        # ========= u = xmb @ W2s^T (partial, fp8 DoubleRow), |xmb|^2 =======
        KT2_D = KT_D // 2       # 48 double-row tiles
        KT2_DS = KT_DS // 2     # 6 own tiles
        cpool = ctx.enter_context(tc.tile_pool(name="cpool", bufs=1))
        c_bf = cpool.tile([P, 2, H], BF16, tag="c_bf")
        ckT8 = cpool.tile([P, KT_H, 2 * P], FP8, tag="ckT")
        w2t_r = w2t8[:].rearrange("(k p) f -> p k f", p=P)
        w2res_cm = tc.tile_pool(name="w2res", bufs=1)
        w2res_pool = w2res_cm.__enter__()
        w2res = w2res_pool.tile([P, KT2_DS, 2, H], FP8, tag="w2res")
        for kt in range(KT2_DS):
            nc.sync.dma_start(
                w2res[:, kt, :, :],
                w2t_r[:, kt, :].rearrange("p (two h) -> p two h", two=2))
        xmbT_sb = work.tile([P, KT2_DS, 2, B], FP8, tag="xmbT")
        nc.sync.dma_start(
            xmbT_sb,
            xmbt8[:].rearrange("(k p) (two b) -> p k two b", p=P, two=2))
        xmb_sb = work.tile([B, DS], F32, tag="xmb")
        nc.sync.dma_start(xmb_sb, xmb[:])
        xmbsq = lin.tile([B, 1], F32, tag="xmbsq")
        scr2 = work.tile([B, H], BF16, tag="scr2")
        nc.scalar.activation(scr2[:, 0:DS], xmb_sb, Act.Square,
                             accum_out=xmbsq)
        u_sb = work.tile([B, H], BF16, tag="u_sb")
        DR = mybir.MatmulPerfMode.DoubleRow
        with tc.tile_pool(name="p_u", bufs=1, space="PSUM") as u_ps_pool:
            u_ps = u_ps_pool.tile([B, H], F32, tag="u_ps")
            for nb in range(4):
                for kt in range(KT2_DS):
                    nc.tensor.matmul(
                        u_ps[:, nb * 512:(nb + 1) * 512],
                        xmbT_sb[:, kt, :, :],
                        w2res[:, kt, :, nb * 512:(nb + 1) * 512],
                        start=(kt == 0), stop=(kt == KT2_DS - 1),
                        perf_mode=DR)
            nc.scalar.activation(u_sb, u_ps, Act.Copy, scale=1.0 / SC8)


        # ================= C' : C_k = W2[hs0,:] @ W2^T  (stream W2^T) =======
        pC_cm = tc.tile_pool(name="pC_s", bufs=3)
        pC = pC_cm.__enter__()
        pCp_cm = tc.tile_pool(name="pC_ps", bufs=1, space="PSUM")
        pCp = pCp_cm.__enter__()
        cps = pCp.tile([P, 8, 512], F32, tag="cps")
        for kt in range(KT2_D):
            if kt < KT2_DS:
                t_in = w2res[:, kt, :, :]
            else:
                t_raw = pC.tile([P, 2, H], FP8, tag="w2_t")
                dq = nc.sync if kt % 2 == 0 else nc.scalar
                dq.dma_start(
                    t_raw,
                    w2t_r[:, kt, :].rearrange("p (two h) -> p two h", two=2))
                t_in = t_raw
            for it in range(2):
                for nb in range(4):
                    nc.tensor.matmul(
                        cps[:, it * 4 + nb, :],
                        t_in[:, :, it * P:(it + 1) * P],
                        t_in[:, :, nb * 512:(nb + 1) * 512],
                        start=(kt == 0), stop=(kt == KT2_D - 1),
                        perf_mode=DR)
        for it in range(2):
            for nb in range(4):
                nc.scalar.activation(c_bf[:, it, nb * 512:(nb + 1) * 512],
                                     cps[:, it * 4 + nb, :], Act.Copy,
                                     scale=1.0 / (SC8 * SC8))
        pCp_cm.__exit__(None, None, None)
        pC_cm.__exit__(None, None, None)
        w2res_cm.__exit__(None, None, None)
        # C_k^T via PE transposes of 128x128 blocks
        with tc.tile_pool(name="p_ct", bufs=2, space="PSUM") as ct_ps_pool:
            for it in range(2):
                for jt in range(KT_H):
                    tp = ct_ps_pool.tile([P, P], BF16, tag="ct")
                    pe_T(nc, tp, c_bf[:, it, jt * P:(jt + 1) * P], identb)
                    nc.scalar.activation(ckT8[:, jt, it * P:(it + 1) * P], tp,
                                         Act.Copy, scale=16.0)

        # ---- z* full (+enc b2), zT, sig1, masks m1 (post-C') ----
        zf_sb = work.tile([B, N], F32, tag="zf")
        nc.sync.dma_start(zf_sb, zst_sh[:])
        zall = lin.tile([B, N], F32, tag="zall")
        zT_sb = work.tile([N, B], F32, tag="zT")
        with tc.tile_pool(name="p_z", bufs=2, space="PSUM") as pz:
            za_ps = pz.tile([B, N], F32, tag="za")
            nc.tensor.matmul(za_ps, ones1[:, 0:B], b2e_sb, start=True, stop=False)
            nc.tensor.matmul(za_ps, identf[0:B, 0:B], zf_sb, start=False,
                             stop=True)
            nc.vector.tensor_copy(zall, za_ps)
            zT_ps = pz.tile([N, B], F32, tag="za")
            pe_T(nc, zT_ps, zall, identf)
            nc.vector.tensor_copy(zT_sb, zT_ps)
        s1 = emit_sig(zall, "s1")
        invsp2 = lin.tile([B, 1], F32, tag="invsp2")
        sp2t = lin.tile([B, 1], F32, tag="sp2t")
        nc.vector.tensor_tensor(sp2t, s1[:, 0:1], s1[:, 0:1], Alu.mult)
        nc.vector.reciprocal(invsp2, sp2t)
        m1T_bf = work.tile([P, KT_H, B], BF16, tag="m1T")
        with tc.tile_pool(name="p_a1t", bufs=1, space="PSUM") as a1t_pool:
            a1T_ps = a1t_pool.tile([P, KT_H, B], F32, tag="a1T")
            for mt in range(KT_H):
                nc.tensor.matmul(a1T_ps[:, mt, :],
                                 w1_sb[:, mt * P:(mt + 1) * P], zT_sb,
                                 start=True, stop=False)
                nc.tensor.matmul(a1T_ps[:, mt, :],
                                 b1d_sb[:, mt * P:(mt + 1) * P], ones1[:, 0:B],
                                 start=False, stop=True)
            nc.vector.tensor_scalar(m1T_bf, a1T_ps, 0.0, None, Alu.is_gt)

        # ================= G partials (shared emitter) ======================
        at_pool = ctx.enter_context(tc.tile_pool(name="at_pool", bufs=2))
        g_pool = ctx.enter_context(tc.tile_pool(name="g_pool", bufs=1))

        def emit_G(mT_bf, tag, pack=None):
            # A is scaled x64 (fp8), C x16 (fp8): M1 psum = 1024*M1; m1sb8
            # stores 16*M1 (scale 16/1024); G psum = 64*16*G -> g_sb scale
            # 1/1024.
            g_sb = g_pool.tile([N, B * N], F32, tag=f"g_{tag}")
            with tc.tile_pool(name=f"pG{tag}", bufs=2, space="PSUM") as gps_pool:
                for cb in range(4):
                    AT = at_pool.tile([P, KT_H, 16, N], FP8, tag="AT")
                    nc.vector.tensor_tensor(
                        AT,
                        w1T8_sb.unsqueeze(2).broadcast_to([P, KT_H, 16, N]),
                        mT_bf[:, :, cb * 16:(cb + 1) * 16]
                        .unsqueeze(3).broadcast_to([P, KT_H, 16, N]),
                        Alu.mult)
                    m1ps = gps_pool.tile([P, 2, 512], F32, tag="m1ps")
                    for it in range(2):
                        for q in range(KT_H // 2):
                            nc.tensor.matmul(
                                m1ps[:, it, :],
                                ckT8[:, 2 * q:2 * q + 2,
                                     it * P:(it + 1) * P],
                                AT[:, 2 * q:2 * q + 2, :, :],
                                start=(q == 0), stop=(q == KT_H // 2 - 1),
                                perf_mode=DR)
                    m1sb = at_pool.tile([P, 2, 512], FP8, tag="m1sb")
                    nc.scalar.activation(m1sb, m1ps, Act.Copy,
                                         scale=16.0 / 1024.0)
                    gps = gps_pool.tile([N, 512], F32, tag="gps")
                    for s in range(16):
                        nc.tensor.matmul(
                            gps[:, s * N:(s + 1) * N],
                            AT[:, 0:2, s, :],
                            m1sb[:, :, s * N:(s + 1) * N],
                            start=True, stop=True, perf_mode=DR)
                    nc.vector.tensor_scalar(
                        g_sb[:, cb * 512:(cb + 1) * 512], gps,
                        1.0 / 1024.0, None, Alu.mult)
                    if pack is not None:
                        pack(cb, g_sb[:, cb * 512:(cb + 1) * 512])
            return g_sb

        # ---- G1 -> pack -> AllReduce ----
        g1_sb = emit_G(m1T_bf, "1", pack=lambda cb, view: nc.sync.dma_start(
            dram_ap(g1d, cb * 16 * N * N, [[N, N], [N * N, 16], [1, N]]),
            view))
        nc.gpsimd.collective_compute("AllReduce", Alu.add, replica_groups=RG,
                                     ins=[g1d[:]], outs=[g1_sh[:]])

        # ---- Prec assembly + LDLT + ltinv + dz + z_s ----
        Tm = lin.tile([B, N * N], F32, tag="Tm")
        nc.sync.dma_start(Tm, g1_sh[:])
        nc.vector.tensor_scalar(Tm, Tm, invsp2, None, Alu.mult)
        nc.vector.tensor_tensor(Tm, Tm, st_rep, Alu.add)
        diag1 = sub_ap(Tm, 0, [[N + 1, N]])
        nc.vector.tensor_scalar(diag1, diag1, 1.0, None, Alu.add)

        invD = lin.tile([B, N], F32, tag="invD")
        SCR = lin.tile([B, N * N], F32, tag="SCR")
        emit_ldlt(nc, Tm, SCR, invD, B)
        LT = lin.tile([B, N * N], F32, tag="LT")
        nc.vector.tensor_tensor(
            LT.rearrange("p (a b) -> p a b", b=N),
            Tm.rearrange("p (a b) -> p a b", b=N),
            invD.unsqueeze(1).broadcast_to([B, N, N]), Alu.mult)
        # dz: solve Lt^T dz = epss by backward substitution, one STT per
        # column: stored[0:k] <- (Ltrow_k * stored[k]) - stored[0:k], which
        # leaves stored[j] = (-1)^(N-1-j) dz[j]; fixed up with sgn_sb.
        srD = lin.tile([B, N], F32, tag="srD")
        nc.scalar.activation(srD, invD, Act.Sqrt)        # 1/sqrt(D)
        dz = lin.tile([B, N], F32, tag="dz")
        nc.vector.tensor_tensor(dz, eps_sb, srD, Alu.mult)
        for k in range(N - 1, 0, -1):
            lrow = sub_ap(LT, k * N, [[1, k]])
            nc.vector.scalar_tensor_tensor(
                dz[:, 0:k], lrow, dz[:, k:k + 1], dz[:, 0:k],
                Alu.mult, Alu.subtract)
        zs = lin.tile([B, N], F32, tag="zs")
        nc.vector.tensor_tensor(dz, dz, sgn_sb, Alu.mult)
        nc.vector.tensor_tensor(zs, zall, dz, Alu.add)
        s2 = emit_sig(zs, "s2")

        # ================= stage 2 ==========================================
        zsT_sb = work.tile([N, B], F32, tag="zsT")
        with tc.tile_pool(name="p_zst", bufs=1, space="PSUM") as zst_pool:
            zsT_ps = zst_pool.tile([N, B], F32, tag="zsT_ps")
            pe_T(nc, zsT_ps, zs, identf)
            nc.vector.tensor_copy(zsT_sb, zsT_ps)

        m2T_bf = work.tile([P, KT_H, B], BF16, tag="m2T")
        h2neg = work.tile([P, 2, B], BF16, tag="h2neg")
        with tc.tile_pool(name="p_a2t", bufs=1, space="PSUM") as a2t_pool:
            a2T_ps = a2t_pool.tile([P, KT_H, B], F32, tag="a2T")
            for mt in range(KT_H):
                nc.tensor.matmul(a2T_ps[:, mt, :],
                                 w1_sb[:, mt * P:(mt + 1) * P], zsT_sb,
                                 start=True, stop=False)
                nc.tensor.matmul(a2T_ps[:, mt, :],
                                 b1d_sb[:, mt * P:(mt + 1) * P], ones1[:, 0:B],
                                 start=False, stop=True)
            nc.vector.tensor_scalar(m2T_bf, a2T_ps, 0.0, None, Alu.is_gt)
            # -relu(a2) for local block (first 2 kt): min(-a2, 0)
            nc.vector.scalar_tensor_tensor(
                h2neg, a2T_ps[:, 0:2, :], -1.0, zeros2, Alu.mult, Alu.min)

        h2_sb = work.tile([B, H], BF16, tag="h2")
        with tc.tile_pool(name="p_a2", bufs=1, space="PSUM") as a2_pool:
            a2_ps = a2_pool.tile([B, H], F32, tag="a2")
            for nb in range(4):
                nc.tensor.matmul(a2_ps[:, nb * 512:(nb + 1) * 512],
                                 zsT_sb, w1_sb[:, nb * 512:(nb + 1) * 512],
                                 start=True, stop=False)
                nc.tensor.matmul(a2_ps[:, nb * 512:(nb + 1) * 512],
                                 ones1[:, 0:B], b1d_sb[:, nb * 512:(nb + 1) * 512],
                                 start=False, stop=True)
            nc.vector.tensor_scalar(h2_sb, a2_ps, 0.0, None, Alu.max)

        uh2 = lin.tile([B, 1], F32, tag="uh2")
        nc.vector.tensor_tensor(scr2, u_sb, h2_sb, Alu.mult)
        nc.vector.tensor_reduce(uh2, scr2, mybir.AxisListType.X, Alu.add)

        wd_sb = work.tile([B, H], BF16, tag="wd")
        vneg_bf = work.tile([B, H], BF16, tag="vneg")
        vh2m = lin.tile([B, 1], F32, tag="vh2m")
        with tc.tile_pool(name="p_v", bufs=1, space="PSUM") as v_pool:
            v_ps = v_pool.tile([B, H], F32, tag="v_ps")   # holds -v
            for nb in range(4):
                for it in range(2):
                    nc.tensor.matmul(v_ps[:, nb * 512:(nb + 1) * 512],
                                     h2neg[:, it, :],
                                     c_bf[:, it, nb * 512:(nb + 1) * 512],
                                     start=(it == 0), stop=(it == 1))
            nc.scalar.copy(vneg_bf, v_ps)
        nc.vector.tensor_tensor(scr2, vneg_bf, h2_sb, Alu.mult)
        nc.vector.tensor_reduce(vh2m, scr2, mybir.AxisListType.X, Alu.add)
        nc.vector.tensor_tensor(wd_sb, u_sb, vneg_bf, Alu.add)
        svec = lin.tile([B, 1], F32, tag="svec")
        nc.vector.scalar_tensor_tensor(svec, uh2, -2.0, xmbsq, Alu.mult, Alu.add)
        nc.vector.tensor_tensor(svec, svec, vh2m, Alu.subtract)

        # t_k = A2 wd_k : transpose wd, mask, matmul
        wdT_sb = work.tile([P, KT_H, B], BF16, tag="wdT")
        with tc.tile_pool(name="p_wdt", bufs=2, space="PSUM") as wdt_pool:
            for jt in range(KT_H):
                tp = wdt_pool.tile([P, B], BF16, tag="wdt")
                pe_T(nc, tp, wd_sb[:, jt * P:(jt + 1) * P], identb)
                nc.scalar.copy(wdT_sb[:, jt, :], tp)
        mwdT = work.tile([P, KT_H, B], BF16, tag="mwdT")
        nc.vector.tensor_tensor(mwdT, wdT_sb, m2T_bf, Alu.mult)
        tk_sb = work.tile([N, B], F32, tag="tk")
        with tc.tile_pool(name="p_tk", bufs=1, space="PSUM") as tk_pool:
            tk_ps = tk_pool.tile([N, B], F32, tag="tk_ps")
            for jt in range(KT_H):
                nc.tensor.matmul(tk_ps, w1Tb_sb[:, jt, :], mwdT[:, jt, :],
                                 start=(jt == 0), stop=(jt == KT_H - 1))
            nc.vector.tensor_copy(tk_sb, tk_ps)

        # ---- G2 + pack [t | svec | G2] -> AllReduce ----
        g2_sb = emit_G(m2T_bf, "2")
        nc.sync.dma_start(dram_ap(pkd, 0, [[1, N], [PKW, B]]), tk_sb)
        nc.sync.dma_start(dram_ap(pkd, N, [[PKW, B]]), svec)
        nc.sync.dma_start(
            dram_ap(pkd, N + 1, [[N, N], [PKW, B], [1, N]]), g2_sb)
        nc.gpsimd.collective_compute("AllReduce", Alu.add, replica_groups=RG,
                                     ins=[pkd[:]], outs=[pk_sh[:]])

        # ---- background (overlaps AllReduce): ltinv, tr inputs ----
        X1 = lin.tile([B, N * N], F32, tag="X1")
        nc.vector.memset(X1, 0.0)
        nc.vector.memset(sub_ap(X1, 0, [[N + 1, N]]), 1.0)
        emit_ltinv(nc, LT, X1, SCR, B)
        scrB = lin.tile([B, N * N], F32, tag="scrB")
        nc.vector.tensor_tensor(SCR, X1, X1, Alu.mult)
        trv = lin.tile([B, 1], F32, tag="trv")
        nc.vector.tensor_tensor(
            scrB.rearrange("p (a b) -> p a b", b=N),
            SCR.rearrange("p (a b) -> p a b", b=N),
            invD.unsqueeze(2).broadcast_to([B, N, N]), Alu.mult)
        logs = lin.tile([B, N], F32, tag="logs")
        nc.scalar.activation(logs, invD, Act.Ln)
        zsq = lin.tile([B, N], F32, tag="zsq")
        nc.vector.tensor_tensor(zsq, zall, zall, Alu.mult)

        # ---- post-AllReduce: solve G2 y = t, d_proj, recon, out ----
        y = lin.tile([B, N], F32, tag="y")
        nc.sync.dma_start(y, pk_sh[:, 0:N])
        svf = lin.tile([B, 1], F32, tag="svf")
        nc.sync.dma_start(svf, pk_sh[:, N:N + 1])
        Tm2 = lin.tile([B, N * N], F32, tag="Tm2")
        nc.sync.dma_start(Tm2, pk_sh[:, N + 1:PKW])
        invD2 = lin.tile([B, N], F32, tag="invD2")
        emit_ldlt(nc, Tm2, SCR, invD2, B)
        LT2 = lin.tile([B, N * N], F32, tag="LT2")
        nc.vector.tensor_tensor(
            LT2.rearrange("p (a b) -> p a b", b=N),
            Tm2.rearrange("p (a b) -> p a b", b=N),
            invD2.unsqueeze(1).broadcast_to([B, N, N]), Alu.mult)
        emit_fwd_solve_alt(nc, LT2, y, B)
        ysq = lin.tile([B, N], F32, tag="ysq")
        yw = lin.tile([B, N], F32, tag="yw")
        dproj = lin.tile([B, 1], F32, tag="dproj")
        nc.vector.tensor_tensor(ysq, y, y, Alu.mult)
        nc.vector.tensor_tensor(yw, ysq, invD2, Alu.mult)
        nc.vector.tensor_reduce(dproj, yw, mybir.AxisListType.X, Alu.add)

        ldv = lin.tile([B, 1], F32, tag="ldv")
        latv = lin.tile([B, 1], F32, tag="latv")
        nc.vector.tensor_reduce(trv, scrB, mybir.AxisListType.X, Alu.add)
        nc.vector.tensor_reduce(ldv, logs, mybir.AxisListType.X, Alu.add)
        nc.vector.tensor_reduce(latv, zsq, mybir.AxisListType.X, Alu.add)
        nc.vector.tensor_tensor(latv, latv, trv, Alu.add)
        nc.vector.tensor_scalar(latv, latv, 0.5, None, Alu.mult)
        nc.vector.tensor_scalar(ldv, ldv, -0.5, None, Alu.mult)

        sq2 = lin.tile([B, 2], F32, tag="sq2")
        nc.vector.tensor_tensor(sq2, s2, s2, Alu.mult)
        nc.vector.tensor_scalar(sq2, sq2, 2.0, None, Alu.mult)
        inv2 = lin.tile([B, 2], F32, tag="inv2")
        nc.vector.reciprocal(inv2, sq2)     # [1/(2sp2^2), 1/(2sv2^2)]
        logs2 = lin.tile([B, 2], F32, tag="logs2")
        logw = lin.tile([B, 2], F32, tag="logw")
        nc.vector.memset(logw[:, 0:1], float(N))
        nc.vector.memset(logw[:, 1:2], float(D - N))
        nc.scalar.activation(logs2, s2, Act.Ln)
        logterm = lin.tile([B, 1], F32, tag="logterm")
        junk2 = lin.tile([B, 2], F32, tag="junk2")
        nc.vector.tensor_tensor(junk2, logs2, logw, Alu.mult)
        nc.vector.tensor_reduce(logterm, junk2, mybir.AxisListType.X, Alu.add)
        isub = lin.tile([B, 1], F32, tag="isub")
        nc.vector.tensor_tensor(isub, inv2[:, 0:1], inv2[:, 1:2], Alu.subtract)
        recon = lin.tile([B, 1], F32, tag="recon")
        nc.vector.tensor_tensor(recon, dproj, isub, Alu.mult)
        p2t = lin.tile([B, 1], F32, tag="p2t")
        nc.vector.tensor_tensor(p2t, svf, inv2[:, 1:2], Alu.mult)
        nc.vector.tensor_tensor(recon, recon, p2t, Alu.add)
        nc.vector.tensor_tensor(recon, recon, logterm, Alu.add)
        ov = lin.tile([B, 1], F32, tag="ov")
        nc.vector.tensor_tensor(ov, recon, latv, Alu.add)
        nc.vector.tensor_tensor(ov, ov, ldv, Alu.add)
        nc.vector.tensor_scalar(ov, ov, 1.0 / D, None, Alu.mult)
        nc.sync.dma_start(out[:], ov)

    legalize_waits(nc)
    return nc


def shard_inputs(inputs):
    """Host-side prep: per-core H-permutation + D-tile reordering."""
    bf = ml_dtypes.bfloat16
    x = np.ascontiguousarray(np.asarray(inputs["x"], np.float32))
    eps = np.ascontiguousarray(np.asarray(inputs["eps"], np.float32))
    eW1 = np.asarray(inputs["enc_W1"], np.float32)
    eb1 = np.asarray(inputs["enc_b1"], np.float32)
    eW2 = np.asarray(inputs["enc_W2"], np.float32)
    eb2 = np.asarray(inputs["enc_b2"], np.float32)
    dW1 = np.asarray(inputs["dec_W1"], np.float32)
    db1 = np.asarray(inputs["dec_b1"], np.float32)
    dW2 = np.asarray(inputs["dec_W2"], np.float32)
    db2 = np.asarray(inputs["dec_b2"], np.float32)
    sW = np.asarray(inputs["sig_W"], np.float32)
    sb = np.asarray(inputs["sig_b"], np.float32)

    xT_bf = np.ascontiguousarray(x.T).astype(bf)
    xmb_full = x - db2[None, :]
    W2T = np.ascontiguousarray(dW2.T)      # [D, H]
    sigv = np.zeros((1, 130), np.float32)
    sigv[0, 0:32] = sW[:, 0]
    sigv[0, 32:64] = sW[:, 1]
    sigv[0, 64:66] = sb
    sigv[0, 66:98] = sW[:, 0] * np.sqrt(N / 2.0)
    sigv[0, 98:130] = sW[:, 1] * np.sqrt((D - N) / 2.0)

    maps = []
    for k in range(NCORES):
        hperm = np.concatenate([np.arange(k * HS, (k + 1) * HS),
                                np.arange(0, k * HS),
                                np.arange((k + 1) * HS, H)])
        # D-tile order: own 12 tiles first
        own = np.arange(k * KT_DS, (k + 1) * KT_DS)
        rest = np.concatenate([np.arange(0, k * KT_DS),
                               np.arange((k + 1) * KT_DS, KT_D)])
        tord = np.concatenate([own, rest])
        w2t_p = W2T[:, hperm].reshape(KT_D, P, H)[tord].reshape(D, H)
        w2q = np.asarray(w2t_p * 256.0, np.float32).astype(ml_dtypes.float8_e4m3)
        w2q = np.ascontiguousarray(
            w2q.reshape(KT_D // 2, 2, P, H).transpose(0, 2, 1, 3)
            .reshape(D // 2, 2 * H))
        dsl = slice(k * DS, (k + 1) * DS)
        xq = np.asarray(xmb_full[:, dsl].T, np.float32).astype(
            ml_dtypes.float8_e4m3)
        xq = np.ascontiguousarray(
            xq.reshape(KT_DS // 2, 2, P, B).transpose(0, 2, 1, 3)
            .reshape(DS // 2, 2 * B))
        maps.append({
            "w2t8": w2q,
            "xt": xT_bf,
            "w1es": np.ascontiguousarray(
                eW1[:, k * HS:(k + 1) * HS]).astype(bf),
            "b1es": np.ascontiguousarray(eb1[None, k * HS:(k + 1) * HS]),
            "w2es": np.ascontiguousarray(eW2[k * HS:(k + 1) * HS, :]),
            "b2e": np.ascontiguousarray(eb2[None, :]),
            "w1p": np.ascontiguousarray(dW1[:, hperm]),
            "w1tp_bf": np.ascontiguousarray(dW1[:, hperm].T).astype(bf),
            "w1tp8": np.ascontiguousarray(dW1[:, hperm].T * 64.0).astype(
                ml_dtypes.float8_e4m3),
            "b1dp": np.ascontiguousarray(db1[None, hperm]),
            "xmbt8": xq,
            "xmb": np.ascontiguousarray(xmb_full[:, dsl]),
            "sigw": sigv,
            "epsin": eps,
        })
    return maps


_NC_CACHE = None


def kernel(**inputs) -> np.ndarray:
    global _NC_CACHE
    from concourse.bass_utils import run_bass_kernel_spmd
    if _NC_CACHE is None:
        _NC_CACHE = build_nc()
    nc = _NC_CACHE
    maps = shard_inputs(inputs)
    res = run_bass_kernel_spmd(nc, maps, list(range(NCORES)))
    return np.asarray(res.results[0]["out"]).reshape(B).astype(np.float32)


# revision 18
# speedup vs baseline: 1.4687x; 1.0633x over previous
"""Trainium2 Bass kernel for nn_EnergyAE (B=64, D=12288, N=32, H=2048) on 8 cores.

v2 restructure (vs v1's contraction-sharded C + 16.8MB AllReduce):
  - C block-row sharding: core k computes C_k = C[hs_k, :] (256 x 2048) locally
    by streaming full W2^T (bf16, 12.6MB); C_k stays in SBUF. No C collective,
    no C DRAM round trip.
  - Per-core H-permutation (host-side) puts each core's block at H-cols 0:256,
    so the SPMD program is core-index-free.
  - G[b] computed as partial sums over ALL 64 samples using C_k; AllReduce of
    packed G (256KB). Every core then factors all 64 (same vector cost) ->
    no z_s AllGather, no sel8 gathers.
  - x_star never materialized: W2 delta = W2(x-b2) - C h2, so
    u = xmb @ W2s^T (partial), v = h2[:, 0:256] @ C_k (partial),
    wd_k = u_k - v_k, t_k = A2 wd_k (pre-AllReduce!),
    d_sq = |xmb|^2 - 2 u.h2 + v.h2 (scalar partials).
  - Final AllReduce packs [t | svec | G2] = 271KB. All cores compute all 64
    outputs; host takes core 0's.

Identities (validated numerically, rel err ~1.7e-3 with bf16 C/A/u):
  Prec = Lt D Lt^T (unit-lower LDLT)
  sum(log eig)/2 = 0.5*sum(log D);  sum(1/eig) = ||D^-1/2 Lt^-1||_F^2
  U^-1 eps = Lt^-T (eps/sqrt(D));   t^T G2^-1 t = ||D2^-1/2 Lt2^-1 t||^2
  sig_term = (n w0 w0^T + (D-n) w1 w1^T)/2   (constant across batch)
"""
import sys

for _p in ("/opt/trn_rl_repo", "/root/.axon_site/_ro/trn_rl_repo"):
    if _p not in sys.path:
        sys.path.append(_p)

import numpy as np
import ml_dtypes
from contextlib import ExitStack

import concourse.bass as bass
import concourse.mybir as mybir
import concourse.tile as tile
from concourse.masks import make_identity

B, D, N, H = 64, 12288, 32, 2048
NCORES = 8
BL = B // NCORES          # 8 local samples (only used for host sharding)
HS = H // NCORES          # 256: C block rows per core
DS = D // NCORES          # 1536
KT_D = D // 128           # 96
KT_DS = DS // 128         # 12
KT_H = H // 128           # 16
P = 128

F32 = mybir.dt.float32
F32R = mybir.dt.float32r
BF16 = mybir.dt.bfloat16
FP8 = mybir.dt.float8e4
SC8 = 256.0
Alu = mybir.AluOpType
Act = mybir.ActivationFunctionType
RG = [list(range(NCORES))]


def sub_ap(t, extra_off, dims):
    """Custom free-dim AP on a [P, F] tile; dims = [[step,count],...] in elems."""
    base = t[:, 0:1]
    return bass.AP(base.tensor, base.offset + extra_off, [base.ap[0]] + dims)


def dram_ap(t, off, dims):
    """Custom AP into a DRAM tensor; dims = [[step,count],...] in elems."""
    base = t[:]
    return bass.AP(base.tensor, off, dims)


def pe_T(nc, out_ps, in_ap, ident):
    """PE transpose: out_ps [f, p] = in_ap [p, f].T"""
    kp = in_ap.shape[0]
    nc.tensor.transpose(out_ps, in_ap, ident[0:kp, 0:kp])


def emit_ldlt(nc, T, OUT, invD, rows, n=32):
    """In-place unit-lower LDLT of T [rows, n*n] (row-major per sample).
    After: strict lower of T holds unscaled columns u; diag holds D; invD=1/D."""
    for j in range(n):
        nc.vector.reciprocal(invD[:, j:j + 1], T[:, (n + 1) * j:(n + 1) * j + 1])
        m = n - 1 - j
        if m == 0:
            break
        base = (j + 1) * n + j
        u_i = sub_ap(T, base, [[n, m], [0, m]])
        u_k = sub_ap(T, base, [[0, m], [n, m]])
        outer = sub_ap(OUT, 0, [[m, m], [1, m]])
        nc.vector.scalar_tensor_tensor(
            outer, u_i, invD[:, j:j + 1], u_k, Alu.mult, Alu.mult)
        trail = sub_ap(T, (j + 1) * (n + 1), [[n, m], [1, m]])
        nc.vector.tensor_tensor(trail, trail, outer, Alu.subtract)


def emit_ltinv(nc, LT, X, OUT, rows, n=32):
    """X = LT^{-1} for unit-lower LT [rows, n*n]; X preset to I by caller."""
    for k in range(n - 1):
        rr = n - 1 - k
        cols = k + 1
        lcol = sub_ap(LT, (k + 1) * n + k, [[n, rr], [0, cols]])
        xrow = sub_ap(X, k * n, [[0, rr], [1, cols]])
        prod = sub_ap(OUT, 0, [[cols, rr], [1, cols]])
        nc.vector.scalar_tensor_tensor(prod, lcol, -1.0, xrow, Alu.mult, Alu.mult)
        xblk = sub_ap(X, (k + 1) * n, [[n, rr], [1, cols]])
        nc.vector.tensor_tensor(xblk, xblk, prod, Alu.add)


def emit_fwd_solve_alt(nc, LT, w, rows, n=32):
    """Forward-substitute LT y = w in place, ONE STT per column:
      stored[k+1:] <- (LT[k+1:,k] * stored[k]) - stored[k+1:]
    This leaves stored[j] = (-1)^j * y[j] (every update flips the sign of the
    remaining entries, and the scalar operand carries the matching sign), so
    it is valid whenever the caller only consumes y elementwise-squared."""
    for k in range(n - 1):
        rr = n - 1 - k
        lcol = sub_ap(LT, (k + 1) * n + k, [[n, rr]])
        nc.vector.scalar_tensor_tensor(
            w[:, k + 1:n], lcol, w[:, k:k + 1], w[:, k + 1:n],
            Alu.mult, Alu.subtract)


def emit_fwd_solve(nc, LT, y, OUT, rows, n=32):
    """y <- LT^{-1} y for unit-lower LT [rows, n*n], y [rows, n] in place."""
    for k in range(n - 1):
        rr = n - 1 - k
        lcol = sub_ap(LT, (k + 1) * n + k, [[n, rr]])
        nc.vector.scalar_tensor_tensor(
            OUT[:, 0:rr], lcol, -1.0, y[:, k:k + 1].broadcast_to([rows, rr]),
            Alu.mult, Alu.mult)
        nc.vector.tensor_tensor(y[:, k + 1:n], y[:, k + 1:n], OUT[:, 0:rr], Alu.add)


def legalize_waits(nc, maxw=1):
    """Split multi-wait sync_info into standalone EventSemaphore instructions."""
    for f in nc.m.functions:
        for bb in f.blocks:
            insts = list(bb.instructions)
            out = []
            changed = False
            for inst in insts:
                si = inst.sync_info
                if si is not None and si.on_wait and len(si.on_wait) > maxw:
                    waits = list(si.on_wait)
                    imm = [w for w in waits if w.uses_immediate]
                    reg = [w for w in waits if not w.uses_immediate]
                    keep = (reg + imm)[:maxw] if len(reg) <= maxw else reg
                    extra = [w for w in waits if w not in keep]
                    if len(keep) > maxw:
                        raise RuntimeError(f"{inst.name}: {len(keep)} register waits")
                    for w in extra:
                        ev = mybir.InstEventSemaphore(
                            name=nc.get_next_instruction_name(), ins=[], outs=[])
                        ev.engine = inst.engine
                        ev.sync_info = mybir.SyncInfo(on_wait=[w], on_update=[])
                        out.append(ev)
                    inst.sync_info = mybir.SyncInfo(
                        on_wait=keep, on_update=list(si.on_update or []))
                    changed = True
                out.append(inst)
            if changed:
                bb.instructions = out
    return nc


def build_nc():
    nc = bass.Bass()

    # ---- I/O (per-core views prepared by host; H-permuted, D-tile-reordered)
    w2t8 = nc.dram_tensor("w2t8", [D // 2, 2 * H], FP8, kind="ExternalInput")
    xt = nc.dram_tensor("xt", [D, B], BF16, kind="ExternalInput")
    w1es = nc.dram_tensor("w1es", [D, HS], BF16, kind="ExternalInput")
    b1es = nc.dram_tensor("b1es", [1, HS], F32, kind="ExternalInput")
    w2es = nc.dram_tensor("w2es", [HS, N], F32, kind="ExternalInput")
    b2e = nc.dram_tensor("b2e", [1, N], F32, kind="ExternalInput")
    w1p = nc.dram_tensor("w1p", [N, H], F32, kind="ExternalInput")
    w1tp_bf = nc.dram_tensor("w1tp_bf", [H, N], BF16, kind="ExternalInput")
    w1tp8 = nc.dram_tensor("w1tp8", [H, N], FP8, kind="ExternalInput")
    b1dp = nc.dram_tensor("b1dp", [1, H], F32, kind="ExternalInput")
    xmbt8 = nc.dram_tensor("xmbt8", [DS // 2, 2 * B], FP8, kind="ExternalInput")
    xmb = nc.dram_tensor("xmb", [B, DS], F32, kind="ExternalInput")
    sigw = nc.dram_tensor("sigw", [1, 130], F32, kind="ExternalInput")
    epsin = nc.dram_tensor("epsin", [B, N], F32, kind="ExternalInput")
    out = nc.dram_tensor("out", [B, 1], F32, kind="ExternalOutput")

    # ---- internal DRAM ----
    zstd = nc.dram_tensor("zstd", [B, N], F32)
    zst_sh = nc.dram_tensor("zst_sh", [B, N], F32, addr_space="Shared")
    g1d = nc.dram_tensor("g1d", [B, N * N], F32)
    g1_sh = nc.dram_tensor("g1_sh", [B, N * N], F32, addr_space="Shared")
    PKW = N + 1 + N * N   # 1057
    pkd = nc.dram_tensor("pkd", [B, PKW], F32)
    pk_sh = nc.dram_tensor("pk_sh", [B, PKW], F32, addr_space="Shared")

    with tile.TileContext(nc) as tc, ExitStack() as ctx:
        consts = ctx.enter_context(tc.tile_pool(name="consts", bufs=1))
        work = ctx.enter_context(tc.tile_pool(name="work", bufs=1))
        lin = ctx.enter_context(tc.tile_pool(name="lin", bufs=1))
        pre_cm = tc.tile_pool(name="pre_ps", bufs=2, space="PSUM")
        pre_ps = pre_cm.__enter__()

        # ---- constants ----
        identf = consts.tile([P, P], F32)
        make_identity(nc, identf)
        identb = consts.tile([P, P], BF16)
        make_identity(nc, identb)
        wu_d = nc.dram_tensor("wu_d", [1, 16], F32)
        wu_sh = nc.dram_tensor("wu_sh", [1, 16], F32, addr_space="Shared")
        nc.gpsimd.collective_compute("AllReduce", Alu.add, replica_groups=RG,
                                     ins=[wu_d[:]], outs=[wu_sh[:]])
        ones1 = consts.tile([1, B], F32)
        nc.vector.memset(ones1, 1.0)
        zeros2 = consts.tile([P, 2, B], F32)
        nc.vector.memset(zeros2, 0.0)
        sigw_sb = consts.tile([1, 130], F32)
        nc.sync.dma_start(sigw_sb, sigw[:])
        eps_sb = consts.tile([B, N], F32)
        nc.sync.dma_start(eps_sb, epsin[:])
        b1es_sb = consts.tile([1, HS], F32)
        nc.sync.dma_start(b1es_sb, b1es[:])
        b2e_sb = consts.tile([1, N], F32)
        nc.sync.dma_start(b2e_sb, b2e[:])
        b1d_sb = consts.tile([1, H], F32)
        nc.sync.dma_start(b1d_sb, b1dp[:])
        w2es_sb = consts.tile([P, 2, N], F32)
        nc.sync.dma_start(w2es_sb, w2es[:].rearrange("(k p) n -> p k n", p=P))
        w1_sb = consts.tile([N, H], F32)
        nc.sync.dma_start(w1_sb, w1p[:])
        w1Tb_sb = consts.tile([P, KT_H, N], BF16)
        nc.sync.dma_start(w1Tb_sb, w1tp_bf[:].rearrange("(k p) n -> p k n", p=P))
        w1T8_sb = consts.tile([P, KT_H, N], FP8)
        nc.sync.dma_start(w1T8_sb, w1tp8[:].rearrange("(k p) n -> p k n", p=P))
        sgn_sb = consts.tile([B, N], F32)
        nc.vector.memset(sgn_sb, -1.0)
        nc.vector.memset(sub_ap(sgn_sb, 1, [[2, N // 2]]), 1.0)

        sigw_rep = consts.tile([B, 130], F32)
        sigw_ps = pre_ps.tile([B, 130], F32, tag="sp")
        nc.tensor.matmul(sigw_ps, ones1, sigw_sb, start=True, stop=True)
        nc.vector.tensor_copy(sigw_rep, sigw_ps)

        # sig_term replicated [B, N*N]
        st_ps = pre_ps.tile([N, N], F32, tag="sp")
        nc.tensor.matmul(st_ps, sigw_sb[:, 66:98], sigw_sb[:, 66:98],
                         start=True, stop=False)
        nc.tensor.matmul(st_ps, sigw_sb[:, 98:130], sigw_sb[:, 98:130],
                         start=False, stop=True)
        st_sb = work.tile([N, N], F32, tag="st_sb")
        nc.vector.tensor_copy(st_sb, st_ps)
        st_flat = work.tile([1, N * N], F32, tag="st_flat")
        nc.sync.dma_start(st_flat, st_sb)
        st_rep = consts.tile([B, N * N], F32)
        for hh in range(2):
            sps2 = pre_ps.tile([B, 512], F32, tag="sp")
            nc.tensor.matmul(sps2, ones1, st_flat[:, hh * 512:(hh + 1) * 512],
                             start=True, stop=True)
            nc.vector.tensor_copy(st_rep[:, hh * 512:(hh + 1) * 512], sps2)

        def emit_sig(z_in, name):
            lg = lin.tile([B, 2, N], F32, tag="sig_lg")
            nc.vector.tensor_tensor(
                lg, z_in.unsqueeze(1).broadcast_to([B, 2, N]),
                sigw_rep[:, 0:64].rearrange("p (c n) -> p c n", c=2), Alu.mult)
            red = lin.tile([B, 2], F32, tag=f"sig_red_{name}")
            nc.vector.tensor_reduce(red, lg, mybir.AxisListType.X, Alu.add)
            nc.vector.tensor_tensor(red, red, sigw_rep[:, 64:66], Alu.add)
            s = lin.tile([B, 2], F32, tag=f"sig_s_{name}")
            nc.scalar.activation(s, red, Act.Exp)
            return s

        # ================= encoder (model-parallel over enc-H) ==============
        with tc.tile_pool(name="p_enc_s", bufs=2) as enc_s, \
             tc.tile_pool(name="p_enc_ps", bufs=1, space="PSUM") as enc_ps:
            xt_r = xt[:].rearrange("(k p) b -> p k b", p=P)
            w1es_r = w1es[:].rearrange("(k p) h -> p k h", p=P)
            a1e_ps = enc_ps.tile([B, HS], F32, tag="a1e")
            for kb in range(12):
                xtile = enc_s.tile([P, 8, B], BF16, tag="xt_t")
                nc.scalar.dma_start(xtile, xt_r[:, kb * 8:(kb + 1) * 8, :])
                wtile = enc_s.tile([P, 8, HS], BF16, tag="w1es_t")
                nc.scalar.dma_start(wtile, w1es_r[:, kb * 8:(kb + 1) * 8, :])
                for j in range(8):
                    nc.tensor.matmul(a1e_ps, xtile[:, j, :], wtile[:, j, :],
                                     start=(kb == 0 and j == 0), stop=False)
            nc.tensor.matmul(a1e_ps, ones1[:, 0:B], b1es_sb,
                             start=False, stop=True)
            h1_sb = work.tile([B, HS], F32, tag="h1")
            nc.vector.tensor_scalar(h1_sb, a1e_ps, 0.0, None, Alu.max)
            h1T_sb = work.tile([P, 2, B], F32, tag="h1T")
            for i in range(2):
                tp = enc_ps.tile([P, B], F32, tag="tp")
                pe_T(nc, tp, h1_sb[:, i * P:(i + 1) * P], identf)
                nc.scalar.copy(h1T_sb[:, i, :], tp)
            zp_ps = enc_ps.tile([B, N], F32, tag="zp")
            for i in range(2):
                nc.tensor.matmul(zp_ps, h1T_sb[:, i, :], w2es_sb[:, i, :],
                                 start=(i == 0), stop=(i == 1))
            zp_sb = work.tile([B, N], F32, tag="zp_sb")
            nc.vector.tensor_copy(zp_sb, zp_ps)
            nc.sync.dma_start(zstd[:], zp_sb)
        nc.gpsimd.collective_compute("AllReduce", Alu.add, replica_groups=RG,
                                     ins=[zstd[:]], outs=[zst_sh[:]])

        pre_cm.__exit__(None, None, None)

        # ========= u = xmb @ W2s^T (partial, fp8 DoubleRow), |xmb|^2 =======
        KT2_D = KT_D // 2       # 48 double-row tiles
        KT2_DS = KT_DS // 2     # 6 own tiles
        cpool = ctx.enter_context(tc.tile_pool(name="cpool", bufs=1))
        c_bf = cpool.tile([P, 2, H], BF16, tag="c_bf")
        ckT8 = cpool.tile([P, KT_H, 2 * P], FP8, tag="ckT")
        w2t_r = w2t8[:].rearrange("(k p) f -> p k f", p=P)
        w2res_pool = ctx.enter_context(tc.tile_pool(name="w2res", bufs=1))
        w2res = w2res_pool.tile([P, KT2_DS, 2, H], FP8, tag="w2res")
        for kt in range(KT2_DS):
            nc.sync.dma_start(
                w2res[:, kt, :, :],
                w2t_r[:, kt, :].rearrange("p (two h) -> p two h", two=2))
        xmbT_sb = work.tile([P, KT2_DS, 2, B], FP8, tag="xmbT")
        nc.sync.dma_start(
            xmbT_sb,
            xmbt8[:].rearrange("(k p) (two b) -> p k two b", p=P, two=2))
        xmb_sb = work.tile([B, DS], F32, tag="xmb")
        nc.sync.dma_start(xmb_sb, xmb[:])
        xmbsq = lin.tile([B, 1], F32, tag="xmbsq")
        scr2 = work.tile([B, H], BF16, tag="scr2")
        nc.scalar.activation(scr2[:, 0:DS], xmb_sb, Act.Square,
                             accum_out=xmbsq)
        DR = mybir.MatmulPerfMode.DoubleRow


        # ================= C' : C_k = W2[hs0,:] @ W2^T  (stream W2^T) =======
        pC_cm = tc.tile_pool(name="pC_s", bufs=6)
        pC = pC_cm.__enter__()
        pCp_cm = tc.tile_pool(name="pC_ps", bufs=1, space="PSUM")
        pCp = pCp_cm.__enter__()
        cps = pCp.tile([P, 8, 512], F32, tag="cps")
        for kt in range(KT2_D):
            if kt < KT2_DS:
                t_in = w2res[:, kt, :, :]
            else:
                t_raw = pC.tile([P, 2, H], FP8, tag="w2_t")
                dq = nc.sync if kt % 2 == 0 else nc.scalar
                dq.dma_start(
                    t_raw,
                    w2t_r[:, kt, :].rearrange("p (two h) -> p two h", two=2))
                t_in = t_raw
            for it in range(2):
                for nb in range(4):
                    nc.tensor.matmul(
                        cps[:, it * 4 + nb, :],
                        t_in[:, :, it * P:(it + 1) * P],
                        t_in[:, :, nb * 512:(nb + 1) * 512],
                        start=(kt == 0), stop=(kt == KT2_D - 1),
                        perf_mode=DR)
        for it in range(2):
            for nb in range(4):
                nc.scalar.activation(c_bf[:, it, nb * 512:(nb + 1) * 512],
                                     cps[:, it * 4 + nb, :], Act.Copy,
                                     scale=1.0 / (SC8 * SC8))
        pCp_cm.__exit__(None, None, None)
        pC_cm.__exit__(None, None, None)
        # C_k^T via PE transposes of 128x128 blocks
        with tc.tile_pool(name="p_ct", bufs=2, space="PSUM") as ct_ps_pool:
            for it in range(2):
                for jt in range(KT_H):
                    tp = ct_ps_pool.tile([P, P], BF16, tag="ct")
                    pe_T(nc, tp, c_bf[:, it, jt * P:(jt + 1) * P], identb)
                    nc.scalar.activation(ckT8[:, jt, it * P:(it + 1) * P], tp,
                                         Act.Copy, scale=16.0)

        # ---- z* full (+enc b2), zT, sig1, masks m1 (post-C') ----
        zf_sb = work.tile([B, N], F32, tag="zf")
        nc.sync.dma_start(zf_sb, zst_sh[:])
        zall = lin.tile([B, N], F32, tag="zall")
        zT_sb = work.tile([N, B], F32, tag="zT")
        with tc.tile_pool(name="p_z", bufs=2, space="PSUM") as pz:
            za_ps = pz.tile([B, N], F32, tag="za")
            nc.tensor.matmul(za_ps, ones1[:, 0:B], b2e_sb, start=True, stop=False)
            nc.tensor.matmul(za_ps, identf[0:B, 0:B], zf_sb, start=False,
                             stop=True)
            nc.vector.tensor_copy(zall, za_ps)
            zT_ps = pz.tile([N, B], F32, tag="za")
            pe_T(nc, zT_ps, zall, identf)
            nc.vector.tensor_copy(zT_sb, zT_ps)
        s1 = emit_sig(zall, "s1")
        invsp2 = lin.tile([B, 1], F32, tag="invsp2")
        sp2t = lin.tile([B, 1], F32, tag="sp2t")
        nc.vector.tensor_tensor(sp2t, s1[:, 0:1], s1[:, 0:1], Alu.mult)
        nc.vector.reciprocal(invsp2, sp2t)
        m1T_bf = work.tile([P, KT_H, B], BF16, tag="m1T")
        with tc.tile_pool(name="p_a1t", bufs=1, space="PSUM") as a1t_pool:
            a1T_ps = a1t_pool.tile([P, KT_H, B], F32, tag="a1T")
            for mt in range(KT_H):
                nc.tensor.matmul(a1T_ps[:, mt, :],
                                 w1_sb[:, mt * P:(mt + 1) * P], zT_sb,
                                 start=True, stop=False)
                nc.tensor.matmul(a1T_ps[:, mt, :],
                                 b1d_sb[:, mt * P:(mt + 1) * P], ones1[:, 0:B],
                                 start=False, stop=True)
            nc.vector.tensor_scalar(m1T_bf, a1T_ps, 0.0, None, Alu.is_gt)

        # ================= G partials (shared emitter) ======================
        at_pool = ctx.enter_context(tc.tile_pool(name="at_pool", bufs=2))
        g_pool = ctx.enter_context(tc.tile_pool(name="g_pool", bufs=1))

        def emit_G(mT_bf, tag, pack=None):
            # A is scaled x64 (fp8), C x16 (fp8): M1 psum = 1024*M1; m1sb8
            # stores 16*M1 (scale 16/1024); G psum = 64*16*G -> g_sb scale
            # 1/1024.
            g_sb = g_pool.tile([N, B * N], F32, tag=f"g_{tag}")
            with tc.tile_pool(name=f"pG{tag}", bufs=2, space="PSUM") as gps_pool:
                for cb in range(4):
                    AT = at_pool.tile([P, KT_H, 16, N], FP8, tag="AT")
                    nc.vector.tensor_tensor(
                        AT,
                        w1T8_sb.unsqueeze(2).broadcast_to([P, KT_H, 16, N]),
                        mT_bf[:, :, cb * 16:(cb + 1) * 16]
                        .unsqueeze(3).broadcast_to([P, KT_H, 16, N]),
                        Alu.mult)
                    m1ps = gps_pool.tile([P, 2, 512], F32, tag="m1ps")
                    for it in range(2):
                        for q in range(KT_H // 2):
                            nc.tensor.matmul(
                                m1ps[:, it, :],
                                ckT8[:, 2 * q:2 * q + 2,
                                     it * P:(it + 1) * P],
                                AT[:, 2 * q:2 * q + 2, :, :],
                                start=(q == 0), stop=(q == KT_H // 2 - 1),
                                perf_mode=DR)
                    m1sb = at_pool.tile([P, 2, 512], FP8, tag="m1sb")
                    nc.scalar.activation(m1sb, m1ps, Act.Copy,
                                         scale=16.0 / 1024.0)
                    gps = gps_pool.tile([N, 512], F32, tag="gps")
                    for s in range(16):
                        nc.tensor.matmul(
                            gps[:, s * N:(s + 1) * N],
                            AT[:, 0:2, s, :],
                            m1sb[:, :, s * N:(s + 1) * N],
                            start=True, stop=True, perf_mode=DR)
                    nc.vector.tensor_scalar(
                        g_sb[:, cb * 512:(cb + 1) * 512], gps,
                        1.0 / 1024.0, None, Alu.mult)
                    if pack is not None:
                        pack(cb, g_sb[:, cb * 512:(cb + 1) * 512])
            return g_sb

        # ---- G1 -> pack -> AllReduce ----
        g1_sb = emit_G(m1T_bf, "1", pack=lambda cb, view: nc.sync.dma_start(
            dram_ap(g1d, cb * 16 * N * N, [[N, N], [N * N, 16], [1, N]]),
            view))
        nc.gpsimd.collective_compute("AllReduce", Alu.add, replica_groups=RG,
                                     ins=[g1d[:]], outs=[g1_sh[:]])

        # u matmuls run here: PE is otherwise idle during AR-G1 + LDLT1
        u_sb = work.tile([B, H], BF16, tag="u_sb")
        with tc.tile_pool(name="p_u", bufs=1, space="PSUM") as u_ps_pool:
            u_ps = u_ps_pool.tile([B, H], F32, tag="u_ps")
            for nb in range(4):
                for kt in range(KT2_DS):
                    nc.tensor.matmul(
                        u_ps[:, nb * 512:(nb + 1) * 512],
                        xmbT_sb[:, kt, :, :],
                        w2res[:, kt, :, nb * 512:(nb + 1) * 512],
                        start=(kt == 0), stop=(kt == KT2_DS - 1),
                        perf_mode=DR)
            nc.scalar.activation(u_sb, u_ps, Act.Copy, scale=1.0 / SC8)

        # ---- Prec assembly + LDLT + ltinv + dz + z_s ----
        Tm = lin.tile([B, N * N], F32, tag="Tm")
        nc.sync.dma_start(Tm, g1_sh[:])
        nc.vector.tensor_scalar(Tm, Tm, invsp2, None, Alu.mult)
        nc.vector.tensor_tensor(Tm, Tm, st_rep, Alu.add)
        diag1 = sub_ap(Tm, 0, [[N + 1, N]])
        nc.vector.tensor_scalar(diag1, diag1, 1.0, None, Alu.add)

        invD = lin.tile([B, N], F32, tag="invD")
        SCR = lin.tile([B, N * N], F32, tag="SCR")
        emit_ldlt(nc, Tm, SCR, invD, B)
        LT = lin.tile([B, N * N], F32, tag="LT")
        nc.vector.tensor_tensor(
            LT.rearrange("p (a b) -> p a b", b=N),
            Tm.rearrange("p (a b) -> p a b", b=N),
            invD.unsqueeze(1).broadcast_to([B, N, N]), Alu.mult)
        # dz: solve Lt^T dz = epss by backward substitution, one STT per
        # column: stored[0:k] <- (Ltrow_k * stored[k]) - stored[0:k], which
        # leaves stored[j] = (-1)^(N-1-j) dz[j]; fixed up with sgn_sb.
        srD = lin.tile([B, N], F32, tag="srD")
        nc.scalar.activation(srD, invD, Act.Sqrt)        # 1/sqrt(D)
        dz = lin.tile([B, N], F32, tag="dz")
        nc.vector.tensor_tensor(dz, eps_sb, srD, Alu.mult)
        for k in range(N - 1, 0, -1):
            lrow = sub_ap(LT, k * N, [[1, k]])
            nc.vector.scalar_tensor_tensor(
                dz[:, 0:k], lrow, dz[:, k:k + 1], dz[:, 0:k],
                Alu.mult, Alu.subtract)
        zs = lin.tile([B, N], F32, tag="zs")
        nc.vector.tensor_tensor(dz, dz, sgn_sb, Alu.mult)
        nc.vector.tensor_tensor(zs, zall, dz, Alu.add)
        s2 = emit_sig(zs, "s2")

        # ================= stage 2 ==========================================
        zsT_sb = work.tile([N, B], F32, tag="zsT")
        with tc.tile_pool(name="p_zst", bufs=1, space="PSUM") as zst_pool:
            zsT_ps = zst_pool.tile([N, B], F32, tag="zsT_ps")
            pe_T(nc, zsT_ps, zs, identf)
            nc.vector.tensor_copy(zsT_sb, zsT_ps)

        m2T_bf = work.tile([P, KT_H, B], BF16, tag="m2T")
        h2neg = work.tile([P, 2, B], BF16, tag="h2neg")
        with tc.tile_pool(name="p_a2t", bufs=1, space="PSUM") as a2t_pool:
            a2T_ps = a2t_pool.tile([P, KT_H, B], F32, tag="a2T")
            for mt in range(KT_H):
                nc.tensor.matmul(a2T_ps[:, mt, :],
                                 w1_sb[:, mt * P:(mt + 1) * P], zsT_sb,
                                 start=True, stop=False)
                nc.tensor.matmul(a2T_ps[:, mt, :],
                                 b1d_sb[:, mt * P:(mt + 1) * P], ones1[:, 0:B],
                                 start=False, stop=True)
            nc.vector.tensor_scalar(m2T_bf, a2T_ps, 0.0, None, Alu.is_gt)
            # -relu(a2) for local block (first 2 kt): min(-a2, 0)
            nc.vector.scalar_tensor_tensor(
                h2neg, a2T_ps[:, 0:2, :], -1.0, zeros2, Alu.mult, Alu.min)

        h2_sb = work.tile([B, H], BF16, tag="h2")
        with tc.tile_pool(name="p_a2", bufs=1, space="PSUM") as a2_pool:
            a2_ps = a2_pool.tile([B, H], F32, tag="a2")
            for nb in range(4):
                nc.tensor.matmul(a2_ps[:, nb * 512:(nb + 1) * 512],
                                 zsT_sb, w1_sb[:, nb * 512:(nb + 1) * 512],
                                 start=True, stop=False)
                nc.tensor.matmul(a2_ps[:, nb * 512:(nb + 1) * 512],
                                 ones1[:, 0:B], b1d_sb[:, nb * 512:(nb + 1) * 512],
                                 start=False, stop=True)
            nc.vector.tensor_scalar(h2_sb, a2_ps, 0.0, None, Alu.max)

        uh2 = lin.tile([B, 1], F32, tag="uh2")
        nc.vector.tensor_tensor(scr2, u_sb, h2_sb, Alu.mult)
        nc.vector.tensor_reduce(uh2, scr2, mybir.AxisListType.X, Alu.add)

        wd_sb = work.tile([B, H], BF16, tag="wd")
        vneg_bf = work.tile([B, H], BF16, tag="vneg")
        vh2m = lin.tile([B, 1], F32, tag="vh2m")
        with tc.tile_pool(name="p_v", bufs=1, space="PSUM") as v_pool:
            v_ps = v_pool.tile([B, H], F32, tag="v_ps")   # holds -v
            for nb in range(4):
                for it in range(2):
                    nc.tensor.matmul(v_ps[:, nb * 512:(nb + 1) * 512],
                                     h2neg[:, it, :],
                                     c_bf[:, it, nb * 512:(nb + 1) * 512],
                                     start=(it == 0), stop=(it == 1))
            nc.scalar.copy(vneg_bf, v_ps)
        nc.vector.tensor_tensor(scr2, vneg_bf, h2_sb, Alu.mult)
        nc.vector.tensor_reduce(vh2m, scr2, mybir.AxisListType.X, Alu.add)
        nc.vector.tensor_tensor(wd_sb, u_sb, vneg_bf, Alu.add)
        svec = lin.tile([B, 1], F32, tag="svec")
        nc.vector.scalar_tensor_tensor(svec, uh2, -2.0, xmbsq, Alu.mult, Alu.add)
        nc.vector.tensor_tensor(svec, svec, vh2m, Alu.subtract)

        # t_k = A2 wd_k : transpose wd, mask, matmul
        wdT_sb = work.tile([P, KT_H, B], BF16, tag="wdT")
        with tc.tile_pool(name="p_wdt", bufs=2, space="PSUM") as wdt_pool:
            for jt in range(KT_H):
                tp = wdt_pool.tile([P, B], BF16, tag="wdt")
                pe_T(nc, tp, wd_sb[:, jt * P:(jt + 1) * P], identb)
                nc.scalar.copy(wdT_sb[:, jt, :], tp)
        mwdT = work.tile([P, KT_H, B], BF16, tag="mwdT")
        nc.vector.tensor_tensor(mwdT, wdT_sb, m2T_bf, Alu.mult)
        tk_sb = work.tile([N, B], F32, tag="tk")
        with tc.tile_pool(name="p_tk", bufs=1, space="PSUM") as tk_pool:
            tk_ps = tk_pool.tile([N, B], F32, tag="tk_ps")
            for jt in range(KT_H):
                nc.tensor.matmul(tk_ps, w1Tb_sb[:, jt, :], mwdT[:, jt, :],
                                 start=(jt == 0), stop=(jt == KT_H - 1))
            nc.vector.tensor_copy(tk_sb, tk_ps)

        # ---- G2 + pack [t | svec | G2] -> AllReduce ----
        g2_sb = emit_G(m2T_bf, "2")
        nc.sync.dma_start(dram_ap(pkd, 0, [[1, N], [PKW, B]]), tk_sb)
        nc.sync.dma_start(dram_ap(pkd, N, [[PKW, B]]), svec)
        nc.sync.dma_start(
            dram_ap(pkd, N + 1, [[N, N], [PKW, B], [1, N]]), g2_sb)
        nc.gpsimd.collective_compute("AllReduce", Alu.add, replica_groups=RG,
                                     ins=[pkd[:]], outs=[pk_sh[:]])

        # ---- background (overlaps AllReduce): ltinv, tr inputs ----
        X1 = lin.tile([B, N * N], F32, tag="X1")
        nc.vector.memset(X1, 0.0)
        nc.vector.memset(sub_ap(X1, 0, [[N + 1, N]]), 1.0)
        emit_ltinv(nc, LT, X1, SCR, B)
        scrB = lin.tile([B, N * N], F32, tag="scrB")
        nc.vector.tensor_tensor(SCR, X1, X1, Alu.mult)
        trv = lin.tile([B, 1], F32, tag="trv")
        nc.vector.tensor_tensor(
            scrB.rearrange("p (a b) -> p a b", b=N),
            SCR.rearrange("p (a b) -> p a b", b=N),
            invD.unsqueeze(2).broadcast_to([B, N, N]), Alu.mult)
        logs = lin.tile([B, N], F32, tag="logs")
        nc.scalar.activation(logs, invD, Act.Ln)
        zsq = lin.tile([B, N], F32, tag="zsq")
        nc.vector.tensor_tensor(zsq, zall, zall, Alu.mult)

        # ---- post-AllReduce: solve G2 y = t, d_proj, recon, out ----
        y = lin.tile([B, N], F32, tag="y")
        nc.sync.dma_start(y, pk_sh[:, 0:N])
        svf = lin.tile([B, 1], F32, tag="svf")
        nc.sync.dma_start(svf, pk_sh[:, N:N + 1])
        Tm2 = lin.tile([B, N * N], F32, tag="Tm2")
        nc.sync.dma_start(Tm2, pk_sh[:, N + 1:PKW])
        invD2 = lin.tile([B, N], F32, tag="invD2")
        emit_ldlt(nc, Tm2, SCR, invD2, B)
        LT2 = lin.tile([B, N * N], F32, tag="LT2")
        nc.vector.tensor_tensor(
            LT2.rearrange("p (a b) -> p a b", b=N),
            Tm2.rearrange("p (a b) -> p a b", b=N),
            invD2.unsqueeze(1).broadcast_to([B, N, N]), Alu.mult)
        emit_fwd_solve_alt(nc, LT2, y, B)
        ysq = lin.tile([B, N], F32, tag="ysq")
        yw = lin.tile([B, N], F32, tag="yw")
        dproj = lin.tile([B, 1], F32, tag="dproj")
        nc.vector.tensor_tensor(ysq, y, y, Alu.mult)
        nc.vector.tensor_tensor(yw, ysq, invD2, Alu.mult)
        nc.vector.tensor_reduce(dproj, yw, mybir.AxisListType.X, Alu.add)

        ldv = lin.tile([B, 1], F32, tag="ldv")
        latv = lin.tile([B, 1], F32, tag="latv")
        nc.vector.tensor_reduce(trv, scrB, mybir.AxisListType.X, Alu.add)
        nc.vector.tensor_reduce(ldv, logs, mybir.AxisListType.X, Alu.add)
        nc.vector.tensor_reduce(latv, zsq, mybir.AxisListType.X, Alu.add)
        nc.vector.tensor_tensor(latv, latv, trv, Alu.add)
        nc.vector.tensor_scalar(latv, latv, 0.5, None, Alu.mult)
        nc.vector.tensor_scalar(ldv, ldv, -0.5, None, Alu.mult)

        sq2 = lin.tile([B, 2], F32, tag="sq2")
        nc.vector.tensor_tensor(sq2, s2, s2, Alu.mult)
        nc.vector.tensor_scalar(sq2, sq2, 2.0, None, Alu.mult)
        inv2 = lin.tile([B, 2], F32, tag="inv2")
        nc.vector.reciprocal(inv2, sq2)     # [1/(2sp2^2), 1/(2sv2^2)]
        logs2 = lin.tile([B, 2], F32, tag="logs2")
        logw = lin.tile([B, 2], F32, tag="logw")
        nc.vector.memset(logw[:, 0:1], float(N))
        nc.vector.memset(logw[:, 1:2], float(D - N))
        nc.scalar.activation(logs2, s2, Act.Ln)
        logterm = lin.tile([B, 1], F32, tag="logterm")
        junk2 = lin.tile([B, 2], F32, tag="junk2")
        nc.vector.tensor_tensor(junk2, logs2, logw, Alu.mult)
        nc.vector.tensor_reduce(logterm, junk2, mybir.AxisListType.X, Alu.add)
        isub = lin.tile([B, 1], F32, tag="isub")
        nc.vector.tensor_tensor(isub, inv2[:, 0:1], inv2[:, 1:2], Alu.subtract)
        recon = lin.tile([B, 1], F32, tag="recon")
        nc.vector.tensor_tensor(recon, dproj, isub, Alu.mult)
        p2t = lin.tile([B, 1], F32, tag="p2t")
        nc.vector.tensor_tensor(p2t, svf, inv2[:, 1:2], Alu.mult)
        nc.vector.tensor_tensor(recon, recon, p2t, Alu.add)
        nc.vector.tensor_tensor(recon, recon, logterm, Alu.add)
        ov = lin.tile([B, 1], F32, tag="ov")
        nc.vector.tensor_tensor(ov, recon, latv, Alu.add)
        nc.vector.tensor_tensor(ov, ov, ldv, Alu.add)
        nc.vector.tensor_scalar(ov, ov, 1.0 / D, None, Alu.mult)
        nc.sync.dma_start(out[:], ov)

    legalize_waits(nc)
    return nc


def shard_inputs(inputs):
    """Host-side prep: per-core H-permutation + D-tile reordering."""
    bf = ml_dtypes.bfloat16
    x = np.ascontiguousarray(np.asarray(inputs["x"], np.float32))
    eps = np.ascontiguousarray(np.asarray(inputs["eps"], np.float32))
    eW1 = np.asarray(inputs["enc_W1"], np.float32)
    eb1 = np.asarray(inputs["enc_b1"], np.float32)
    eW2 = np.asarray(inputs["enc_W2"], np.float32)
    eb2 = np.asarray(inputs["enc_b2"], np.float32)
    dW1 = np.asarray(inputs["dec_W1"], np.float32)
    db1 = np.asarray(inputs["dec_b1"], np.float32)
    dW2 = np.asarray(inputs["dec_W2"], np.float32)
    db2 = np.asarray(inputs["dec_b2"], np.float32)
    sW = np.asarray(inputs["sig_W"], np.float32)
    sb = np.asarray(inputs["sig_b"], np.float32)

    xT_bf = np.ascontiguousarray(x.T).astype(bf)
    xmb_full = x - db2[None, :]
    W2T = np.ascontiguousarray(dW2.T)      # [D, H]
    sigv = np.zeros((1, 130), np.float32)
    sigv[0, 0:32] = sW[:, 0]
    sigv[0, 32:64] = sW[:, 1]
    sigv[0, 64:66] = sb
    sigv[0, 66:98] = sW[:, 0] * np.sqrt(N / 2.0)
    sigv[0, 98:130] = sW[:, 1] * np.sqrt((D - N) / 2.0)

    maps = []
    for k in range(NCORES):
        hperm = np.concatenate([np.arange(k * HS, (k + 1) * HS),
                                np.arange(0, k * HS),
                                np.arange((k + 1) * HS, H)])
        # D-tile order: own 12 tiles first
        own = np.arange(k * KT_DS, (k + 1) * KT_DS)
        rest = np.concatenate([np.arange(0, k * KT_DS),
                               np.arange((k + 1) * KT_DS, KT_D)])
        tord = np.concatenate([own, rest])
        w2t_p = W2T[:, hperm].reshape(KT_D, P, H)[tord].reshape(D, H)
        w2q = np.asarray(w2t_p * 256.0, np.float32).astype(ml_dtypes.float8_e4m3)
        w2q = np.ascontiguousarray(
            w2q.reshape(KT_D // 2, 2, P, H).transpose(0, 2, 1, 3)
            .reshape(D // 2, 2 * H))
        dsl = slice(k * DS, (k + 1) * DS)
        xq = np.asarray(xmb_full[:, dsl].T, np.float32).astype(
            ml_dtypes.float8_e4m3)
        xq = np.ascontiguousarray(
            xq.reshape(KT_DS // 2, 2, P, B).transpose(0, 2, 1, 3)
            .reshape(DS // 2, 2 * B))
        maps.append({
            "w2t8": w2q,
            "xt": xT_bf,
            "w1es": np.ascontiguousarray(
                eW1[:, k * HS:(k + 1) * HS]).astype(bf),
            "b1es": np.ascontiguousarray(eb1[None, k * HS:(k + 1) * HS]),
            "w2es": np.ascontiguousarray(eW2[k * HS:(k + 1) * HS, :]),
            "b2e": np.ascontiguousarray(eb2[None, :]),
            "w1p": np.ascontiguousarray(dW1[:, hperm]),
            "w1tp_bf": np.ascontiguousarray(dW1[:, hperm].T).astype(bf),
            "w1tp8": np.ascontiguousarray(dW1[:, hperm].T * 64.0).astype(
                ml_dtypes.float8_e4m3),
            "b1dp": np.ascontiguousarray(db1[None, hperm]),
            "xmbt8": xq,
            "xmb": np.ascontiguousarray(xmb_full[:, dsl]),
            "sigw": sigv,
            "epsin": eps,
        })
    return maps


_NC_CACHE = None


def kernel(**inputs) -> np.ndarray:
    global _NC_CACHE
    from concourse.bass_utils import run_bass_kernel_spmd
    if _NC_CACHE is None:
        _NC_CACHE = build_nc()
    nc = _NC_CACHE
    maps = shard_inputs(inputs)
    res = run_bass_kernel_spmd(nc, maps, list(range(NCORES)))
    return np.asarray(res.results[0]["out"]).reshape(B).astype(np.float32)
